# revision 1
# baseline (speedup 1.0000x reference)
"""Multi-scale deformable attention on 8 Trainium2 NeuronCores.

Sharding: (batch x query-quarter) -> 8 cores; each core does all 8 heads for
2048 queries of one batch (value projection recomputed per core).

Per-core pipeline (v2):
  1. Host passes pre-transposed qT/refT (fp32) and vT (bf16); no on-device
     transposes of inputs.
  2. v-projection on PE -> vsb staging -> per-head bf16 quad maps in DRAM:
     map row g = the 4 bilinear corner cells (32ch each) of anchor g, so one
     256B dma_gather row fetches all 4 corners of one sample.
  3. Phase 3 computes positions / corner weights / softmax in a
     [(h,l,k) x query] layout; indices are wrapped for the gather via narrow
     PE transposes written straight into idxw partitions 0..15 (replicated
     16->128 by doubling DMAs); corner weights are repacked once into
     w4all[s*32+ci, m*2048+q] = w4p[s][32m+ci, q] with 16 large DMAs.
  4. 32 chunks (head x query-quarter): dma_gather (Pool) -> G[128=(s,ch),
     8192]; a one-hot sel32 matmul broadcasts w4all rows {s*32+ci} into
     psW; Act copies psW->SBUF bf16 so DVE multiplies G*W at 2x rate (7 of
     8 j-tiles; 1 read PSUM directly); PE reduce-matmul accumulates over
     corners and (m, qt) into psO4.
  5. Per-head output projection (scrambled per the reference reshape quirk,
     bf16) interleaved with remaining chunks; host assembles the overlap.

Engine budget per chunk ~= Act 7.6us (psW->wc copies), Pool 7.1 (gather),
DVE 6.3 (G*W + osb), PE 6.1 (broadcast + reduce + out-proj).
"""
import sys

sys.path.insert(0, '/opt/trn_rl_repo')

import numpy as np
import ml_dtypes

import concourse.bass as bass
import concourse.bacc as bacc
import concourse.mybir as mybir
import concourse.tile as tile
from concourse.bass_utils import run_bass_kernel_spmd

dt = mybir.dt
F32, BF16, I16, I32 = dt.float32, dt.bfloat16, dt.int16, dt.int32
ALU = mybir.AluOpType
ACTF = mybir.ActivationFunctionType
BF = ml_dtypes.bfloat16

# ---------------------------------------------------------------- geometry
LEVELS = ((76, 76), (38, 38), (19, 19), (10, 10))
NUM_HEADS, NUM_LEVELS, NUM_POINTS = 8, 4, 4
C, D = 256, 32
BS, NQ = 2, 7681
QP = 2048                      # queries per core (padded)
NCORES = 8
SAMP = NUM_HEADS * NUM_LEVELS * NUM_POINTS * QP       # 262144
STARTS = [0]
for (_h, _w) in LEVELS:
    STARTS.append(STARTS[-1] + _h * _w)
NV = STARTS[-1]                # 7681
NVPAD = 7808                   # 61 * 128
GAP = 256
CELLSTART = []
_pos = GAP
for (_h, _w) in LEVELS:
    CELLSTART.append(_pos)
    _pos += _h * _w + GAP
ES = ((_pos - GAP) // 128 + 1) * 128      # quad-map rows
TS = ES // 128 + 1                        # vsb stream blocks (+1 zero blk)
# disjoint 128-aligned map-write ranges: split between level l's last valid
# anchor (cellend_l) and level l+1's first anchor (cellstart - W - 1)
SPLITS = [0]
for _l in range(3):
    _lo = CELLSTART[_l] + LEVELS[_l][0] * LEVELS[_l][1]
    _hi = CELLSTART[_l + 1] - LEVELS[_l + 1][1] - 1
    _cand = (_hi // 128) * 128
    assert _lo <= _cand <= _hi, (_l, _lo, _cand, _hi)
    SPLITS.append(_cand)
SPLITS.append(ES)
VSB_HT = TS * 32
CHUNK = 8192                   # gather chunk = 4 m-blocks x 2048 q
NCHUNK = SAMP // CHUNK         # 32


def align_down(x, a=128):
    return (x // a) * a


def mkap(base_ap, ap_list, offset=None):
    ap = base_ap.copy()
    ap.ap = mybir.VecI64Pair([list(x) for x in ap_list])
    if offset is not None:
        ap.offset = offset
    return ap


# ---------------------------------------------------------------- program
def build_nc():
    nc = bacc.Bacc("TRN2", target_bir_lowering=False)
    qT_d = nc.dram_tensor("qT", [128, 2, QP], F32, kind="ExternalInput")
    refT_d = nc.dram_tensor("refT", [2, QP], F32, kind="ExternalInput")
    vT_d = nc.dram_tensor("vT", [128, 2, NVPAD], BF16, kind="ExternalInput")
    woffx_d = nc.dram_tensor("woffx", [C, 128], F32, kind="ExternalInput")
    woffy_d = nc.dram_tensor("woffy", [C, 128], F32, kind="ExternalInput")
    wattn_d = nc.dram_tensor("wattn", [C, 128], F32, kind="ExternalInput")
    boffx_d = nc.dram_tensor("boffx", [1, 128], F32, kind="ExternalInput")
    boffy_d = nc.dram_tensor("boffy", [1, 128], F32, kind="ExternalInput")
    battn_d = nc.dram_tensor("battn", [1, 128], F32, kind="ExternalInput")
    wval_d = nc.dram_tensor("wval", [C, C], BF16, kind="ExternalInput")
    bval_d = nc.dram_tensor("bval", [1, C], BF16, kind="ExternalInput")
    wout_d = nc.dram_tensor("wout", [128, 8, C], BF16,
                           kind="ExternalInput")
    selx_d = nc.dram_tensor("selx", [2, 128], F32, kind="ExternalInput")
    sely_d = nc.dram_tensor("sely", [2, 128], F32, kind="ExternalInput")
    onesq_d = nc.dram_tensor("onesq", [1, QP], F32, kind="ExternalInput")
    onesbf_d = nc.dram_tensor("onesbf", [1, 128], BF16,
                              kind="ExternalInput")
    r128_d = nc.dram_tensor("r128", [128, 32], BF16, kind="ExternalInput")
    sel32_d = nc.dram_tensor("sel32", [128, 32, 128], BF16,
                             kind="ExternalInput")
    s16_d = nc.dram_tensor("s16", [128, 8], F32, kind="ExternalInput")
    b8_d = nc.dram_tensor("b8", [8, 128], F32, kind="ExternalInput")
    sclw_d = nc.dram_tensor("sclw", [128, 1], F32, kind="ExternalInput")
    sclh_d = nc.dram_tensor("sclh", [128, 1], F32, kind="ExternalInput")
    wlm1_d = nc.dram_tensor("wlm1", [128, 1], F32, kind="ExternalInput")
    hlm1_d = nc.dram_tensor("hlm1", [128, 1], F32, kind="ExternalInput")
    cbase_d = nc.dram_tensor("cbase", [128, 1], F32, kind="ExternalInput")
    qmask_d = nc.dram_tensor("qmask", [128, QP], BF16, kind="ExternalInput")
    iden_d = nc.dram_tensor("iden", [128, 128], F32, kind="ExternalInput")
    out_d = nc.dram_tensor("out", [8, 3, 128, C], F32,
                           kind="ExternalOutput")
    maps_d = [nc.dram_tensor(f"map{h}", [ES, 128], BF16)
              for h in range(NUM_HEADS)]

    with tile.TileContext(nc) as tc:
        with tc.tile_pool(name="const", bufs=1) as cpool:
            selx = cpool.tile([2, 128], F32)
            sely = cpool.tile([2, 128], F32)
            onesq = cpool.tile([1, QP], F32)
            onesbf = cpool.tile([1, 128], BF16)
            r128 = cpool.tile([128, 32], BF16)
            sel32 = cpool.tile([128, 32, 128], BF16)
            s16 = cpool.tile([128, 8], F32)
            b8c = cpool.tile([8, 128], F32)
            sclw = cpool.tile([128, 1], F32)
            sclh = cpool.tile([128, 1], F32)
            wlm1 = cpool.tile([128, 1], F32)
            hlm1 = cpool.tile([128, 1], F32)
            cbase = cpool.tile([128, 1], F32)
            woffx = cpool.tile([128, 2, 128], F32)
            woffy = cpool.tile([128, 2, 128], F32)
            wattn = cpool.tile([128, 2, 128], F32)
            boffx = cpool.tile([1, 128], F32)
            boffy = cpool.tile([1, 128], F32)
            battn = cpool.tile([1, 128], F32)
            wvalb = cpool.tile([128, 2, C], BF16)
            bvalb = cpool.tile([1, C], BF16)
            woutc = cpool.tile([128, 8, C], BF16)
            qmask = cpool.tile([128, QP], BF16)
            iden = cpool.tile([128, 128], F32)
            qT = cpool.tile([128, 2, QP], F32)
            refT = cpool.tile([2, QP], F32)
            nc.sync.dma_start(qT[:], qT_d[:])
            nc.sync.dma_start(refT[:], refT_d[:])
            for t, s in [(woffx, woffx_d), (woffy, woffy_d)]:
                nc.scalar.dma_start(
                    t[:], s[:].rearrange("(h p) x -> p h x", p=128))
            for t, s in [(onesq, onesq_d), (boffx, boffx_d),
                         (boffy, boffy_d), (selx, selx_d),
                         (sely, sely_d), (sclw, sclw_d), (sclh, sclh_d),
                         (wlm1, wlm1_d), (hlm1, hlm1_d),
                         (cbase, cbase_d), (iden, iden_d),
                         (qmask, qmask_d)]:
                nc.scalar.dma_start(t[:], s[:])
            for t, s in [(wattn, wattn_d), (wvalb, wval_d)]:
                nc.scalar.dma_start(
                    t[:], s[:].rearrange("(h p) x -> p h x", p=128))
            for t, s in [(battn, battn_d), (bvalb, bval_d),
                         (onesbf, onesbf_d), (s16, s16_d), (b8c, b8_d),
                         (r128, r128_d), (sel32, sel32_d),
                         (woutc, wout_d)]:
                nc.scalar.dma_start(t[:], s[:])

            HQ = QP // 2
            ps3 = tc.alloc_tile_pool(name="ps3", bufs=2, space="PSUM")

            def projf(hq, wof, bof, sel):
                ps = ps3.tile([128, HQ], F32, tag="pp")
                for c0 in range(0, HQ, 512):
                    cs = slice(hq * HQ + c0, hq * HQ + c0 + 512)
                    po = ps[:, c0:c0 + 512]
                    for half in range(2):
                        nc.tensor.matmul(
                            po, wof[:, half, :], qT[:, half, cs],
                            start=(half == 0), stop=False)
                    nc.tensor.matmul(po, bof[:], onesq[:, cs],
                                     start=False, stop=(sel is None))
                    if sel is not None:
                        nc.tensor.matmul(po, sel[:], refT[:, cs],
                                         start=False, stop=True)
                return ps

            # hq0 x/y projections run on PE before anything else
            psX0 = projf(0, woffx, boffx, selx)
            psY0 = projf(0, woffy, boffy, sely)

            # ======== phase 2: vT load, v-proj, quad maps ================
            pVS = tc.alloc_tile_pool(name="pVS", bufs=1, side="right")
            if True:
                vsb = pVS.tile([128, NUM_HEADS, TS, 32], BF16)
                # zero only the gap/pad blocks; v-proj copies fill the rest
                gapblks = [(0, 2), (47, 50), (60, 63), (65, TS)]
                for b0, b1 in gapblks:
                    nc.vector.memset(
                        mkap(vsb[:], [[NUM_HEADS * VSB_HT, 128],
                                      [VSB_HT, NUM_HEADS],
                                      [1, 32 * (b1 - b0)]],
                             offset=b0 * 32), 0.0)
                pVT = tc.alloc_tile_pool(name="pVT", bufs=1, side="right")
                p1t = tc.alloc_tile_pool(name="p1t", bufs=2)
                ps1 = tc.alloc_tile_pool(name="ps1", bufs=3, space="PSUM")
                vT = pVT.tile([128, 2, NVPAD], BF16)
                nc.gpsimd.dma_start(vT[:], vT_d[:])
                for lvl in range(NUM_LEVELS):
                    hw = LEVELS[lvl][0] * LEVELS[lvl][1]
                    shift = CELLSTART[lvl] - STARTS[lvl]  # mult of 128
                    c0 = STARTS[lvl]
                    while c0 < STARTS[lvl] + hw:
                        tbeg = align_down(c0)
                        cend = min(tbeg + 128, STARTS[lvl] + hw)
                        lo, hi = c0 - tbeg, cend - tbeg
                        psv = ps1.tile([128, C], F32, tag="psv")
                        for half in range(2):
                            nc.tensor.matmul(
                                psv[:], vT[:, half, tbeg:tbeg + 128],
                                wvalb[:, half, :], start=(half == 0),
                                stop=False)
                        nc.tensor.matmul(psv[:], onesbf[:], bvalb[:],
                                         start=False, stop=True)
                        sp = c0 + shift
                        assert sp % 128 == lo
                        dst = mkap(vsb[:],
                                   [[NUM_HEADS * VSB_HT, hi - lo],
                                    [VSB_HT, NUM_HEADS], [1, 32]],
                                   offset=lo * (NUM_HEADS * VSB_HT)
                                   + (sp // 128) * 32)
                        if lo == 0 and hi == 128:
                            src = mkap(psv[:],
                                       [[C, 128], [32, NUM_HEADS],
                                        [1, 32]])
                            if (c0 // 128) % 2 == 0:
                                nc.scalar.copy(dst, src)
                            else:
                                nc.vector.tensor_copy(dst, src)
                        else:
                            vstage = p1t.tile([128, C], BF16,
                                              tag="vstage")
                            nc.scalar.copy(vstage[:], psv[:])
                            src = mkap(vstage[:],
                                       [[C, hi - lo], [32, NUM_HEADS],
                                        [1, 32]], offset=lo * C)
                            nc.sync.dma_start(dst, src)
                        c0 = cend
                ps1.release()
                p1t.release()
                pVT.release()

            # ---- quad-map writes, head-major so gathers start early;
            # heads 4-7 are emitted after phase 3 so Act's queue stays
            # clear for softmax (their gathers happen much later)
            engs = [nc.sync, nc.scalar]

            def write_maps(h0, h1, only_eng=None, nmap=[0]):
                for hh in range(h0, h1):
                    for lvl, (H, W) in enumerate(LEVELS):
                        g0 = SPLITS[lvl]
                        g1 = SPLITS[lvl + 1]
                        ng = (g1 - g0) // 128
                        for s, dl in enumerate([0, 1, W, W + 1]):
                            p0 = (g0 + dl) % 128
                            tb0 = (g0 + dl) // 128
                            eng = only_eng or engs[nmap[0] % 2]
                            nmap[0] += 1
                            outA = mkap(
                                maps_d[hh][:],
                                [[128, 128 - p0], [128 * 128, ng],
                                 [1, 32]],
                                offset=g0 * 128 + s * 32)
                            inA = mkap(
                                vsb[:],
                                [[NUM_HEADS * VSB_HT, 128 - p0],
                                 [32, ng], [1, 32]],
                                offset=p0 * (NUM_HEADS * VSB_HT)
                                + hh * VSB_HT + tb0 * 32)
                            eng.dma_start(outA, inA)
                            if p0:
                                outB = mkap(
                                    maps_d[hh][:],
                                    [[128, p0], [128 * 128, ng],
                                     [1, 32]],
                                    offset=(g0 + 128 - p0) * 128 + s * 32)
                                inB = mkap(
                                    vsb[:],
                                    [[NUM_HEADS * VSB_HT, p0],
                                     [32, ng], [1, 32]],
                                    offset=hh * VSB_HT + (tb0 + 1) * 32)
                                eng.dma_start(outB, inB)

            write_maps(0, 4)


            # ======== phase 3: positions / weights / indices ==========
            pw = tc.alloc_tile_pool(name="pw", bufs=1)
            idxw = pw.tile([128, SAMP // 16], I16)
            w4all = pw.tile([128, CHUNK], BF16)
            with tc.tile_pool(name="p3", bufs=1) as p3, \
                 tc.tile_pool(name="ps3s", bufs=1,
                              space="PSUM") as ps3s, \
                 tc.tile_pool(name="psQ", bufs=2, space="PSUM") as psQp:
                w4p = [p3.tile([128, QP], BF16, name=f"w4p{s}",
                               tag=f"w4p{s}") for s in range(4)]
                for hq in range(2):
                    qs = slice(hq * HQ, (hq + 1) * HQ)

                    def corner_weights(psP, scl, wm1, pref):
                        t1 = p3.tile([128, HQ], F32, tag="cwt1")
                        xpp = p3.tile([128, HQ], F32, tag="cwxpp")
                        xi = p3.tile([128, HQ], I32, tag="cwxi")
                        x0p = p3.tile([128, HQ], F32,
                                      tag=f"{pref}x0p")
                        wx = p3.tile([128, HQ], F32, tag="cwwx")
                        v0 = p3.tile([128, HQ], F32, tag="cwv0")
                        v1 = p3.tile([128, HQ], F32, tag="cwv1")
                        a0 = p3.tile([128, HQ], F32, tag=f"{pref}a0")
                        a1 = p3.tile([128, HQ], F32, tag=f"{pref}a1")
                        nc.vector.tensor_scalar(
                            t1[:], psP[:], 1.0, 0.0,
                            op0=ALU.min, op1=ALU.max)
                        nc.vector.tensor_scalar(
                            xpp[:], t1[:], scl[:], 0.5,
                            op0=ALU.mult, op1=ALU.add)
                        nc.vector.tensor_copy(xi[:], xpp[:])
                        nc.vector.tensor_copy(x0p[:], xi[:])
                        # trunc/round-agnostic floor: subtract 1 where
                        # the int cast rounded up
                        nc.vector.tensor_tensor(t1[:], x0p[:], xpp[:],
                                                op=ALU.is_gt)
                        nc.vector.tensor_sub(x0p[:], x0p[:], t1[:])
                        nc.vector.tensor_sub(wx[:], xpp[:], x0p[:])
                        nc.vector.tensor_scalar(
                            v0[:], x0p[:], 1.0, 0.0,
                            op0=ALU.is_ge, op1=ALU.bypass)
                        nc.vector.tensor_scalar(
                            v1[:], x0p[:], wm1[:], 0.0,
                            op0=ALU.is_le, op1=ALU.bypass)
                        nc.vector.scalar_tensor_tensor(
                            a1[:], wx[:], 1.0, v1[:],
                            op0=ALU.mult, op1=ALU.mult)
                        nc.vector.tensor_scalar(
                            wx[:], wx[:], -1.0, 1.0,
                            op0=ALU.mult, op1=ALU.add)
                        nc.vector.tensor_tensor(
                            a0[:], wx[:], v0[:], op=ALU.mult)
                        return x0p, a0, a1

                    if hq == 0:
                        psX, psY = psX0, psY0
                    else:
                        psX = projf(hq, woffx, boffx, selx)
                        psY = projf(hq, woffy, boffy, sely)
                    x0p, ax0, ax1 = corner_weights(psX, sclw, wlm1,
                                                   "x")
                    y0p, ay0, ay1 = corner_weights(psY, sclh, hlm1,
                                                   "y")
                    idxf = p3.tile([128, HQ], F32, tag="cwxi")
                    nc.vector.scalar_tensor_tensor(
                        idxf[:], y0p[:], sclw[:], x0p[:],
                        op0=ALU.mult, op1=ALU.add)
                    nc.vector.tensor_scalar(
                        idxf[:], idxf[:], cbase[:], 0.0,
                        op0=ALU.add, op1=ALU.bypass)
                    # narrow transposes: idxf cols [16jj,16jj+16) ->
                    # psQ[q%16, c]; copies write idxw partitions 0..15:
                    # idxw[s, ci*512 + m*128 + jj] = idx(c=32m+ci, 16jj+s)
                    IW = SAMP // 16
                    for jj in range(HQ // 16):
                        jg = hq * (HQ // 16) + jj
                        psQ = psQp.tile([16, 128], F32, tag="psQ")
                        nc.tensor.transpose(
                            psQ[:], idxf[:, jj * 16:(jj + 1) * 16],
                            iden[:])
                        cp = nc.vector.tensor_copy
                        cp(mkap(idxw[:],
                                [[IW, 16], [128, 4], [512, 32]],
                                offset=jg),
                           mkap(psQ[:], [[128, 16], [32, 4], [1, 32]]))
                    psZ = projf(hq, wattn, battn, None)
                    esb = p3.tile([128, HQ], F32, tag="cwxpp")
                    nc.scalar.activation(esb[:], psZ[:], ACTF.Exp,
                                         bias=0.0, scale=1.0)
                    pss = ps3s.tile([8, HQ], F32, tag="pss")
                    for c0 in range(0, HQ, 512):
                        nc.tensor.matmul(pss[:, c0:c0 + 512], s16[:],
                                         esb[:, c0:c0 + 512],
                                         start=True, stop=True)
                    rsb = p3.tile([8, HQ], F32, tag="cwwx")
                    nc.vector.reciprocal(rsb[:], pss[:])
                    psr2 = ps3.tile([128, HQ], F32, tag="pp")
                    for c0 in range(0, HQ, 512):
                        nc.tensor.matmul(psr2[:, c0:c0 + 512], b8c[:],
                                         rsb[:, c0:c0 + 512],
                                         start=True, stop=True)
                    aw = p3.tile([128, HQ], F32, tag="cwv0")
                    nc.vector.tensor_tensor(aw[:], esb[:], psr2[:],
                                            op=ALU.mult)
                    nc.vector.tensor_tensor(aw[:], aw[:],
                                            qmask[:, qs], op=ALU.mult)
                    nc.vector.tensor_tensor(ay0[:], ay0[:], aw[:],
                                            op=ALU.mult)
                    nc.vector.tensor_tensor(ay1[:], ay1[:], aw[:],
                                            op=ALU.mult)
                    for s, (ax, ay) in enumerate(
                            [(ax0, ay0), (ax1, ay0),
                             (ax0, ay1), (ax1, ay1)]):
                        nc.gpsimd.tensor_tensor(
                            w4p[s][:, qs], ax[:], ay[:], op=ALU.mult)
                    # w4all[s*32+ci, m*2048 + hq-half] = w4p[s][32m+ci, .]
                    for s in range(4):
                        for m in range(4):
                            eng = nc.sync if (s * 4 + m) % 2 == 0 \
                                else nc.scalar
                            eng.dma_start(
                                mkap(w4all[:], [[CHUNK, 32], [1, HQ]],
                                     offset=s * 32 * CHUNK + m * QP
                                     + hq * HQ),
                                mkap(w4p[s][:], [[QP, 32], [1, HQ]],
                                     offset=32 * m * QP + hq * HQ))
                # replicate 16 -> 128 partitions by doubling
                IW = SAMP // 16
                for (r0, nr) in [(16, 16), (32, 32), (64, 64)]:
                    for half in range(2):
                        eng = nc.sync if half == 0 else nc.gpsimd
                        cs = half * (IW // 2)
                        eng.dma_start(
                            mkap(idxw[:], [[IW, nr], [1, IW // 2]],
                                 offset=r0 * IW + cs),
                            mkap(idxw[:], [[IW, nr], [1, IW // 2]],
                                 offset=(r0 - nr) * IW + cs))


            write_maps(4, NUM_HEADS)
            pVS.release()
            ps3.release()
            # ======== phase 4: gather / multiply / reduce =================
            import os as _os
            _kph = int(_os.environ.get("KPH", "9"))
            with tc.tile_pool(name="p4", bufs=2) as p4, \
                 tc.tile_pool(name="p4t", bufs=2) as p4t, \
                 tc.tile_pool(name="pwc", bufs=6) as pwc, \
                 tc.tile_pool(name="psW", bufs=2, space="PSUM") as psWp, \
                 tc.tile_pool(name="psO", bufs=1, space="PSUM") as psO, \
                 tc.tile_pool(name="pOs", bufs=1) as pOs, \
                 tc.tile_pool(name="pg", bufs=2) as pg, \
                 tc.tile_pool(name="pfo", bufs=3) as pfo:
                osb = [pOs.tile([128, QP + 2048], BF16, name=f"osb{g}",
                                tag=f"osb{g}")
                       for g in range(2)]
                for g in range(2):
                    nc.vector.memset(osb[g][:], 0.0)

                def phase5_head(m):
                    # reference reshape quirk: out row q column c takes
                    # O[m, qq, d] with u = m*7681 + qq = 8q + (c//32 slot),
                    # W_out row 32*((m+qq)%8) + d.  7681 % 8 == 1 makes the
                    # structure identical on every core (host assembles).
                    gsb = osb[m // 4]
                    grp = m % 4
                    dlt = 64 * (m % 2)
                    for j in range(3):
                        psF4 = psWp.tile([128, 1024], F32, tag="psW")
                        psF = psF4[:, 0:C]
                        for s in range(8):
                            q0 = -m - 8 * dlt + 1024 * j + s
                            col0 = 1024 + q0
                            assert 0 <= col0 and col0 + 8 * 127 < QP + 2048
                            lhsT = mkap(
                                gsb[:], [[QP + 2048, 32], [8, 128]],
                                offset=(grp * 32) * (QP + 2048) + col0)
                            kw = {}
                            if grp == 3:
                                kw["tile_position"] = (96, 0)
                            nc.tensor.matmul(
                                psF, lhsT,
                                woutc[grp * 32:grp * 32 + 32, s, :],
                                start=(s == 0), stop=(s == 7), **kw)
                        fo = pfo.tile([128, C], F32, tag="fo")
                        nc.vector.tensor_copy(fo[:], psF)
                        nc.gpsimd.dma_start(out_d[m, j], fo[:])

                psO4 = None
                for ci in range(NCHUNK if _kph >= 4 else 0):
                    hh, qt = ci // 4, ci % 4
                    grp = hh % 4
                    if grp == 0 and qt == 0:
                        psO4 = psO.tile([128, QP], F32, tag="psO4")
                    g = pg.tile([128, 1, CHUNK], BF16, tag="g")
                    nc.gpsimd.dma_gather(
                        g[:], maps_d[hh][:],
                        idxw[:, ci * (CHUNK // 16):
                             (ci + 1) * (CHUNK // 16)],
                        CHUNK, CHUNK, 128,
                        transpose=True, single_packet=False)
                    tt = p4t.tile([128, CHUNK], BF16, tag="tt")
                    lhsT = mkap(sel32[:], [[32 * 128, 128], [1, 128]],
                                offset=ci * 128)
                    for j in range(CHUNK // 1024):
                        psW = psWp.tile([128, 1024], F32, tag="psW")
                        for jj in range(2):
                            cs = slice(j * 1024 + jj * 512,
                                       j * 1024 + (jj + 1) * 512)
                            nc.tensor.matmul(psW[:, jj * 512:
                                                 (jj + 1) * 512],
                                             lhsT, w4all[:, cs],
                                             start=True, stop=True)
                        if j > 0:
                            wc = pwc.tile([128, 1024], BF16, tag="wc")
                            nc.scalar.copy(wc[:], psW[:])
                            nc.vector.tensor_tensor(
                                tt[:, j * 1024:(j + 1) * 1024],
                                g[:, 0, j * 1024:(j + 1) * 1024], wc[:],
                                op=ALU.mult)
                        else:
                            nc.vector.tensor_tensor(
                                tt[:, j * 1024:(j + 1) * 1024],
                                g[:, 0, j * 1024:(j + 1) * 1024], psW[:],
                                op=ALU.mult)
                    for m4 in range(4):
                        for j2 in range(4):
                            kw = {}
                            if grp == 3:
                                kw["tile_position"] = (0, 96)
                            nc.tensor.matmul(
                                psO4[grp * 32:(grp + 1) * 32,
                                     j2 * 512:(j2 + 1) * 512],
                                r128[:],
                                tt[:, m4 * QP + j2 * 512:
                                   m4 * QP + (j2 + 1) * 512],
                                start=(qt == 0 and m4 == 0),
                                stop=(qt == 3 and m4 == 3), **kw)
                    if qt == 3:
                        nc.vector.tensor_copy(
                            osb[hh // 4][grp * 32:(grp + 1) * 32,
                                         1024:1024 + QP],
                            psO4[grp * 32:(grp + 1) * 32, :])
                    if _kph >= 5 and ci >= 5 and (ci - 5) % 4 == 0:
                        phase5_head((ci - 5) // 4)

                # ======== phase 5 tail ====================================
                if _kph >= 5:
                    phase5_head(7)
                else:
                    foz = p4.tile([128, C], F32, tag="foz")
                    nc.vector.memset(foz[:], 0.0)
                    for m in range(NUM_HEADS):
                        for j in range(3):
                            nc.sync.dma_start(out_d[m, j], foz[:])
            pw.release()
    nc.compile()
    return nc


# ---------------------------------------------------------------- host side
_CACHE = {}


def _consts(W_off, b_off, W_attn, b_attn, W_val, b_val, W_out, b_out):
    M = NUM_HEADS
    # partition layout c = b*32 + h*4 + a  (old: h*16 + a*4 + b)
    woff = np.asarray(W_off, np.float32).reshape(C, M, 4, 4, 2)
    woff = np.transpose(woff, (0, 3, 1, 2, 4))          # (C, b, h, a, 2)
    wattn = np.asarray(W_attn, np.float32).reshape(C, M, 4, 4)
    # partition (b, h, a) holds attention logit (level=b, point=a) so that
    # sample (h, a, b) pairs with aw(level=b, point=a)  (reference quirk)
    wattn = np.transpose(wattn, (0, 2, 1, 3))           # (C, l, h, k)
    boff = np.asarray(b_off, np.float32).reshape(M, 4, 4, 2)
    boff = np.transpose(boff, (2, 0, 1, 3))             # (b, h, a, 2)
    battn = np.asarray(b_attn, np.float32).reshape(M, 4, 4)
    battn = np.transpose(battn, (1, 0, 2))              # (l, h, k)
    cm = {}
    cm["woffx"] = np.ascontiguousarray(woff[..., 0].reshape(C, 128))
    cm["woffy"] = np.ascontiguousarray(woff[..., 1].reshape(C, 128))
    cm["wattn"] = np.ascontiguousarray(wattn.reshape(C, 128))
    cm["boffx"] = np.ascontiguousarray(boff[..., 0].reshape(1, 128))
    cm["boffy"] = np.ascontiguousarray(boff[..., 1].reshape(1, 128))
    cm["battn"] = np.ascontiguousarray(battn.reshape(1, 128))
    cm["wval"] = np.asarray(W_val, np.float32).astype(BF)
    cm["bval"] = np.asarray(b_val, np.float32).reshape(1, C).astype(BF)
    wof = np.asarray(W_out, np.float32).reshape(8, 32, C).transpose(1, 0, 2)
    cm["wout"] = np.ascontiguousarray(
        np.broadcast_to(wof[None], (4, 32, 8, C)).reshape(128, 8, C)
    ).astype(BF)
    sel = np.zeros((2, 128), np.float32)
    sel[0] = 1.0
    cm["selx"] = sel
    cm["sely"] = sel[::-1].copy()
    cm["onesq"] = np.ones((1, QP), np.float32)
    cm["onesbf"] = np.ones((1, 128), np.float32).astype(BF)
    r = np.zeros((128, 32), np.float32)
    for p in range(128):
        r[p, p % 32] = 1.0
    cm["r128"] = r.astype(BF)
    s32 = np.zeros((128, 32, 128), np.float32)
    for ci in range(32):
        for p in range(128):
            s32[(p // 32) * 32 + ci, ci, p] = 1.0
    cm["sel32"] = s32.astype(BF)
    s16 = np.zeros((128, 8), np.float32)
    b8 = np.zeros((8, 128), np.float32)
    for p in range(128):
        h = (p % 32) // 4
        s16[p, h] = 1.0
        b8[h, p] = 1.0
    cm["s16"] = s16
    cm["b8"] = b8
    lvl_of_p = np.arange(128) % 4                       # level = a = c%4
    Wl = np.array([LEVELS[l][1] for l in lvl_of_p], np.float32)
    Hl = np.array([LEVELS[l][0] for l in lvl_of_p], np.float32)
    cb = np.array([CELLSTART[l] - LEVELS[l][1] - 1 for l in lvl_of_p],
                  np.float32)
    cm["iden"] = np.eye(128, dtype=np.float32)
    cm["sclw"] = Wl.reshape(128, 1)
    cm["sclh"] = Hl.reshape(128, 1)
    cm["wlm1"] = (Wl - 1).reshape(128, 1)
    cm["hlm1"] = (Hl - 1).reshape(128, 1)
    cm["cbase"] = cb.reshape(128, 1)
    return cm


def kernel(**inputs):
    if "nc" not in _CACHE:
        _CACHE["nc"] = build_nc()
    nc = _CACHE["nc"]
    cm = _consts(inputs["W_off"], inputs["b_off"], inputs["W_attn"],
                 inputs["b_attn"], inputs["W_val"], inputs["b_val"],
                 inputs["W_out"], inputs["b_out"])
    query = np.asarray(inputs["query"], np.float32)
    refp = np.asarray(inputs["reference_points"], np.float32)
    value = np.asarray(inputs["value"], np.float32)
    vpad = np.zeros((BS, NVPAD, C), np.float32)
    vpad[:, :NV] = value
    qpad = np.zeros((BS, 4 * QP, C), np.float32)
    qpad[:, :NQ] = query
    rpad = np.zeros((BS, 4 * QP, 2), np.float32)
    rpad[:, :NQ] = refp
    vT_b = []
    for b in range(BS):
        vT = vpad[b].T.reshape(2, 128, NVPAD).transpose(1, 0, 2)
        vT_b.append(np.ascontiguousarray(vT.astype(BF)))
    in_maps = []
    for core in range(NCORES):
        b, qc = core // 4, core % 4
        nvalid = min(QP, max(0, NQ - qc * QP))
        qm = np.zeros((128, QP), np.float32)
        qm[:, :nvalid] = 1.0
        qm = qm.astype(BF)
        qs = qpad[b, qc * QP:(qc + 1) * QP]
        rs = rpad[b, qc * QP:(qc + 1) * QP]
        qT = qs.T.reshape(2, 128, QP).transpose(1, 0, 2)
        m = {"qT": np.ascontiguousarray(qT),
             "refT": np.ascontiguousarray(rs.T),
             "vT": vT_b[b],
             "qmask": qm}
        m.update({k: np.ascontiguousarray(v) for k, v in cm.items()})
        in_maps.append(m)
    res = run_bass_kernel_spmd(nc, in_maps, list(range(NCORES)),
                               **_CACHE.get("run_kw", {}))
    _CACHE["last_res"] = res
    out = np.zeros((BS, NQ + 512, C), np.float32)
    for core in range(NCORES):
        b, qc = core // 4, core % 4
        slab = res.results[core]["out"]        # [8, 3, 128, 256]
        for m in range(NUM_HEADS):
            dlt = 64 * (m % 2)
            tb = 960 * m + 256 * qc - dlt      # absolute tile base
            for j in range(3):
                if m % 2 == 0:
                    row_lo, row_hi = 0, (128, 128, 32)[j]
                else:
                    row_lo, row_hi = ((64, 0, 0)[j], (128, 128, 96)[j])
                r0 = tb + 128 * j + row_lo
                r1 = tb + 128 * j + row_hi
                r1c = min(r1, NQ + 512)
                if r0 < 0 or r1c <= r0:
                    continue
                out[b, r0:r1c] += slab[m, j, row_lo:row_lo + (r1c - r0)]
    out = out[:, :NQ] + np.asarray(inputs["b_out"], np.float32)[None, None]
    return out



# revision 11
# speedup vs baseline: 1.1781x; 1.1781x over previous
"""Multi-scale deformable attention on 8 Trainium2 NeuronCores.

Sharding: (batch x query-quarter) -> 8 cores; each core does all 8 heads for
2048 queries of one batch (value projection recomputed per core).

v3 design (cost-model-aware):
  - v-proj -> vsb_full (cell-major bf16) -> vproj_d DRAM -> 16 cheap
    DRAM->DRAM DMAs build the per-head quad maps (maps_all[8, ES, 128]).
  - phase 3 computes positions/weights per query-half; floors use trunc
    (xpp >= 0.5 always); clamp+scale on Act (Relu chain); idx transposed to
    the 16-wrap via i16 PE transposes; idxw partitions 16..127 are zeroed
    once (executor only reads [:16]).
  - weights repacked to w4all_d DRAM; per half-chunk a 0-stride-src DMA
    replicates rows to wc[128, 4096] bf16 in SBUF (sync+scalar queues).
  - 64 half-chunks (head, point, q-half): Pool gather -> DVE single
    multiply -> PE reduce into psO4 -> Act copy to osb -> per-head out-proj.
"""
import sys

sys.path.insert(0, '/opt/trn_rl_repo')

import numpy as np
import ml_dtypes

import concourse.bass as bass
import concourse.bacc as bacc
import concourse.mybir as mybir
import concourse.tile as tile
from concourse.bass_utils import run_bass_kernel_spmd

dt = mybir.dt
F32, BF16, I16, I32 = dt.float32, dt.bfloat16, dt.int16, dt.int32
F32R = dt.float32r
ALU = mybir.AluOpType
ACTF = mybir.ActivationFunctionType
BF = ml_dtypes.bfloat16

# ---------------------------------------------------------------- geometry
LEVELS = ((76, 76), (38, 38), (19, 19), (10, 10))
NUM_HEADS, NUM_LEVELS, NUM_POINTS = 8, 4, 4
C, D = 256, 32
BS, NQ = 2, 7681
QP = 2048                      # queries per core (padded)
HQ = QP // 2
NCORES = 8
SAMP = NUM_HEADS * NUM_LEVELS * NUM_POINTS * QP       # 262144
IW = SAMP // 16                                       # idxw cols = 16384
STARTS = [0]
for (_h, _w) in LEVELS:
    STARTS.append(STARTS[-1] + _h * _w)
NV = STARTS[-1]                # 7681
NVPAD = 7808                   # 61 * 128
GAP = 256
CELLSTART = []
_pos = GAP
for (_h, _w) in LEVELS:
    CELLSTART.append(_pos)
    _pos += _h * _w + GAP
ES = ((_pos - GAP) // 128 + 1) * 128      # cell-space rows (8832)
TS2 = ES // 128                           # vsb_full blocks (69)
HCH = 4096                     # half-chunk samples
NHCH = SAMP // HCH             # 64


def align_down(x, a=128):
    return (x // a) * a


def mkap(base_ap, ap_list, offset=None):
    ap = base_ap.copy()
    ap.ap = mybir.VecI64Pair([list(x) for x in ap_list])
    if offset is not None:
        ap.offset = offset
    return ap


# ---------------------------------------------------------------- program
def build_nc():
    nc = bacc.Bacc("TRN2", target_bir_lowering=False)
    qT_d = nc.dram_tensor("qT", [128, 2, QP], F32R, kind="ExternalInput")
    refT_d = nc.dram_tensor("refT", [2, QP], F32R, kind="ExternalInput")
    vT_d = nc.dram_tensor("vT", [128, 2, NVPAD], BF16, kind="ExternalInput")
    woffx_d = nc.dram_tensor("woffx", [C, 128], F32R, kind="ExternalInput")
    woffy_d = nc.dram_tensor("woffy", [C, 128], F32R, kind="ExternalInput")
    wattn_d = nc.dram_tensor("wattn", [C, 128], F32R, kind="ExternalInput")
    boffx_d = nc.dram_tensor("boffx", [1, 128], BF16, kind="ExternalInput")
    boffy_d = nc.dram_tensor("boffy", [1, 128], BF16, kind="ExternalInput")
    battn_d = nc.dram_tensor("battn", [1, 128], BF16, kind="ExternalInput")
    wval_d = nc.dram_tensor("wval", [C, C], BF16, kind="ExternalInput")
    bval_d = nc.dram_tensor("bval", [1, C], BF16, kind="ExternalInput")
    cmask_d = nc.dram_tensor("cmask", [1, NVPAD], BF16, kind="ExternalInput")
    wout_d = nc.dram_tensor("wout", [128, 8, C], BF16, kind="ExternalInput")
    selx_d = nc.dram_tensor("selx", [2, 128], F32R, kind="ExternalInput")
    sely_d = nc.dram_tensor("sely", [2, 128], F32R, kind="ExternalInput")
    onesq_d = nc.dram_tensor("onesq", [1, QP], BF16, kind="ExternalInput")
    onesbf_d = nc.dram_tensor("onesbf", [1, 128], BF16, kind="ExternalInput")
    r128_d = nc.dram_tensor("r128", [128, 32], BF16, kind="ExternalInput")
    s16_d = nc.dram_tensor("s16", [128, 8], BF16, kind="ExternalInput")
    b8_d = nc.dram_tensor("b8", [8, 128], BF16, kind="ExternalInput")
    sclw_d = nc.dram_tensor("sclw", [128, 1], F32, kind="ExternalInput")
    sclh_d = nc.dram_tensor("sclh", [128, 1], F32, kind="ExternalInput")
    nsclw_d = nc.dram_tensor("nsclw", [128, 1], F32, kind="ExternalInput")
    nsclh_d = nc.dram_tensor("nsclh", [128, 1], F32, kind="ExternalInput")
    wlm1_d = nc.dram_tensor("wlm1", [128, 1], F32, kind="ExternalInput")
    hlm1_d = nc.dram_tensor("hlm1", [128, 1], F32, kind="ExternalInput")
    cbase_d = nc.dram_tensor("cbase", [128, 1], F32, kind="ExternalInput")
    qmask_d = nc.dram_tensor("qmask", [128, QP], BF16, kind="ExternalInput")
    iden_d = nc.dram_tensor("iden", [128, 128], BF16, kind="ExternalInput")
    out_d = nc.dram_tensor("out", [8, 3, 128, C], F32,
                           kind="ExternalOutput")
    vproj_d = nc.dram_tensor("vproj", [ES + 128, C], BF16)
    maps_d = nc.dram_tensor("mapsall", [NUM_HEADS, ES, 128], BF16)
    w4a_d = nc.dram_tensor("w4ad", [128, 2 * HCH], BF16)

    with tile.TileContext(nc) as tc:
        with tc.tile_pool(name="const", bufs=1) as cpool:
            selx = cpool.tile([2, 128], F32R)
            sely = cpool.tile([2, 128], F32R)
            onesq = cpool.tile([1, QP], BF16)
            onesbf = cpool.tile([1, 128], BF16)
            r128 = cpool.tile([128, 32], BF16)
            s16 = cpool.tile([128, 8], BF16)
            b8c = cpool.tile([8, 128], BF16)
            sclw = cpool.tile([128, 1], F32)
            sclh = cpool.tile([128, 1], F32)
            nsclw = cpool.tile([128, 1], F32)
            nsclh = cpool.tile([128, 1], F32)
            wlm1 = cpool.tile([128, 1], F32)
            hlm1 = cpool.tile([128, 1], F32)
            cbase = cpool.tile([128, 1], F32)
            woffx = cpool.tile([128, 2, 128], F32R)
            woffy = cpool.tile([128, 2, 128], F32R)
            wattn = cpool.tile([128, 2, 128], F32R)
            boffx = cpool.tile([1, 128], BF16)
            boffy = cpool.tile([1, 128], BF16)
            battn = cpool.tile([1, 128], BF16)
            wvalb = cpool.tile([128, 2, C], BF16)
            bvalb = cpool.tile([1, C], BF16)
            cmask = cpool.tile([1, NVPAD], BF16)
            woutc = cpool.tile([128, 8, C], BF16)
            qmask = cpool.tile([128, QP], BF16)
            iden = cpool.tile([128, 128], BF16)
            qT = cpool.tile([128, 2, QP], F32R)
            refT = cpool.tile([2, QP], F32R)
            # big loads split across sync/scalar
            nc.sync.dma_start(qT[:, 0], qT_d[:, 0])
            nc.scalar.dma_start(qT[:, 1], qT_d[:, 1])
            nc.sync.dma_start(refT[:], refT_d[:])
            for t, s in [(woffx, woffx_d), (woffy, woffy_d),
                         (wattn, wattn_d), (wvalb, wval_d)]:
                nc.scalar.dma_start(
                    t[:], s[:].rearrange("(h p) x -> p h x", p=128))
            for t, s in [(onesq, onesq_d), (boffx, boffx_d),
                         (boffy, boffy_d), (selx, selx_d),
                         (sely, sely_d), (sclw, sclw_d), (sclh, sclh_d),
                         (nsclw, nsclw_d), (nsclh, nsclh_d),
                         (wlm1, wlm1_d), (hlm1, hlm1_d),
                         (cbase, cbase_d), (iden, iden_d),
                         (qmask, qmask_d)]:
                nc.sync.dma_start(t[:], s[:])
            for t, s in [(battn, battn_d), (bvalb, bval_d),
                         (cmask, cmask_d), (onesbf, onesbf_d),
                         (s16, s16_d), (b8c, b8_d),
                         (r128, r128_d), (woutc, wout_d)]:
                nc.scalar.dma_start(t[:], s[:])

            ps3 = tc.alloc_tile_pool(name="ps3", bufs=2, space="PSUM")

            def projf(hq, wof, bof, sel):
                ps = ps3.tile([128, HQ], F32, tag="pp")
                for c0 in range(0, HQ, 512):
                    cs = slice(hq * HQ + c0, hq * HQ + c0 + 512)
                    po = ps[:, c0:c0 + 512]
                    for half in range(2):
                        nc.tensor.matmul(
                            po, wof[:, half, :], qT[:, half, cs],
                            start=(half == 0), stop=False)
                    nc.tensor.matmul(po, bof[:], onesq[:, cs],
                                     start=False, stop=(sel is None))
                    if sel is not None:
                        nc.tensor.matmul(po, sel[:], refT[:, cs],
                                         start=False, stop=True)
                return ps

            # hq0 x/y projections run on PE before anything else
            psX0 = projf(0, woffx, boffx, selx)
            psY0 = projf(0, woffy, boffy, sely)

            # ======== phase 2: vT load, v-proj, vproj_d, maps ============
            pVS = tc.alloc_tile_pool(name="pVS", bufs=1, side="right")
            vsb = pVS.tile([128, TS2, C], BF16)
            # zero the whole staging tile once (gaps + pad stay zero)
            nc.scalar.memzero(vsb[:])
            pVT = tc.alloc_tile_pool(name="pVT", bufs=1, side="right")
            p1t = tc.alloc_tile_pool(name="p1t", bufs=2)
            ps1 = tc.alloc_tile_pool(name="ps1", bufs=3, space="PSUM")
            vT = pVT.tile([128, 2, NVPAD], BF16)
            nc.sync.dma_start(vT[:, 0], vT_d[:, 0])
            nc.scalar.dma_start(vT[:, 1], vT_d[:, 1])
            ncp = [0]
            for lvl in range(NUM_LEVELS):
                hw = LEVELS[lvl][0] * LEVELS[lvl][1]
                shift = CELLSTART[lvl] - STARTS[lvl]  # mult of 128
                c0 = STARTS[lvl]
                while c0 < STARTS[lvl] + hw:
                    tbeg = align_down(c0)
                    cend = min(tbeg + 128, STARTS[lvl] + hw)
                    lo, hi = c0 - tbeg, cend - tbeg
                    psv = ps1.tile([128, C], F32, tag="psv")
                    for half in range(2):
                        nc.tensor.matmul(
                            psv[:], vT[:, half, tbeg:tbeg + 128],
                            wvalb[:, half, :], start=(half == 0),
                            stop=False)
                    nc.tensor.matmul(psv[:], cmask[:, tbeg:tbeg + 128],
                                     bvalb[:], start=False, stop=True)
                    sp = c0 + shift
                    assert sp % 128 == lo
                    blk = (tbeg + shift) // 128
                    if lo == 0 and hi == 128:
                        eng = nc.scalar if ncp[0] % 2 == 0 else nc.vector
                        ncp[0] += 1
                        if eng is nc.scalar:
                            eng.copy(vsb[:, blk, :], psv[:])
                        else:
                            eng.tensor_copy(vsb[:, blk, :], psv[:])
                    else:
                        vstage = p1t.tile([128, C], BF16, tag="vstage")
                        nc.scalar.copy(vstage[:], psv[:])
                        dst = mkap(vsb[:], [[TS2 * C, hi - lo], [1, C]],
                                   offset=lo * (TS2 * C) + blk * C)
                        src = mkap(vstage[:], [[C, hi - lo], [1, C]],
                                   offset=lo * C)
                        nc.sync.dma_start(dst, src)
                    c0 = cend
            # vproj_d write: 4 slices alternating queues
            bsl = [(0, 18), (18, 17), (35, 17), (52, TS2 - 52)]
            for i, (b0, nb) in enumerate(bsl):
                eng = nc.sync if i % 2 == 0 else nc.scalar
                eng.dma_start(
                    mkap(vproj_d[:], [[C, 128], [128 * C, nb], [1, C]],
                         offset=b0 * 128 * C),
                    mkap(vsb[:], [[TS2 * C, 128], [C, nb], [1, C]],
                         offset=b0 * C))
            nc.sync.dma_start(
                mkap(vproj_d[:], [[C, 128], [1, C]],
                     offset=ES * C),
                mkap(vsb[:], [[TS2 * C, 128], [1, C]]))
            # maps: one DRAM->DRAM DMA per (lvl, corner); ranges cover every
            # row of maps_d so the finite-checker never sees uninit DRAM
            MB = [0] + [CELLSTART[l] - LEVELS[l][1] - 1 for l in (1, 2, 3)] \
                + [ES]
            for lvl, (H, W) in enumerate(LEVELS):
                g0 = MB[lvl]
                n = MB[lvl + 1] - MB[lvl]
                for s, dl in enumerate([0, 1, W, W + 1]):
                    eng = nc.sync if (lvl * 4 + s) % 2 == 0 else nc.scalar
                    eng.dma_start(
                        mkap(maps_d[:], [[128, n], [ES * 128, 8], [1, 32]],
                             offset=g0 * 128 + s * 32),
                        mkap(vproj_d[:], [[C, n], [32, 8], [1, 32]],
                             offset=(g0 + dl) * C))
            ps1.release()
            p1t.release()
            pVT.release()
            pVS.release()

            # ======== phase 3: positions / weights / indices ==========
            pw = tc.alloc_tile_pool(name="pw", bufs=1)
            idxw = pw.tile([128, IW], I16)
            # executor reads idx partitions [:16] but asserts all 128 are
            # in-range: zero the tile once (copies then fill rows 0..15)
            nc.scalar.memzero(idxw[:])
            w4p = [pw.tile([128, QP], BF16, name=f"w4p{s}")
                   for s in range(4)]
            with tc.tile_pool(name="p3", bufs=1) as p3, \
                 tc.tile_pool(name="ps3s", bufs=1, space="PSUM") as ps3s, \
                 tc.tile_pool(name="psQ", bufs=2, space="PSUM") as psQp:
                for hq in range(2):
                    qs = slice(hq * HQ, (hq + 1) * HQ)

                    def corner_weights(psP, sclp, nsclp, wm1, pref):
                        rr = p3.tile([128, HQ], F32, tag="cwr")
                        xq = p3.tile([128, HQ], F32, tag=f"{pref}xq")
                        xi = p3.tile([128, HQ], I32, tag="cwxi")
                        x0p = p3.tile([128, HQ], F32, tag=f"{pref}x0p")
                        wx = p3.tile([128, HQ], F32, tag=f"{pref}wx")
                        v0 = p3.tile([128, HQ], BF16, tag="cwv0")
                        v1 = p3.tile([128, HQ], BF16, tag="cwv1")
                        wxf = p3.tile([128, HQ], BF16, tag="cwwxf")
                        a0 = p3.tile([128, HQ], BF16, tag=f"{pref}a0")
                        a1 = p3.tile([128, HQ], BF16, tag=f"{pref}a1")
                        # rr = max(1-p, 0); xq = max(scl - scl*rr, 0)
                        #   = scl*clamp01(p)
                        nc.scalar.activation(rr[:], psP[:], ACTF.Relu,
                                             bias=1.0, scale=-1.0)
                        nc.scalar.activation(xq[:], rr[:], ACTF.Relu,
                                             bias=sclp[:], scale=nsclp[:])
                        # x0p = floor(xq + 0.5) (trunc ok: arg >= 0.5)
                        nc.vector.tensor_scalar(xi[:], xq[:], 0.5, 0.0,
                                                op0=ALU.add,
                                                op1=ALU.bypass)
                        nc.vector.tensor_copy(x0p[:], xi[:])
                        # rounding-mode-agnostic floor: subtract 1 where
                        # the int conversion rounded up
                        t1 = p3.tile([128, HQ], F32, tag="cwt1")
                        nc.vector.scalar_tensor_tensor(
                            t1[:], xq[:], 0.5, x0p[:],
                            op0=ALU.add, op1=ALU.is_lt)
                        nc.vector.tensor_tensor(x0p[:], x0p[:], t1[:],
                                                op=ALU.subtract)
                        nc.vector.scalar_tensor_tensor(
                            wx[:], xq[:], 0.5, x0p[:],
                            op0=ALU.add, op1=ALU.subtract)
                        nc.vector.tensor_scalar(
                            v0[:], x0p[:], 1.0, 0.0,
                            op0=ALU.is_ge, op1=ALU.bypass)
                        nc.vector.tensor_scalar(
                            v1[:], x0p[:], wm1[:], 0.0,
                            op0=ALU.is_le, op1=ALU.bypass)
                        nc.vector.tensor_tensor(a1[:], wx[:], v1[:],
                                                op=ALU.mult)
                        nc.vector.tensor_scalar(
                            wxf[:], wx[:], -1.0, 1.0,
                            op0=ALU.mult, op1=ALU.add)
                        nc.vector.tensor_tensor(a0[:], wxf[:], v0[:],
                                                op=ALU.mult)
                        return x0p, a0, a1

                    if hq == 0:
                        psX, psY = psX0, psY0
                    else:
                        psX = projf(hq, woffx, boffx, selx)
                        psY = projf(hq, woffy, boffy, sely)
                    x0p, ax0, ax1 = corner_weights(psX, sclw, nsclw,
                                                   wlm1, "x")
                    y0p, ay0, ay1 = corner_weights(psY, sclh, nsclh,
                                                   hlm1, "y")
                    idxf = p3.tile([128, HQ], F32, tag="cwxq2")
                    idxf16 = p3.tile([128, HQ], I16, tag=f"i16{hq}")
                    nc.vector.scalar_tensor_tensor(
                        idxf[:], y0p[:], sclw[:], x0p[:],
                        op0=ALU.mult, op1=ALU.add)
                    nc.vector.tensor_scalar(
                        idxf16[:], idxf[:], cbase[:], 0.0,
                        op0=ALU.add, op1=ALU.bypass)
                    # i16 transposes -> idxw 16-wrap
                    # col = ci*512 + hq*256 + m*64 + jj  (ci=h*4+a, m=pt)
                    for jj in range(HQ // 16):
                        psQ = psQp.tile([16, 128], BF16, tag="psQ")
                        nc.tensor.transpose(
                            psQ[:],
                            idxf16[:, jj * 16:(jj + 1) * 16].bitcast(BF16),
                            iden[:])
                        dst = mkap(idxw[:], [[IW, 16], [64, 4], [512, 32]],
                                   offset=hq * 256 + jj)
                        src = mkap(psQ[:].bitcast(I16),
                                   [[128, 16], [32, 4], [1, 32]])
                        if jj % 4 < 3:
                            nc.vector.tensor_copy(dst, src)
                        else:
                            nc.scalar.activation(dst, src, ACTF.Copy)
                    # ---- attention weights ----
                    psZ = projf(hq, wattn, battn, None)
                    esb = p3.tile([128, HQ], BF16, tag="cwesb")
                    nc.scalar.activation(esb[:], psZ[:], ACTF.Exp,
                                         bias=0.0, scale=1.0)
                    pss = ps3s.tile([8, HQ], F32, tag="pss")
                    for c0 in range(0, HQ, 512):
                        nc.tensor.matmul(pss[:, c0:c0 + 512], s16[:],
                                         esb[:, c0:c0 + 512],
                                         start=True, stop=True)
                    rsb = p3.tile([8, HQ], F32, tag="cwrsb")
                    nc.vector.reciprocal(rsb[:], pss[:])
                    rsbq = p3.tile([8, HQ], BF16, tag="cwrsbq")
                    nc.vector.tensor_tensor(rsbq[:], rsb[:],
                                            qmask[0:8, qs], op=ALU.mult)
                    psr2 = ps3.tile([128, HQ], F32, tag="pp")
                    for c0 in range(0, HQ, 512):
                        nc.tensor.matmul(psr2[:, c0:c0 + 512], b8c[:],
                                         rsbq[:, c0:c0 + 512],
                                         start=True, stop=True)
                    aw = p3.tile([128, HQ], BF16, tag="cwaw")
                    nc.vector.tensor_tensor(aw[:], esb[:], psr2[:],
                                            op=ALU.mult)
                    nc.vector.tensor_tensor(ay0[:], ay0[:], aw[:],
                                            op=ALU.mult)
                    nc.vector.tensor_tensor(ay1[:], ay1[:], aw[:],
                                            op=ALU.mult)
                    for s, (ax, ay) in enumerate(
                            [(ax0, ay0), (ax1, ay0),
                             (ax0, ay1), (ax1, ay1)]):
                        nc.vector.tensor_tensor(
                            w4p[s][:, qs], ax[:], ay[:], op=ALU.mult)
                    # repack w4p -> w4a_d DRAM: rows (s,ci), cols
                    # hq*4096 + m*1024 + q_l
                    for s in range(4):
                        for m in range(4):
                            eng = nc.sync if (s * 4 + m) % 2 == 0 \
                                else nc.scalar
                            eng.dma_start(
                                mkap(w4a_d[:],
                                     [[2 * HCH, 32], [1, HQ]],
                                     offset=s * 32 * 2 * HCH
                                     + hq * HCH + m * HQ),
                                mkap(w4p[s][:], [[QP, 32], [1, HQ]],
                                     offset=32 * m * QP + hq * HQ))
                # replicate idx 16 -> 128 partitions (HW reads replicas)
                for (r0, nr) in [(16, 16), (32, 32), (64, 64)]:
                    for half in range(2):
                        eng = nc.sync if half == 0 else nc.scalar
                        cs = half * (IW // 2)
                        eng.dma_start(
                            mkap(idxw[:], [[IW, nr], [1, IW // 2]],
                                 offset=r0 * IW + cs),
                            mkap(idxw[:], [[IW, nr], [1, IW // 2]],
                                 offset=(r0 - nr) * IW + cs))

            ps3.release()
            # ======== phase 4: gather / multiply / reduce =================
            with tc.tile_pool(name="p4t", bufs=2) as p4t, \
                 tc.tile_pool(name="pwc", bufs=3) as pwc, \
                 tc.tile_pool(name="psW", bufs=2, space="PSUM") as psWp, \
                 tc.tile_pool(name="psO", bufs=1, space="PSUM") as psO, \
                 tc.tile_pool(name="pOs", bufs=1) as pOs, \
                 tc.tile_pool(name="pg", bufs=3) as pg, \
                 tc.tile_pool(name="pfo", bufs=3) as pfo:
                osb = [pOs.tile([128, QP + 2048], BF16, name=f"osb{g}",
                                tag=f"osb{g}")
                       for g in range(2)]
                for g in range(2):
                    nc.scalar.memzero(osb[g][:])

                def phase5_head(m):
                    # reference reshape quirk: out row q column c takes
                    # O[m, qq, d] with u = m*7681 + qq = 8q + (c//32 slot),
                    # W_out row 32*((m+qq)%8) + d.  7681 % 8 == 1 makes the
                    # structure identical on every core (host assembles).
                    gsb = osb[m // 4]
                    grp = m % 4
                    dlt = 64 * (m % 2)
                    for j in range(3):
                        psF4 = psWp.tile([128, 1024], F32, tag="psW")
                        psF = psF4[:, 0:C]
                        for s in range(8):
                            q0 = -m - 8 * dlt + 1024 * j + s
                            col0 = 1024 + q0
                            assert 0 <= col0 and col0 + 8 * 127 < QP + 2048
                            lhsT = mkap(
                                gsb[:], [[QP + 2048, 32], [8, 128]],
                                offset=(grp * 32) * (QP + 2048) + col0)
                            kw = {}
                            if grp == 3:
                                kw["tile_position"] = (96, 0)
                            nc.tensor.matmul(
                                psF, lhsT,
                                woutc[grp * 32:grp * 32 + 32, s, :],
                                start=(s == 0), stop=(s == 7), **kw)
                        fo = pfo.tile([128, C], F32, tag="fo")
                        nc.scalar.copy(fo[:], psF)
                        eng = nc.sync if j % 2 == 0 else nc.scalar
                        eng.dma_start(out_d[m, j], fo[:])

                psO4 = None
                for ck in range(NHCH):
                    hh, a, hq = ck // 8, (ck % 8) // 2, ck % 2
                    ci = hh * 4 + a
                    grp = hh % 4
                    if grp == 0 and a == 0 and hq == 0:
                        psO4 = psO.tile([128, QP], F32, tag="psO4")
                    g = pg.tile([128, 1, HCH], BF16, tag="g")
                    nc.gpsimd.dma_gather(
                        g[:], maps_d[hh],
                        idxw[:, ck * (HCH // 16):(ck + 1) * (HCH // 16)],
                        HCH, HCH, 128,
                        transpose=True, single_packet=False)
                    # replicate weights: wc[s*32+ch, :] = w4a_d[s*32+ci,
                    # hq*4096 + :]
                    wc = pwc.tile([128, HCH], BF16, tag="wc")
                    for half in range(2):
                        eng = nc.sync if half == 0 else nc.scalar
                        eng.dma_start(
                            mkap(wc[:], [[HCH, 128], [1, HCH // 2]],
                                 offset=half * (HCH // 2)),
                            mkap(w4a_d[:],
                                 [[32 * 2 * HCH, 4], [0, 32],
                                  [1, HCH // 2]],
                                 offset=ci * 2 * HCH + hq * HCH
                                 + half * (HCH // 2)))
                    tt = p4t.tile([128, HCH], BF16, tag="tt")
                    nc.vector.tensor_tensor(tt[:], g[:, 0, :], wc[:],
                                            op=ALU.mult)
                    for m4 in range(4):
                        for j2 in range(2):
                            kw = {}
                            if grp == 3:
                                kw["tile_position"] = (0, 96)
                            cs = slice(hq * 1024 + j2 * 512,
                                       hq * 1024 + (j2 + 1) * 512)
                            nc.tensor.matmul(
                                psO4[grp * 32:(grp + 1) * 32, cs],
                                r128[:],
                                tt[:, m4 * 1024 + j2 * 512:
                                   m4 * 1024 + (j2 + 1) * 512],
                                start=(a == 0 and m4 == 0),
                                stop=(a == 3 and m4 == 3), **kw)
                    if a == 3 and hq == 1:
                        nc.scalar.activation(
                            osb[hh // 4][grp * 32:(grp + 1) * 32,
                                         1024:1024 + QP],
                            psO4[grp * 32:(grp + 1) * 32, :], ACTF.Copy)
                    if ck >= 11 and (ck - 11) % 8 == 0:
                        phase5_head((ck - 11) // 8)

                # ======== phase 5 tail ====================================
                phase5_head(7)
            pw.release()
    nc.compile()
    return nc


# ---------------------------------------------------------------- host side
_CACHE = {}


def _consts(W_off, b_off, W_attn, b_attn, W_val, b_val, W_out, b_out):
    M = NUM_HEADS
    # partition layout c = b*32 + h*4 + a  (old: h*16 + a*4 + b)
    woff = np.asarray(W_off, np.float32).reshape(C, M, 4, 4, 2)
    woff = np.transpose(woff, (0, 3, 1, 2, 4))          # (C, b, h, a, 2)
    wattn = np.asarray(W_attn, np.float32).reshape(C, M, 4, 4)
    # partition (b, h, a) holds attention logit (level=b, point=a) so that
    # sample (h, a, b) pairs with aw(level=b, point=a)  (reference quirk)
    wattn = np.transpose(wattn, (0, 2, 1, 3))           # (C, l, h, k)
    boff = np.asarray(b_off, np.float32).reshape(M, 4, 4, 2)
    boff = np.transpose(boff, (2, 0, 1, 3))             # (b, h, a, 2)
    battn = np.asarray(b_attn, np.float32).reshape(M, 4, 4)
    battn = np.transpose(battn, (1, 0, 2))              # (l, h, k)
    cm = {}
    cm["woffx"] = np.ascontiguousarray(woff[..., 0].reshape(C, 128))
    cm["woffy"] = np.ascontiguousarray(woff[..., 1].reshape(C, 128))
    cm["wattn"] = np.ascontiguousarray(wattn.reshape(C, 128))
    cm["boffx"] = np.ascontiguousarray(boff[..., 0].reshape(1, 128)).astype(BF)
    cm["boffy"] = np.ascontiguousarray(boff[..., 1].reshape(1, 128)).astype(BF)
    cm["battn"] = np.ascontiguousarray(battn.reshape(1, 128)).astype(BF)
    cm["wval"] = np.asarray(W_val, np.float32).astype(BF)
    cm["bval"] = np.asarray(b_val, np.float32).reshape(1, C).astype(BF)
    wof = np.asarray(W_out, np.float32).reshape(8, 32, C).transpose(1, 0, 2)
    cm["wout"] = np.ascontiguousarray(
        np.broadcast_to(wof[None], (4, 32, 8, C)).reshape(128, 8, C)
    ).astype(BF)
    sel = np.zeros((2, 128), np.float32)
    sel[0] = 1.0
    cm["selx"] = sel
    cm["sely"] = sel[::-1].copy()
    cm["onesq"] = np.ones((1, QP), np.float32).astype(BF)
    cm["onesbf"] = np.ones((1, 128), np.float32).astype(BF)
    cmk = np.zeros((1, NVPAD), np.float32)
    cmk[0, :NV] = 1.0
    cm["cmask"] = cmk.astype(BF)
    r = np.zeros((128, 32), np.float32)
    for p in range(128):
        r[p, p % 32] = 1.0
    cm["r128"] = r.astype(BF)
    s16 = np.zeros((128, 8), np.float32)
    b8 = np.zeros((8, 128), np.float32)
    for p in range(128):
        h = (p % 32) // 4
        s16[p, h] = 1.0
        b8[h, p] = 1.0
    cm["s16"] = s16.astype(BF)
    cm["b8"] = b8.astype(BF)
    lvl_of_p = np.arange(128) % 4                       # level = a = c%4
    Wl = np.array([LEVELS[l][1] for l in lvl_of_p], np.float32)
    Hl = np.array([LEVELS[l][0] for l in lvl_of_p], np.float32)
    cb = np.array([CELLSTART[l] - LEVELS[l][1] - 1 for l in lvl_of_p],
                  np.float32)
    cm["iden"] = np.eye(128, dtype=np.float32).astype(BF)
    cm["sclw"] = Wl.reshape(128, 1)
    cm["sclh"] = Hl.reshape(128, 1)
    cm["nsclw"] = (-Wl).reshape(128, 1)
    cm["nsclh"] = (-Hl).reshape(128, 1)
    cm["wlm1"] = (Wl - 1).reshape(128, 1)
    cm["hlm1"] = (Hl - 1).reshape(128, 1)
    cm["cbase"] = cb.reshape(128, 1)
    return cm


def kernel(**inputs):
    if "nc" not in _CACHE:
        _CACHE["nc"] = build_nc()
    nc = _CACHE["nc"]
    cm = _consts(inputs["W_off"], inputs["b_off"], inputs["W_attn"],
                 inputs["b_attn"], inputs["W_val"], inputs["b_val"],
                 inputs["W_out"], inputs["b_out"])
    query = np.asarray(inputs["query"], np.float32)
    refp = np.asarray(inputs["reference_points"], np.float32)
    value = np.asarray(inputs["value"], np.float32)
    vpad = np.zeros((BS, NVPAD, C), np.float32)
    vpad[:, :NV] = value
    qpad = np.zeros((BS, 4 * QP, C), np.float32)
    qpad[:, :NQ] = query
    rpad = np.zeros((BS, 4 * QP, 2), np.float32)
    rpad[:, :NQ] = refp
    vT_b = []
    for b in range(BS):
        vT = vpad[b].T.reshape(2, 128, NVPAD).transpose(1, 0, 2)
        vT_b.append(np.ascontiguousarray(vT.astype(BF)))
    in_maps = []
    for core in range(NCORES):
        b, qc = core // 4, core % 4
        nvalid = min(QP, max(0, NQ - qc * QP))
        qm = np.zeros((128, QP), np.float32)
        qm[:, :nvalid] = 1.0
        qm = qm.astype(BF)
        qs = qpad[b, qc * QP:(qc + 1) * QP]
        rs = rpad[b, qc * QP:(qc + 1) * QP]
        qT = qs.T.reshape(2, 128, QP).transpose(1, 0, 2)
        m = {"qT": np.ascontiguousarray(qT),
             "refT": np.ascontiguousarray(rs.T),
             "vT": vT_b[b],
             "qmask": qm}
        m.update({k: np.ascontiguousarray(v) for k, v in cm.items()})
        in_maps.append(m)
    res = run_bass_kernel_spmd(nc, in_maps, list(range(NCORES)),
                               **_CACHE.get("run_kw", {}))
    _CACHE["last_res"] = res
    out = np.zeros((BS, NQ + 512, C), np.float32)
    for core in range(NCORES):
        b, qc = core // 4, core % 4
        slab = res.results[core]["out"]        # [8, 3, 128, 256]
        for m in range(NUM_HEADS):
            dlt = 64 * (m % 2)
            tb = 960 * m + 256 * qc - dlt      # absolute tile base
            for j in range(3):
                if m % 2 == 0:
                    row_lo, row_hi = 0, (128, 128, 32)[j]
                else:
                    row_lo, row_hi = ((64, 0, 0)[j], (128, 128, 96)[j])
                r0 = tb + 128 * j + row_lo
                r1 = tb + 128 * j + row_hi
                r1c = min(r1, NQ + 512)
                if r0 < 0 or r1c <= r0:
                    continue
                out[b, r0:r1c] += slab[m, j, row_lo:row_lo + (r1c - r0)]
    out = out[:, :NQ] + np.asarray(inputs["b_out"], np.float32)[None, None]
    return out


# revision 13
# speedup vs baseline: 1.1799x; 1.0015x over previous
"""Multi-scale deformable attention on 8 Trainium2 NeuronCores.

Sharding: (batch x query-quarter) -> 8 cores; each core does all 8 heads for
2048 queries of one batch (value projection recomputed per core).

v3 design (cost-model-aware):
  - v-proj -> vsb_full (cell-major bf16) -> vproj_d DRAM -> 16 cheap
    DRAM->DRAM DMAs build the per-head quad maps (maps_all[8, ES, 128]).
  - phase 3 computes positions/weights per query-half; floors use trunc
    (xpp >= 0.5 always); clamp+scale on Act (Relu chain); idx transposed to
    the 16-wrap via i16 PE transposes; idxw partitions 16..127 are zeroed
    once (executor only reads [:16]).
  - weights repacked to w4all_d DRAM; per half-chunk a 0-stride-src DMA
    replicates rows to wc[128, 4096] bf16 in SBUF (sync+scalar queues).
  - 64 half-chunks (head, point, q-half): Pool gather -> DVE single
    multiply -> PE reduce into psO4 -> Act copy to osb -> per-head out-proj.
"""
import sys

sys.path.insert(0, '/opt/trn_rl_repo')

import numpy as np
import ml_dtypes

import concourse.bass as bass
import concourse.bacc as bacc
import concourse.mybir as mybir
import concourse.tile as tile
from concourse.bass_utils import run_bass_kernel_spmd

dt = mybir.dt
F32, BF16, I16, I32 = dt.float32, dt.bfloat16, dt.int16, dt.int32
F32R = dt.float32r
ALU = mybir.AluOpType
ACTF = mybir.ActivationFunctionType
BF = ml_dtypes.bfloat16

# ---------------------------------------------------------------- geometry
LEVELS = ((76, 76), (38, 38), (19, 19), (10, 10))
NUM_HEADS, NUM_LEVELS, NUM_POINTS = 8, 4, 4
C, D = 256, 32
BS, NQ = 2, 7681
QP = 2048                      # queries per core (padded)
HQ = QP // 2
NCORES = 8
SAMP = NUM_HEADS * NUM_LEVELS * NUM_POINTS * QP       # 262144
IW = SAMP // 16                                       # idxw cols = 16384
STARTS = [0]
for (_h, _w) in LEVELS:
    STARTS.append(STARTS[-1] + _h * _w)
NV = STARTS[-1]                # 7681
NVPAD = 7808                   # 61 * 128
GAP = 256
CELLSTART = []
_pos = GAP
for (_h, _w) in LEVELS:
    CELLSTART.append(_pos)
    _pos += _h * _w + GAP
ES = ((_pos - GAP) // 128 + 1) * 128      # cell-space rows (8832)
TS2 = ES // 128                           # vsb_full blocks (69)
HCH = 4096                     # half-chunk samples
NHCH = SAMP // HCH             # 64


def align_down(x, a=128):
    return (x // a) * a


def mkap(base_ap, ap_list, offset=None):
    ap = base_ap.copy()
    ap.ap = mybir.VecI64Pair([list(x) for x in ap_list])
    if offset is not None:
        ap.offset = offset
    return ap


# ---------------------------------------------------------------- program
def build_nc():
    nc = bacc.Bacc("TRN2", target_bir_lowering=False)
    qT_d = nc.dram_tensor("qT", [128, 2, QP], F32, kind="ExternalInput")
    refT_d = nc.dram_tensor("refT", [2, QP], F32, kind="ExternalInput")
    vT_d = nc.dram_tensor("vT", [128, 2, NVPAD], BF16, kind="ExternalInput")
    woffx_d = nc.dram_tensor("woffx", [C, 128], F32, kind="ExternalInput")
    woffy_d = nc.dram_tensor("woffy", [C, 128], F32, kind="ExternalInput")
    wattn_d = nc.dram_tensor("wattn", [C, 128], F32, kind="ExternalInput")
    boffx_d = nc.dram_tensor("boffx", [1, 128], BF16, kind="ExternalInput")
    boffy_d = nc.dram_tensor("boffy", [1, 128], BF16, kind="ExternalInput")
    battn_d = nc.dram_tensor("battn", [1, 128], BF16, kind="ExternalInput")
    wval_d = nc.dram_tensor("wval", [C, C], BF16, kind="ExternalInput")
    bval_d = nc.dram_tensor("bval", [1, C], BF16, kind="ExternalInput")
    cmask_d = nc.dram_tensor("cmask", [1, NVPAD], BF16, kind="ExternalInput")
    wout_d = nc.dram_tensor("wout", [128, 8, C], BF16, kind="ExternalInput")
    selx_d = nc.dram_tensor("selx", [2, 128], F32, kind="ExternalInput")
    sely_d = nc.dram_tensor("sely", [2, 128], F32, kind="ExternalInput")
    onesq_d = nc.dram_tensor("onesq", [1, QP], BF16, kind="ExternalInput")
    onesbf_d = nc.dram_tensor("onesbf", [1, 128], BF16, kind="ExternalInput")
    r128_d = nc.dram_tensor("r128", [128, 32], BF16, kind="ExternalInput")
    s16_d = nc.dram_tensor("s16", [128, 8], BF16, kind="ExternalInput")
    b8_d = nc.dram_tensor("b8", [8, 128], BF16, kind="ExternalInput")
    sclw_d = nc.dram_tensor("sclw", [128, 1], F32, kind="ExternalInput")
    sclh_d = nc.dram_tensor("sclh", [128, 1], F32, kind="ExternalInput")
    nsclw_d = nc.dram_tensor("nsclw", [128, 1], F32, kind="ExternalInput")
    nsclh_d = nc.dram_tensor("nsclh", [128, 1], F32, kind="ExternalInput")
    wlm1_d = nc.dram_tensor("wlm1", [128, 1], F32, kind="ExternalInput")
    hlm1_d = nc.dram_tensor("hlm1", [128, 1], F32, kind="ExternalInput")
    cbase_d = nc.dram_tensor("cbase", [128, 1], F32, kind="ExternalInput")
    qmask_d = nc.dram_tensor("qmask", [128, QP], BF16, kind="ExternalInput")
    iden_d = nc.dram_tensor("iden", [128, 128], BF16, kind="ExternalInput")
    out_d = nc.dram_tensor("out", [8, 3, 128, C], F32,
                           kind="ExternalOutput")
    vproj_d = nc.dram_tensor("vproj", [ES + 128, C], BF16)
    maps_d = nc.dram_tensor("mapsall", [NUM_HEADS, ES, 128], BF16)
    w4a_d = nc.dram_tensor("w4ad", [128, 2 * HCH], BF16)

    with tile.TileContext(nc) as tc:
        with tc.tile_pool(name="const", bufs=1) as cpool:
            selx = cpool.tile([2, 128], F32)
            sely = cpool.tile([2, 128], F32)
            onesq = cpool.tile([1, QP], BF16)
            onesbf = cpool.tile([1, 128], BF16)
            r128 = cpool.tile([128, 32], BF16)
            s16 = cpool.tile([128, 8], BF16)
            b8c = cpool.tile([8, 128], BF16)
            sclw = cpool.tile([128, 1], F32)
            sclh = cpool.tile([128, 1], F32)
            nsclw = cpool.tile([128, 1], F32)
            nsclh = cpool.tile([128, 1], F32)
            wlm1 = cpool.tile([128, 1], F32)
            hlm1 = cpool.tile([128, 1], F32)
            cbase = cpool.tile([128, 1], F32)
            woffx = cpool.tile([128, 2, 128], F32)
            woffy = cpool.tile([128, 2, 128], F32)
            wattn = cpool.tile([128, 2, 128], F32)
            boffx = cpool.tile([1, 128], BF16)
            boffy = cpool.tile([1, 128], BF16)
            battn = cpool.tile([1, 128], BF16)
            wvalb = cpool.tile([128, 2, C], BF16)
            bvalb = cpool.tile([1, C], BF16)
            cmask = cpool.tile([1, NVPAD], BF16)
            woutc = cpool.tile([128, 8, C], BF16)
            qmask = cpool.tile([128, QP], BF16)
            iden = cpool.tile([128, 128], BF16)
            qT = cpool.tile([128, 2, QP], F32)
            refT = cpool.tile([2, QP], F32)
            # big loads split across sync/scalar
            nc.sync.dma_start(qT[:, 0], qT_d[:, 0])
            nc.scalar.dma_start(qT[:, 1], qT_d[:, 1])
            nc.sync.dma_start(refT[:], refT_d[:])
            for t, s in [(woffx, woffx_d), (woffy, woffy_d),
                         (wattn, wattn_d), (wvalb, wval_d)]:
                nc.scalar.dma_start(
                    t[:], s[:].rearrange("(h p) x -> p h x", p=128))
            for t, s in [(onesq, onesq_d), (boffx, boffx_d),
                         (boffy, boffy_d), (selx, selx_d),
                         (sely, sely_d), (sclw, sclw_d), (sclh, sclh_d),
                         (nsclw, nsclw_d), (nsclh, nsclh_d),
                         (wlm1, wlm1_d), (hlm1, hlm1_d),
                         (cbase, cbase_d), (iden, iden_d),
                         (qmask, qmask_d)]:
                nc.sync.dma_start(t[:], s[:])
            for t, s in [(battn, battn_d), (bvalb, bval_d),
                         (cmask, cmask_d), (onesbf, onesbf_d),
                         (s16, s16_d), (b8c, b8_d),
                         (r128, r128_d), (woutc, wout_d)]:
                nc.scalar.dma_start(t[:], s[:])

            ps3 = tc.alloc_tile_pool(name="ps3", bufs=2, space="PSUM")

            def projf(hq, wof, bof, sel):
                ps = ps3.tile([128, HQ], F32, tag="pp")
                for c0 in range(0, HQ, 512):
                    cs = slice(hq * HQ + c0, hq * HQ + c0 + 512)
                    po = ps[:, c0:c0 + 512]
                    for half in range(2):
                        nc.tensor.matmul(
                            po, wof[:, half, :], qT[:, half, cs],
                            start=(half == 0), stop=False)
                    nc.tensor.matmul(po, bof[:], onesq[:, cs],
                                     start=False, stop=(sel is None))
                    if sel is not None:
                        nc.tensor.matmul(po, sel[:], refT[:, cs],
                                         start=False, stop=True)
                return ps

            # hq0 x/y projections run on PE before anything else
            psX0 = projf(0, woffx, boffx, selx)
            psY0 = projf(0, woffy, boffy, sely)

            # ======== phase 2: vT load, v-proj, vproj_d, maps ============
            pVS = tc.alloc_tile_pool(name="pVS", bufs=1, side="right")
            vsb = pVS.tile([128, TS2, C], BF16)
            # zero the whole staging tile once (gaps + pad stay zero)
            nc.scalar.memzero(vsb[:])
            pVT = tc.alloc_tile_pool(name="pVT", bufs=1, side="right")
            p1t = tc.alloc_tile_pool(name="p1t", bufs=2)
            ps1 = tc.alloc_tile_pool(name="ps1", bufs=3, space="PSUM")
            vT = pVT.tile([128, 2, NVPAD], BF16)
            nc.sync.dma_start(vT[:, 0], vT_d[:, 0])
            nc.scalar.dma_start(vT[:, 1], vT_d[:, 1])
            ncp = [0]
            for lvl in range(NUM_LEVELS):
                hw = LEVELS[lvl][0] * LEVELS[lvl][1]
                shift = CELLSTART[lvl] - STARTS[lvl]  # mult of 128
                c0 = STARTS[lvl]
                while c0 < STARTS[lvl] + hw:
                    tbeg = align_down(c0)
                    cend = min(tbeg + 128, STARTS[lvl] + hw)
                    lo, hi = c0 - tbeg, cend - tbeg
                    psv = ps1.tile([128, C], F32, tag="psv")
                    for half in range(2):
                        nc.tensor.matmul(
                            psv[:], vT[:, half, tbeg:tbeg + 128],
                            wvalb[:, half, :], start=(half == 0),
                            stop=False)
                    nc.tensor.matmul(psv[:], cmask[:, tbeg:tbeg + 128],
                                     bvalb[:], start=False, stop=True)
                    sp = c0 + shift
                    assert sp % 128 == lo
                    blk = (tbeg + shift) // 128
                    if lo == 0 and hi == 128:
                        eng = nc.scalar if ncp[0] % 2 == 0 else nc.vector
                        ncp[0] += 1
                        if eng is nc.scalar:
                            eng.copy(vsb[:, blk, :], psv[:])
                        else:
                            eng.tensor_copy(vsb[:, blk, :], psv[:])
                    else:
                        vstage = p1t.tile([128, C], BF16, tag="vstage")
                        nc.scalar.copy(vstage[:], psv[:])
                        dst = mkap(vsb[:], [[TS2 * C, hi - lo], [1, C]],
                                   offset=lo * (TS2 * C) + blk * C)
                        src = mkap(vstage[:], [[C, hi - lo], [1, C]],
                                   offset=lo * C)
                        nc.sync.dma_start(dst, src)
                    c0 = cend
            # vproj_d write: 4 slices alternating queues
            bsl = [(0, 18), (18, 17), (35, 17), (52, TS2 - 52)]
            for i, (b0, nb) in enumerate(bsl):
                eng = nc.sync if i % 2 == 0 else nc.scalar
                eng.dma_start(
                    mkap(vproj_d[:], [[C, 128], [128 * C, nb], [1, C]],
                         offset=b0 * 128 * C),
                    mkap(vsb[:], [[TS2 * C, 128], [C, nb], [1, C]],
                         offset=b0 * C))
            nc.sync.dma_start(
                mkap(vproj_d[:], [[C, 128], [1, C]],
                     offset=ES * C),
                mkap(vsb[:], [[TS2 * C, 128], [1, C]]))
            # maps: one DRAM->DRAM DMA per (lvl, corner); ranges cover every
            # row of maps_d so the finite-checker never sees uninit DRAM
            MB = [0] + [CELLSTART[l] - LEVELS[l][1] - 1 for l in (1, 2, 3)] \
                + [ES]
            for lvl, (H, W) in enumerate(LEVELS):
                g0 = MB[lvl]
                n = MB[lvl + 1] - MB[lvl]
                for s, dl in enumerate([0, 1, W, W + 1]):
                    eng = nc.sync if (lvl * 4 + s) % 2 == 0 else nc.scalar
                    eng.dma_start(
                        mkap(maps_d[:], [[128, n], [ES * 128, 8], [1, 32]],
                             offset=g0 * 128 + s * 32),
                        mkap(vproj_d[:], [[C, n], [32, 8], [1, 32]],
                             offset=(g0 + dl) * C))
            ps1.release()
            p1t.release()
            pVT.release()
            pVS.release()

            # ======== phase 3: positions / weights / indices ==========
            pw = tc.alloc_tile_pool(name="pw", bufs=1)
            idxw = pw.tile([128, IW], I16)
            # executor reads idx partitions [:16] but asserts all 128 are
            # in-range: zero the tile once (copies then fill rows 0..15)
            nc.scalar.memzero(idxw[:])
            w4p = [pw.tile([128, QP], BF16, name=f"w4p{s}")
                   for s in range(4)]
            with tc.tile_pool(name="p3", bufs=1) as p3, \
                 tc.tile_pool(name="ps3s", bufs=1, space="PSUM") as ps3s, \
                 tc.tile_pool(name="psQ", bufs=2, space="PSUM") as psQp:
                for hq in range(2):
                    qs = slice(hq * HQ, (hq + 1) * HQ)

                    def corner_weights(psP, sclp, nsclp, wm1, pref):
                        rr = p3.tile([128, HQ], F32, tag="cwr")
                        xq = p3.tile([128, HQ], F32, tag=f"{pref}xq")
                        xi = p3.tile([128, HQ], I32, tag="cwxi")
                        x0p = p3.tile([128, HQ], F32, tag=f"{pref}x0p")
                        wx = p3.tile([128, HQ], F32, tag=f"{pref}wx")
                        v0 = p3.tile([128, HQ], BF16, tag="cwv0")
                        v1 = p3.tile([128, HQ], BF16, tag="cwv1")
                        wxf = p3.tile([128, HQ], BF16, tag="cwwxf")
                        a0 = p3.tile([128, HQ], BF16, tag=f"{pref}a0")
                        a1 = p3.tile([128, HQ], BF16, tag=f"{pref}a1")
                        # rr = max(1-p, 0); xq = max(scl - scl*rr, 0)
                        #   = scl*clamp01(p)
                        nc.scalar.activation(rr[:], psP[:], ACTF.Relu,
                                             bias=1.0, scale=-1.0)
                        nc.scalar.activation(xq[:], rr[:], ACTF.Relu,
                                             bias=sclp[:], scale=nsclp[:])
                        # x0p = floor(xq + 0.5) (trunc ok: arg >= 0.5)
                        nc.vector.tensor_scalar(xi[:], xq[:], 0.5, 0.0,
                                                op0=ALU.add,
                                                op1=ALU.bypass)
                        nc.vector.tensor_copy(x0p[:], xi[:])
                        # rounding-mode-agnostic floor: subtract 1 where
                        # the int conversion rounded up
                        t1 = p3.tile([128, HQ], F32, tag="cwt1")
                        nc.vector.scalar_tensor_tensor(
                            t1[:], xq[:], 0.5, x0p[:],
                            op0=ALU.add, op1=ALU.is_lt)
                        nc.vector.tensor_tensor(x0p[:], x0p[:], t1[:],
                                                op=ALU.subtract)
                        nc.vector.scalar_tensor_tensor(
                            wx[:], xq[:], 0.5, x0p[:],
                            op0=ALU.add, op1=ALU.subtract)
                        nc.vector.tensor_scalar(
                            v0[:], x0p[:], 1.0, 0.0,
                            op0=ALU.is_ge, op1=ALU.bypass)
                        nc.vector.tensor_scalar(
                            v1[:], x0p[:], wm1[:], 0.0,
                            op0=ALU.is_le, op1=ALU.bypass)
                        nc.vector.tensor_tensor(a1[:], wx[:], v1[:],
                                                op=ALU.mult)
                        nc.vector.tensor_scalar(
                            wxf[:], wx[:], -1.0, 1.0,
                            op0=ALU.mult, op1=ALU.add)
                        nc.vector.tensor_tensor(a0[:], wxf[:], v0[:],
                                                op=ALU.mult)
                        return x0p, a0, a1

                    if hq == 0:
                        psX, psY = psX0, psY0
                    else:
                        psX = projf(hq, woffx, boffx, selx)
                        psY = projf(hq, woffy, boffy, sely)
                    x0p, ax0, ax1 = corner_weights(psX, sclw, nsclw,
                                                   wlm1, "x")
                    y0p, ay0, ay1 = corner_weights(psY, sclh, nsclh,
                                                   hlm1, "y")
                    idxf = p3.tile([128, HQ], F32, tag="cwxq2")
                    idxf16 = p3.tile([128, HQ], I16, tag=f"i16{hq}")
                    nc.vector.scalar_tensor_tensor(
                        idxf[:], y0p[:], sclw[:], x0p[:],
                        op0=ALU.mult, op1=ALU.add)
                    nc.vector.tensor_scalar(
                        idxf16[:], idxf[:], cbase[:], 0.0,
                        op0=ALU.add, op1=ALU.bypass)
                    # i16 transposes -> idxw 16-wrap
                    # col = ci*512 + hq*256 + m*64 + jj  (ci=h*4+a, m=pt)
                    for jj in range(HQ // 16):
                        psQ = psQp.tile([16, 128], BF16, tag="psQ")
                        nc.tensor.transpose(
                            psQ[:],
                            idxf16[:, jj * 16:(jj + 1) * 16].bitcast(BF16),
                            iden[:])
                        dst = mkap(idxw[:], [[IW, 16], [64, 4], [512, 32]],
                                   offset=hq * 256 + jj)
                        src = mkap(psQ[:].bitcast(I16),
                                   [[128, 16], [32, 4], [1, 32]])
                        if jj % 4 < 3:
                            nc.vector.tensor_copy(dst, src)
                        else:
                            nc.scalar.activation(dst, src, ACTF.Copy)
                    # ---- attention weights ----
                    psZ = projf(hq, wattn, battn, None)
                    esb = p3.tile([128, HQ], BF16, tag="cwesb")
                    nc.scalar.activation(esb[:], psZ[:], ACTF.Exp,
                                         bias=0.0, scale=1.0)
                    pss = ps3s.tile([8, HQ], F32, tag="pss")
                    for c0 in range(0, HQ, 512):
                        nc.tensor.matmul(pss[:, c0:c0 + 512], s16[:],
                                         esb[:, c0:c0 + 512],
                                         start=True, stop=True)
                    rsb = p3.tile([8, HQ], F32, tag="cwrsb")
                    nc.vector.reciprocal(rsb[:], pss[:])
                    rsbq = p3.tile([8, HQ], BF16, tag="cwrsbq")
                    nc.vector.tensor_tensor(rsbq[:], rsb[:],
                                            qmask[0:8, qs], op=ALU.mult)
                    psr2 = ps3.tile([128, HQ], F32, tag="pp")
                    for c0 in range(0, HQ, 512):
                        nc.tensor.matmul(psr2[:, c0:c0 + 512], b8c[:],
                                         rsbq[:, c0:c0 + 512],
                                         start=True, stop=True)
                    aw = p3.tile([128, HQ], BF16, tag="cwaw")
                    nc.vector.tensor_tensor(aw[:], esb[:], psr2[:],
                                            op=ALU.mult)
                    nc.vector.tensor_tensor(ay0[:], ay0[:], aw[:],
                                            op=ALU.mult)
                    nc.vector.tensor_tensor(ay1[:], ay1[:], aw[:],
                                            op=ALU.mult)
                    for s, (ax, ay) in enumerate(
                            [(ax0, ay0), (ax1, ay0),
                             (ax0, ay1), (ax1, ay1)]):
                        nc.vector.tensor_tensor(
                            w4p[s][:, qs], ax[:], ay[:], op=ALU.mult)
                    # repack w4p -> w4a_d DRAM: rows (s,ci), cols
                    # hq*4096 + m*1024 + q_l
                    for s in range(4):
                        for m in range(4):
                            eng = nc.sync if (s * 4 + m) % 2 == 0 \
                                else nc.scalar
                            eng.dma_start(
                                mkap(w4a_d[:],
                                     [[2 * HCH, 32], [1, HQ]],
                                     offset=s * 32 * 2 * HCH
                                     + hq * HCH + m * HQ),
                                mkap(w4p[s][:], [[QP, 32], [1, HQ]],
                                     offset=32 * m * QP + hq * HQ))
                # replicate idx 16 -> 128 partitions (HW reads replicas)
                for (r0, nr) in [(16, 16), (32, 32), (64, 64)]:
                    for half in range(2):
                        eng = nc.sync if half == 0 else nc.scalar
                        cs = half * (IW // 2)
                        eng.dma_start(
                            mkap(idxw[:], [[IW, nr], [1, IW // 2]],
                                 offset=r0 * IW + cs),
                            mkap(idxw[:], [[IW, nr], [1, IW // 2]],
                                 offset=(r0 - nr) * IW + cs))

            ps3.release()
            # ======== phase 4: gather / multiply / reduce =================
            with tc.tile_pool(name="p4t", bufs=2) as p4t, \
                 tc.tile_pool(name="pwc", bufs=3) as pwc, \
                 tc.tile_pool(name="psW", bufs=2, space="PSUM") as psWp, \
                 tc.tile_pool(name="psO", bufs=1, space="PSUM") as psO, \
                 tc.tile_pool(name="pOs", bufs=1) as pOs, \
                 tc.tile_pool(name="pg", bufs=3) as pg, \
                 tc.tile_pool(name="pfo", bufs=3) as pfo:
                osb = [pOs.tile([128, QP + 2048], BF16, name=f"osb{g}",
                                tag=f"osb{g}")
                       for g in range(2)]
                for g in range(2):
                    nc.scalar.memzero(osb[g][:])

                def phase5_head(m):
                    # reference reshape quirk: out row q column c takes
                    # O[m, qq, d] with u = m*7681 + qq = 8q + (c//32 slot),
                    # W_out row 32*((m+qq)%8) + d.  7681 % 8 == 1 makes the
                    # structure identical on every core (host assembles).
                    gsb = osb[m // 4]
                    grp = m % 4
                    dlt = 64 * (m % 2)
                    for j in range(3):
                        psF4 = psWp.tile([128, 1024], F32, tag="psW")
                        psF = psF4[:, 0:C]
                        for s in range(8):
                            q0 = -m - 8 * dlt + 1024 * j + s
                            col0 = 1024 + q0
                            assert 0 <= col0 and col0 + 8 * 127 < QP + 2048
                            lhsT = mkap(
                                gsb[:], [[QP + 2048, 32], [8, 128]],
                                offset=(grp * 32) * (QP + 2048) + col0)
                            kw = {}
                            if grp == 3:
                                kw["tile_position"] = (96, 0)
                            nc.tensor.matmul(
                                psF, lhsT,
                                woutc[grp * 32:grp * 32 + 32, s, :],
                                start=(s == 0), stop=(s == 7), **kw)
                        fo = pfo.tile([128, C], F32, tag="fo")
                        nc.scalar.copy(fo[:], psF)
                        eng = nc.sync if j % 2 == 0 else nc.scalar
                        eng.dma_start(out_d[m, j], fo[:])

                psO4 = None
                for ck in range(NHCH):
                    hh, a, hq = ck // 8, (ck % 8) // 2, ck % 2
                    ci = hh * 4 + a
                    grp = hh % 4
                    if grp == 0 and a == 0 and hq == 0:
                        psO4 = psO.tile([128, QP], F32, tag="psO4")
                    g = pg.tile([128, 1, HCH], BF16, tag="g")
                    nc.gpsimd.dma_gather(
                        g[:], maps_d[hh],
                        idxw[:, ck * (HCH // 16):(ck + 1) * (HCH // 16)],
                        HCH, HCH, 128,
                        transpose=True, single_packet=False)
                    # replicate weights: wc[s*32+ch, :] = w4a_d[s*32+ci,
                    # hq*4096 + :]
                    wc = pwc.tile([128, HCH], BF16, tag="wc")
                    for half in range(2):
                        eng = nc.sync if half == 0 else nc.scalar
                        eng.dma_start(
                            mkap(wc[:], [[HCH, 128], [1, HCH // 2]],
                                 offset=half * (HCH // 2)),
                            mkap(w4a_d[:],
                                 [[32 * 2 * HCH, 4], [0, 32],
                                  [1, HCH // 2]],
                                 offset=ci * 2 * HCH + hq * HCH
                                 + half * (HCH // 2)))
                    tt = p4t.tile([128, HCH], BF16, tag="tt")
                    nc.vector.tensor_tensor(tt[:], g[:, 0, :], wc[:],
                                            op=ALU.mult)
                    for m4 in range(4):
                        for j2 in range(2):
                            kw = {}
                            if grp == 3:
                                kw["tile_position"] = (0, 96)
                            cs = slice(hq * 1024 + j2 * 512,
                                       hq * 1024 + (j2 + 1) * 512)
                            nc.tensor.matmul(
                                psO4[grp * 32:(grp + 1) * 32, cs],
                                r128[:],
                                tt[:, m4 * 1024 + j2 * 512:
                                   m4 * 1024 + (j2 + 1) * 512],
                                start=(a == 0 and m4 == 0),
                                stop=(a == 3 and m4 == 3), **kw)
                    if a == 3 and hq == 1:
                        nc.scalar.activation(
                            osb[hh // 4][grp * 32:(grp + 1) * 32,
                                         1024:1024 + QP],
                            psO4[grp * 32:(grp + 1) * 32, :], ACTF.Copy)
                    if ck >= 11 and (ck - 11) % 8 == 0:
                        phase5_head((ck - 11) // 8)

                # ======== phase 5 tail ====================================
                phase5_head(7)
            pw.release()
    nc.compile()
    return nc


# ---------------------------------------------------------------- host side
_CACHE = {}


def _consts(W_off, b_off, W_attn, b_attn, W_val, b_val, W_out, b_out):
    M = NUM_HEADS
    # partition layout c = b*32 + h*4 + a  (old: h*16 + a*4 + b)
    woff = np.asarray(W_off, np.float32).reshape(C, M, 4, 4, 2)
    woff = np.transpose(woff, (0, 3, 1, 2, 4))          # (C, b, h, a, 2)
    wattn = np.asarray(W_attn, np.float32).reshape(C, M, 4, 4)
    # partition (b, h, a) holds attention logit (level=b, point=a) so that
    # sample (h, a, b) pairs with aw(level=b, point=a)  (reference quirk)
    wattn = np.transpose(wattn, (0, 2, 1, 3))           # (C, l, h, k)
    boff = np.asarray(b_off, np.float32).reshape(M, 4, 4, 2)
    boff = np.transpose(boff, (2, 0, 1, 3))             # (b, h, a, 2)
    battn = np.asarray(b_attn, np.float32).reshape(M, 4, 4)
    battn = np.transpose(battn, (1, 0, 2))              # (l, h, k)
    cm = {}
    cm["woffx"] = np.ascontiguousarray(woff[..., 0].reshape(C, 128))
    cm["woffy"] = np.ascontiguousarray(woff[..., 1].reshape(C, 128))
    cm["wattn"] = np.ascontiguousarray(wattn.reshape(C, 128))
    cm["boffx"] = np.ascontiguousarray(boff[..., 0].reshape(1, 128)).astype(BF)
    cm["boffy"] = np.ascontiguousarray(boff[..., 1].reshape(1, 128)).astype(BF)
    cm["battn"] = np.ascontiguousarray(battn.reshape(1, 128)).astype(BF)
    cm["wval"] = np.asarray(W_val, np.float32).astype(BF)
    cm["bval"] = np.asarray(b_val, np.float32).reshape(1, C).astype(BF)
    wof = np.asarray(W_out, np.float32).reshape(8, 32, C).transpose(1, 0, 2)
    cm["wout"] = np.ascontiguousarray(
        np.broadcast_to(wof[None], (4, 32, 8, C)).reshape(128, 8, C)
    ).astype(BF)
    sel = np.zeros((2, 128), np.float32)
    sel[0] = 1.0
    cm["selx"] = sel
    cm["sely"] = sel[::-1].copy()
    cm["onesq"] = np.ones((1, QP), np.float32).astype(BF)
    cm["onesbf"] = np.ones((1, 128), np.float32).astype(BF)
    cmk = np.zeros((1, NVPAD), np.float32)
    cmk[0, :NV] = 1.0
    cm["cmask"] = cmk.astype(BF)
    r = np.zeros((128, 32), np.float32)
    for p in range(128):
        r[p, p % 32] = 1.0
    cm["r128"] = r.astype(BF)
    s16 = np.zeros((128, 8), np.float32)
    b8 = np.zeros((8, 128), np.float32)
    for p in range(128):
        h = (p % 32) // 4
        s16[p, h] = 1.0
        b8[h, p] = 1.0
    cm["s16"] = s16.astype(BF)
    cm["b8"] = b8.astype(BF)
    lvl_of_p = np.arange(128) % 4                       # level = a = c%4
    Wl = np.array([LEVELS[l][1] for l in lvl_of_p], np.float32)
    Hl = np.array([LEVELS[l][0] for l in lvl_of_p], np.float32)
    cb = np.array([CELLSTART[l] - LEVELS[l][1] - 1 for l in lvl_of_p],
                  np.float32)
    cm["iden"] = np.eye(128, dtype=np.float32).astype(BF)
    cm["sclw"] = Wl.reshape(128, 1)
    cm["sclh"] = Hl.reshape(128, 1)
    cm["nsclw"] = (-Wl).reshape(128, 1)
    cm["nsclh"] = (-Hl).reshape(128, 1)
    cm["wlm1"] = (Wl - 1).reshape(128, 1)
    cm["hlm1"] = (Hl - 1).reshape(128, 1)
    cm["cbase"] = cb.reshape(128, 1)
    return cm


def kernel(**inputs):
    if "nc" not in _CACHE:
        _CACHE["nc"] = build_nc()
    nc = _CACHE["nc"]
    cm = _consts(inputs["W_off"], inputs["b_off"], inputs["W_attn"],
                 inputs["b_attn"], inputs["W_val"], inputs["b_val"],
                 inputs["W_out"], inputs["b_out"])
    query = np.asarray(inputs["query"], np.float32)
    refp = np.asarray(inputs["reference_points"], np.float32)
    value = np.asarray(inputs["value"], np.float32)
    vpad = np.zeros((BS, NVPAD, C), np.float32)
    vpad[:, :NV] = value
    qpad = np.zeros((BS, 4 * QP, C), np.float32)
    qpad[:, :NQ] = query
    rpad = np.zeros((BS, 4 * QP, 2), np.float32)
    rpad[:, :NQ] = refp
    vT_b = []
    for b in range(BS):
        vT = vpad[b].T.reshape(2, 128, NVPAD).transpose(1, 0, 2)
        vT_b.append(np.ascontiguousarray(vT.astype(BF)))
    in_maps = []
    for core in range(NCORES):
        b, qc = core // 4, core % 4
        nvalid = min(QP, max(0, NQ - qc * QP))
        qm = np.zeros((128, QP), np.float32)
        qm[:, :nvalid] = 1.0
        qm = qm.astype(BF)
        qs = qpad[b, qc * QP:(qc + 1) * QP]
        rs = rpad[b, qc * QP:(qc + 1) * QP]
        qT = qs.T.reshape(2, 128, QP).transpose(1, 0, 2)
        m = {"qT": np.ascontiguousarray(qT),
             "refT": np.ascontiguousarray(rs.T),
             "vT": vT_b[b],
             "qmask": qm}
        m.update({k: np.ascontiguousarray(v) for k, v in cm.items()})
        in_maps.append(m)
    res = run_bass_kernel_spmd(nc, in_maps, list(range(NCORES)),
                               **_CACHE.get("run_kw", {}))
    _CACHE["last_res"] = res
    out = np.zeros((BS, NQ + 512, C), np.float32)
    for core in range(NCORES):
        b, qc = core // 4, core % 4
        slab = res.results[core]["out"]        # [8, 3, 128, 256]
        for m in range(NUM_HEADS):
            dlt = 64 * (m % 2)
            tb = 960 * m + 256 * qc - dlt      # absolute tile base
            for j in range(3):
                if m % 2 == 0:
                    row_lo, row_hi = 0, (128, 128, 32)[j]
                else:
                    row_lo, row_hi = ((64, 0, 0)[j], (128, 128, 96)[j])
                r0 = tb + 128 * j + row_lo
                r1 = tb + 128 * j + row_hi
                r1c = min(r1, NQ + 512)
                if r0 < 0 or r1c <= r0:
                    continue
                out[b, r0:r1c] += slab[m, j, row_lo:row_lo + (r1c - r0)]
    out = out[:, :NQ] + np.asarray(inputs["b_out"], np.float32)[None, None]
    return out


# revision 16
# speedup vs baseline: 1.2417x; 1.0524x over previous
"""Multi-scale deformable attention on 8 Trainium2 NeuronCores.

Sharding: (batch x query-quarter) -> 8 cores; each core does all 8 heads for
2048 queries of one batch (value projection recomputed per core).

v3 design (cost-model-aware):
  - v-proj -> vsb_full (cell-major bf16) -> vproj_d DRAM -> 16 cheap
    DRAM->DRAM DMAs build the per-head quad maps (maps_all[8, ES, 128]).
  - phase 3 computes positions/weights per query-half; floors use trunc
    (xpp >= 0.5 always); clamp+scale on Act (Relu chain); idx transposed to
    the 16-wrap via i16 PE transposes; idxw partitions 16..127 are zeroed
    once (executor only reads [:16]).
  - weights repacked to w4all_d DRAM; per half-chunk a 0-stride-src DMA
    replicates rows to wc[128, 4096] bf16 in SBUF (sync+scalar queues).
  - 64 half-chunks (head, point, q-half): Pool gather -> DVE single
    multiply -> PE reduce into psO4 -> Act copy to osb -> per-head out-proj.
"""
import sys

sys.path.insert(0, '/opt/trn_rl_repo')

import numpy as np
import ml_dtypes

import concourse.bass as bass
import concourse.bacc as bacc
import concourse.mybir as mybir
import concourse.tile as tile
from concourse.bass_utils import run_bass_kernel_spmd

dt = mybir.dt
F32, BF16, I16, I32 = dt.float32, dt.bfloat16, dt.int16, dt.int32
F32R = dt.float32r
ALU = mybir.AluOpType
ACTF = mybir.ActivationFunctionType
BF = ml_dtypes.bfloat16

# ---------------------------------------------------------------- geometry
LEVELS = ((76, 76), (38, 38), (19, 19), (10, 10))
NUM_HEADS, NUM_LEVELS, NUM_POINTS = 8, 4, 4
C, D = 256, 32
BS, NQ = 2, 7681
QP = 2048                      # queries per core (padded)
HQ = QP // 2
NCORES = 8
SAMP = NUM_HEADS * NUM_LEVELS * NUM_POINTS * QP       # 262144
IW = SAMP // 16                                       # idxw cols = 16384
STARTS = [0]
for (_h, _w) in LEVELS:
    STARTS.append(STARTS[-1] + _h * _w)
NV = STARTS[-1]                # 7681
NVPAD = 7808                   # 61 * 128
GAP = 256
CELLSTART = []
_pos = GAP
for (_h, _w) in LEVELS:
    CELLSTART.append(_pos)
    _pos += _h * _w + GAP
ES = ((_pos - GAP) // 128 + 1) * 128      # cell-space rows (8832)
TS2 = ES // 128                           # vsb_full blocks (69)
HCH = 4096                     # half-chunk samples
NHCH = SAMP // HCH             # 64


def align_down(x, a=128):
    return (x // a) * a


def mkap(base_ap, ap_list, offset=None):
    ap = base_ap.copy()
    ap.ap = mybir.VecI64Pair([list(x) for x in ap_list])
    if offset is not None:
        ap.offset = offset
    return ap


# ---------------------------------------------------------------- program
def build_nc():
    nc = bacc.Bacc("TRN2", target_bir_lowering=False)
    qT_d = nc.dram_tensor("qT", [128, 2, QP], F32, kind="ExternalInput")
    refT_d = nc.dram_tensor("refT", [2, QP], F32, kind="ExternalInput")
    vT_d = nc.dram_tensor("vT", [128, 2, NVPAD], BF16, kind="ExternalInput")
    woffx_d = nc.dram_tensor("woffx", [C, 128], F32, kind="ExternalInput")
    woffy_d = nc.dram_tensor("woffy", [C, 128], F32, kind="ExternalInput")
    wattn_d = nc.dram_tensor("wattn", [C, 128], F32, kind="ExternalInput")
    boffx_d = nc.dram_tensor("boffx", [1, 128], BF16, kind="ExternalInput")
    boffy_d = nc.dram_tensor("boffy", [1, 128], BF16, kind="ExternalInput")
    battn_d = nc.dram_tensor("battn", [1, 128], BF16, kind="ExternalInput")
    wval_d = nc.dram_tensor("wval", [C, C], BF16, kind="ExternalInput")
    bval_d = nc.dram_tensor("bval", [1, C], BF16, kind="ExternalInput")
    cmask_d = nc.dram_tensor("cmask", [1, NVPAD], BF16, kind="ExternalInput")
    wout_d = nc.dram_tensor("wout", [128, 8, C], BF16, kind="ExternalInput")
    selx_d = nc.dram_tensor("selx", [2, 128], F32, kind="ExternalInput")
    sely_d = nc.dram_tensor("sely", [2, 128], F32, kind="ExternalInput")
    onesq_d = nc.dram_tensor("onesq", [1, QP], BF16, kind="ExternalInput")
    onesbf_d = nc.dram_tensor("onesbf", [1, 128], BF16, kind="ExternalInput")
    r128_d = nc.dram_tensor("r128", [128, 32], BF16, kind="ExternalInput")
    s16_d = nc.dram_tensor("s16", [128, 8], BF16, kind="ExternalInput")
    b8_d = nc.dram_tensor("b8", [8, 128], BF16, kind="ExternalInput")
    sclw_d = nc.dram_tensor("sclw", [128, 1], F32, kind="ExternalInput")
    sclh_d = nc.dram_tensor("sclh", [128, 1], F32, kind="ExternalInput")
    nsclw_d = nc.dram_tensor("nsclw", [128, 1], F32, kind="ExternalInput")
    nsclh_d = nc.dram_tensor("nsclh", [128, 1], F32, kind="ExternalInput")
    wlm1_d = nc.dram_tensor("wlm1", [128, 1], F32, kind="ExternalInput")
    hlm1_d = nc.dram_tensor("hlm1", [128, 1], F32, kind="ExternalInput")
    cbase_d = nc.dram_tensor("cbase", [128, 1], F32, kind="ExternalInput")
    qmask_d = nc.dram_tensor("qmask", [128, QP], BF16, kind="ExternalInput")
    iden_d = nc.dram_tensor("iden", [128, 128], BF16, kind="ExternalInput")
    out_d = nc.dram_tensor("out", [8, 3, 128, C], F32,
                           kind="ExternalOutput")
    vproj_d = nc.dram_tensor("vproj", [ES + 128, C], BF16)
    maps_d = nc.dram_tensor("mapsall", [NUM_HEADS, ES, 128], BF16)
    w4a_d = nc.dram_tensor("w4ad", [128, 2 * HCH], BF16)

    with tile.TileContext(nc) as tc:
        with tc.tile_pool(name="const", bufs=1) as cpool:
            selx = cpool.tile([2, 128], F32)
            sely = cpool.tile([2, 128], F32)
            onesq = cpool.tile([1, QP], BF16)
            onesbf = cpool.tile([1, 128], BF16)
            r128 = cpool.tile([128, 32], BF16)
            s16 = cpool.tile([128, 8], BF16)
            b8c = cpool.tile([8, 128], BF16)
            sclw = cpool.tile([128, 1], F32)
            sclh = cpool.tile([128, 1], F32)
            nsclw = cpool.tile([128, 1], F32)
            nsclh = cpool.tile([128, 1], F32)
            wlm1 = cpool.tile([128, 1], F32)
            hlm1 = cpool.tile([128, 1], F32)
            cbase = cpool.tile([128, 1], F32)
            woffx = cpool.tile([128, 2, 128], F32)
            woffy = cpool.tile([128, 2, 128], F32)
            wattn = cpool.tile([128, 2, 128], F32)
            boffx = cpool.tile([1, 128], BF16)
            boffy = cpool.tile([1, 128], BF16)
            battn = cpool.tile([1, 128], BF16)
            wvalb = cpool.tile([128, 2, C], BF16)
            bvalb = cpool.tile([1, C], BF16)
            cmask = cpool.tile([1, NVPAD], BF16)
            woutc = cpool.tile([128, 8, C], BF16)
            qmask = cpool.tile([128, QP], BF16)
            iden = cpool.tile([128, 128], BF16)
            qT = cpool.tile([128, 2, QP], F32)
            refT = cpool.tile([2, QP], F32)
            # big loads split across sync/scalar
            nc.sync.dma_start(qT[:, 0], qT_d[:, 0])
            nc.scalar.dma_start(qT[:, 1], qT_d[:, 1])
            nc.sync.dma_start(refT[:], refT_d[:])
            for t, s in [(woffx, woffx_d), (woffy, woffy_d),
                         (wattn, wattn_d), (wvalb, wval_d)]:
                nc.scalar.dma_start(
                    t[:], s[:].rearrange("(h p) x -> p h x", p=128))
            for t, s in [(onesq, onesq_d), (boffx, boffx_d),
                         (boffy, boffy_d), (selx, selx_d),
                         (sely, sely_d), (sclw, sclw_d), (sclh, sclh_d),
                         (nsclw, nsclw_d), (nsclh, nsclh_d),
                         (wlm1, wlm1_d), (hlm1, hlm1_d),
                         (cbase, cbase_d), (iden, iden_d),
                         (qmask, qmask_d)]:
                nc.sync.dma_start(t[:], s[:])
            for t, s in [(battn, battn_d), (bvalb, bval_d),
                         (cmask, cmask_d), (onesbf, onesbf_d),
                         (s16, s16_d), (b8c, b8_d),
                         (r128, r128_d), (woutc, wout_d)]:
                nc.scalar.dma_start(t[:], s[:])

            ps3 = tc.alloc_tile_pool(name="ps3", bufs=2, space="PSUM")

            def projf(hq, wof, bof, sel):
                ps = ps3.tile([128, HQ], F32, tag="pp")
                for c0 in range(0, HQ, 512):
                    cs = slice(hq * HQ + c0, hq * HQ + c0 + 512)
                    po = ps[:, c0:c0 + 512]
                    for half in range(2):
                        nc.tensor.matmul(
                            po, wof[:, half, :], qT[:, half, cs],
                            start=(half == 0), stop=False)
                    nc.tensor.matmul(po, bof[:], onesq[:, cs],
                                     start=False, stop=(sel is None))
                    if sel is not None:
                        nc.tensor.matmul(po, sel[:], refT[:, cs],
                                         start=False, stop=True)
                return ps

            # hq0 x/y projections run on PE before anything else
            psX0 = projf(0, woffx, boffx, selx)
            psY0 = projf(0, woffy, boffy, sely)

            # ======== phase 2: vT load, v-proj, vproj_d, maps ============
            pVS = tc.alloc_tile_pool(name="pVS", bufs=1, side="right")
            vsb = pVS.tile([128, TS2, C], BF16)
            # zero the whole staging tile once (gaps + pad stay zero)
            nc.scalar.memzero(vsb[:])
            pVT = tc.alloc_tile_pool(name="pVT", bufs=1, side="right")
            p1t = tc.alloc_tile_pool(name="p1t", bufs=2)
            ps1 = tc.alloc_tile_pool(name="ps1", bufs=3, space="PSUM")
            vT = pVT.tile([128, 2, NVPAD], BF16)
            nc.sync.dma_start(vT[:, 0], vT_d[:, 0])
            nc.scalar.dma_start(vT[:, 1], vT_d[:, 1])
            ncp = [0]
            for lvl in range(NUM_LEVELS):
                hw = LEVELS[lvl][0] * LEVELS[lvl][1]
                shift = CELLSTART[lvl] - STARTS[lvl]  # mult of 128
                c0 = STARTS[lvl]
                while c0 < STARTS[lvl] + hw:
                    tbeg = align_down(c0)
                    cend = min(tbeg + 128, STARTS[lvl] + hw)
                    lo, hi = c0 - tbeg, cend - tbeg
                    psv = ps1.tile([128, C], F32, tag="psv")
                    for half in range(2):
                        nc.tensor.matmul(
                            psv[:], vT[:, half, tbeg:tbeg + 128],
                            wvalb[:, half, :], start=(half == 0),
                            stop=False)
                    nc.tensor.matmul(psv[:], cmask[:, tbeg:tbeg + 128],
                                     bvalb[:], start=False, stop=True)
                    sp = c0 + shift
                    assert sp % 128 == lo
                    blk = (tbeg + shift) // 128
                    if lo == 0 and hi == 128:
                        eng = nc.scalar if ncp[0] % 2 == 0 else nc.vector
                        ncp[0] += 1
                        if eng is nc.scalar:
                            eng.copy(vsb[:, blk, :], psv[:])
                        else:
                            eng.tensor_copy(vsb[:, blk, :], psv[:])
                    else:
                        vstage = p1t.tile([128, C], BF16, tag="vstage")
                        nc.scalar.copy(vstage[:], psv[:])
                        dst = mkap(vsb[:], [[TS2 * C, hi - lo], [1, C]],
                                   offset=lo * (TS2 * C) + blk * C)
                        src = mkap(vstage[:], [[C, hi - lo], [1, C]],
                                   offset=lo * C)
                        nc.sync.dma_start(dst, src)
                    c0 = cend
            # vproj_d write: 4 slices alternating queues
            bsl = [(0, 18), (18, 17), (35, 17), (52, TS2 - 52)]
            for i, (b0, nb) in enumerate(bsl):
                eng = nc.sync if i % 2 == 0 else nc.scalar
                eng.dma_start(
                    mkap(vproj_d[:], [[C, 128], [128 * C, nb], [1, C]],
                         offset=b0 * 128 * C),
                    mkap(vsb[:], [[TS2 * C, 128], [C, nb], [1, C]],
                         offset=b0 * C))
            nc.sync.dma_start(
                mkap(vproj_d[:], [[C, 128], [1, C]],
                     offset=ES * C),
                mkap(vsb[:], [[TS2 * C, 128], [1, C]]))
            # maps: one DRAM->DRAM DMA per (lvl, corner); ranges cover every
            # row of maps_d so the finite-checker never sees uninit DRAM
            MB = [0] + [CELLSTART[l] - LEVELS[l][1] - 1 for l in (1, 2, 3)] \
                + [ES]
            for lvl, (H, W) in enumerate(LEVELS):
                g0 = MB[lvl]
                n = MB[lvl + 1] - MB[lvl]
                for s, dl in enumerate([0, 1, W, W + 1]):
                    eng = nc.sync if (lvl * 4 + s) % 2 == 0 else nc.scalar
                    eng.dma_start(
                        mkap(maps_d[:], [[128, n], [ES * 128, 8], [1, 32]],
                             offset=g0 * 128 + s * 32),
                        mkap(vproj_d[:], [[C, n], [32, 8], [1, 32]],
                             offset=(g0 + dl) * C))
            ps1.release()
            p1t.release()
            pVT.release()
            pVS.release()

            # ======== phase 3: positions / weights / indices ==========
            pw = tc.alloc_tile_pool(name="pw", bufs=1)
            idxw = pw.tile([128, IW], I16)
            # executor reads idx partitions [:16] but asserts all 128 are
            # in-range: zero the tile once (copies then fill rows 0..15)
            nc.scalar.memzero(idxw[:])
            w4p = [pw.tile([128, QP], BF16, name=f"w4p{s}")
                   for s in range(4)]
            with tc.tile_pool(name="p3", bufs=1) as p3, \
                 tc.tile_pool(name="ps3s", bufs=1, space="PSUM") as ps3s, \
                 tc.tile_pool(name="psQ", bufs=2, space="PSUM") as psQp:
                for hq in range(2):
                    qs = slice(hq * HQ, (hq + 1) * HQ)

                    def corner_weights(psP, sclp, nsclp, wm1, pref):
                        rr = p3.tile([128, HQ], F32, tag="cwr")
                        xq = p3.tile([128, HQ], F32, tag=f"{pref}xq")
                        xi = p3.tile([128, HQ], I32, tag="cwxi")
                        x0p = p3.tile([128, HQ], F32, tag=f"{pref}x0p")
                        wx = p3.tile([128, HQ], F32, tag=f"{pref}wx")
                        v0 = p3.tile([128, HQ], BF16, tag="cwv0")
                        v1 = p3.tile([128, HQ], BF16, tag="cwv1")
                        wxf = p3.tile([128, HQ], BF16, tag="cwwxf")
                        a0 = p3.tile([128, HQ], BF16, tag=f"{pref}a0")
                        a1 = p3.tile([128, HQ], BF16, tag=f"{pref}a1")
                        # rr = max(1-p, 0); xq = max(scl - scl*rr, 0)
                        #   = scl*clamp01(p)
                        nc.scalar.activation(rr[:], psP[:], ACTF.Relu,
                                             bias=1.0, scale=-1.0)
                        nc.scalar.activation(xq[:], rr[:], ACTF.Relu,
                                             bias=sclp[:], scale=nsclp[:])
                        # x0p = floor(xq + 0.5) (trunc ok: arg >= 0.5)
                        nc.vector.tensor_scalar(xi[:], xq[:], 0.5, 0.0,
                                                op0=ALU.add,
                                                op1=ALU.bypass)
                        nc.vector.tensor_copy(x0p[:], xi[:])
                        # rounding-mode-agnostic floor: subtract 1 where
                        # the int conversion rounded up
                        t1 = p3.tile([128, HQ], F32, tag="cwt1")
                        nc.vector.scalar_tensor_tensor(
                            t1[:], xq[:], 0.5, x0p[:],
                            op0=ALU.add, op1=ALU.is_lt)
                        nc.vector.tensor_tensor(x0p[:], x0p[:], t1[:],
                                                op=ALU.subtract)
                        nc.vector.scalar_tensor_tensor(
                            wx[:], xq[:], 0.5, x0p[:],
                            op0=ALU.add, op1=ALU.subtract)
                        nc.vector.tensor_scalar(
                            v0[:], x0p[:], 1.0, 0.0,
                            op0=ALU.is_ge, op1=ALU.bypass)
                        nc.vector.tensor_scalar(
                            v1[:], x0p[:], wm1[:], 0.0,
                            op0=ALU.is_le, op1=ALU.bypass)
                        nc.vector.tensor_tensor(a1[:], wx[:], v1[:],
                                                op=ALU.mult)
                        nc.vector.tensor_scalar(
                            wxf[:], wx[:], -1.0, 1.0,
                            op0=ALU.mult, op1=ALU.add)
                        nc.vector.tensor_tensor(a0[:], wxf[:], v0[:],
                                                op=ALU.mult)
                        return x0p, a0, a1

                    if hq == 0:
                        psX, psY = psX0, psY0
                    else:
                        psX = projf(hq, woffx, boffx, selx)
                        psY = projf(hq, woffy, boffy, sely)
                    x0p, ax0, ax1 = corner_weights(psX, sclw, nsclw,
                                                   wlm1, "x")
                    y0p, ay0, ay1 = corner_weights(psY, sclh, nsclh,
                                                   hlm1, "y")
                    idxf = p3.tile([128, HQ], F32, tag="cwxq2")
                    idxf16 = p3.tile([128, HQ], I16, tag=f"i16{hq}")
                    nc.vector.scalar_tensor_tensor(
                        idxf[:], y0p[:], sclw[:], x0p[:],
                        op0=ALU.mult, op1=ALU.add)
                    nc.vector.tensor_scalar(
                        idxf16[:], idxf[:], cbase[:], 0.0,
                        op0=ALU.add, op1=ALU.bypass)
                    # i16 transposes -> idxw 16-wrap
                    # col = ci*512 + hq*256 + m*64 + jj  (ci=h*4+a, m=pt)
                    for jj in range(HQ // 16):
                        psQ = psQp.tile([16, 128], BF16, tag="psQ")
                        nc.tensor.transpose(
                            psQ[:],
                            idxf16[:, jj * 16:(jj + 1) * 16].bitcast(BF16),
                            iden[:])
                        dst = mkap(idxw[:], [[IW, 16], [64, 4], [256, 32]],
                                   offset=hq * 8192 + jj)
                        src = mkap(psQ[:].bitcast(I16),
                                   [[128, 16], [32, 4], [1, 32]])
                        if jj % 4 < 3:
                            nc.vector.tensor_copy(dst, src)
                        else:
                            nc.scalar.activation(dst, src, ACTF.Copy)
                    # ---- attention weights ----
                    psZ = projf(hq, wattn, battn, None)
                    esb = p3.tile([128, HQ], BF16, tag="cwesb")
                    nc.scalar.activation(esb[:], psZ[:], ACTF.Exp,
                                         bias=0.0, scale=1.0)
                    pss = ps3s.tile([8, HQ], F32, tag="pss")
                    for c0 in range(0, HQ, 512):
                        nc.tensor.matmul(pss[:, c0:c0 + 512], s16[:],
                                         esb[:, c0:c0 + 512],
                                         start=True, stop=True)
                    rsb = p3.tile([8, HQ], F32, tag="cwrsb")
                    nc.vector.reciprocal(rsb[:], pss[:])
                    rsbq = p3.tile([8, HQ], BF16, tag="cwrsbq")
                    nc.vector.tensor_tensor(rsbq[:], rsb[:],
                                            qmask[0:8, qs], op=ALU.mult)
                    psr2 = ps3.tile([128, HQ], F32, tag="pp")
                    for c0 in range(0, HQ, 512):
                        nc.tensor.matmul(psr2[:, c0:c0 + 512], b8c[:],
                                         rsbq[:, c0:c0 + 512],
                                         start=True, stop=True)
                    aw = p3.tile([128, HQ], BF16, tag="cwaw")
                    nc.vector.tensor_tensor(aw[:], esb[:], psr2[:],
                                            op=ALU.mult)
                    nc.vector.tensor_tensor(ay0[:], ay0[:], aw[:],
                                            op=ALU.mult)
                    nc.vector.tensor_tensor(ay1[:], ay1[:], aw[:],
                                            op=ALU.mult)
                    for s, (ax, ay) in enumerate(
                            [(ax0, ay0), (ax1, ay0),
                             (ax0, ay1), (ax1, ay1)]):
                        nc.vector.tensor_tensor(
                            w4p[s][:, qs], ax[:], ay[:], op=ALU.mult)
                    # repack w4p -> w4a_d DRAM: rows (s,ci), cols
                    # hq*4096 + m*1024 + q_l
                    for s in range(4):
                        for m in range(4):
                            eng = nc.sync if (s * 4 + m) % 2 == 0 \
                                else nc.scalar
                            eng.dma_start(
                                mkap(w4a_d[:],
                                     [[2 * HCH, 32], [1, HQ]],
                                     offset=s * 32 * 2 * HCH
                                     + hq * HCH + m * HQ),
                                mkap(w4p[s][:], [[QP, 32], [1, HQ]],
                                     offset=32 * m * QP + hq * HQ))
                    # replicate this hq's idx cols 16 -> 128 partitions
                    for (r0, nr) in [(16, 16), (32, 32), (64, 64)]:
                        for half in range(2):
                            eng = nc.sync if half == 0 else nc.scalar
                            cs = hq * 8192 + half * 4096
                            eng.dma_start(
                                mkap(idxw[:], [[IW, nr], [1, 4096]],
                                     offset=r0 * IW + cs),
                                mkap(idxw[:], [[IW, nr], [1, 4096]],
                                     offset=(r0 - nr) * IW + cs))

            ps3.release()
            # ======== phase 4: gather / multiply / reduce =================
            with tc.tile_pool(name="p4t", bufs=2) as p4t, \
                 tc.tile_pool(name="pwc", bufs=3) as pwc, \
                 tc.tile_pool(name="psW", bufs=2, space="PSUM") as psWp, \
                 tc.tile_pool(name="psO", bufs=1, space="PSUM") as psO, \
                 tc.tile_pool(name="pOs", bufs=1) as pOs, \
                 tc.tile_pool(name="pg", bufs=3) as pg, \
                 tc.tile_pool(name="pfo", bufs=3) as pfo:
                osb = [pOs.tile([128, QP + 2048], BF16, name=f"osb{g}",
                                tag=f"osb{g}")
                       for g in range(2)]
                for g in range(2):
                    nc.scalar.memzero(osb[g][:])

                def phase5_head(m):
                    # reference reshape quirk: out row q column c takes
                    # O[m, qq, d] with u = m*7681 + qq = 8q + (c//32 slot),
                    # W_out row 32*((m+qq)%8) + d.  7681 % 8 == 1 makes the
                    # structure identical on every core (host assembles).
                    gsb = osb[m // 4]
                    grp = m % 4
                    dlt = 64 * (m % 2)
                    for j in range(3):
                        psF4 = psWp.tile([128, 1024], F32, tag="psW")
                        psF = psF4[:, 0:C]
                        for s in range(8):
                            q0 = -m - 8 * dlt + 1024 * j + s
                            col0 = 1024 + q0
                            assert 0 <= col0 and col0 + 8 * 127 < QP + 2048
                            lhsT = mkap(
                                gsb[:], [[QP + 2048, 32], [8, 128]],
                                offset=(grp * 32) * (QP + 2048) + col0)
                            kw = {}
                            if grp == 3:
                                kw["tile_position"] = (96, 0)
                            nc.tensor.matmul(
                                psF, lhsT,
                                woutc[grp * 32:grp * 32 + 32, s, :],
                                start=(s == 0), stop=(s == 7), **kw)
                        fo = pfo.tile([128, C], F32, tag="fo")
                        nc.scalar.copy(fo[:], psF)
                        eng = nc.sync if j % 2 == 0 else nc.scalar
                        eng.dma_start(out_d[m, j], fo[:])

                psT = {}
                for ck in range(NHCH):
                    hq, hh, a = ck // 32, (ck % 32) // 4, ck % 4
                    ci = hh * 4 + a
                    grp = hh % 4
                    if grp == 0 and a == 0:
                        psT[hh // 4] = psO.tile(
                            [128, HQ], F32, name=f"psO4{hh // 4}",
                            tag=f"psO4{hh // 4}")
                    psO4 = psT[hh // 4]
                    i0 = hq * 8192 + ci * 256
                    g = pg.tile([128, 1, HCH], BF16, tag="g")
                    nc.gpsimd.dma_gather(
                        g[:], maps_d[hh],
                        idxw[:, i0:i0 + 256],
                        HCH, HCH, 128,
                        transpose=True, single_packet=False)
                    # replicate weights: wc[s*32+ch, :] = w4a_d[s*32+ci,
                    # hq*4096 + :]
                    wc = pwc.tile([128, HCH], BF16, tag="wc")
                    for half in range(2):
                        eng = nc.sync if half == 0 else nc.scalar
                        eng.dma_start(
                            mkap(wc[:], [[HCH, 128], [1, HCH // 2]],
                                 offset=half * (HCH // 2)),
                            mkap(w4a_d[:],
                                 [[32 * 2 * HCH, 4], [0, 32],
                                  [1, HCH // 2]],
                                 offset=ci * 2 * HCH + hq * HCH
                                 + half * (HCH // 2)))
                    tt = p4t.tile([128, HCH], BF16, tag="tt")
                    nc.vector.tensor_tensor(tt[:], g[:, 0, :], wc[:],
                                            op=ALU.mult)
                    for m4 in range(4):
                        for j2 in range(2):
                            kw = {}
                            if grp == 3:
                                kw["tile_position"] = (0, 96)
                            cs = slice(j2 * 512, (j2 + 1) * 512)
                            nc.tensor.matmul(
                                psO4[grp * 32:(grp + 1) * 32, cs],
                                r128[:],
                                tt[:, m4 * 1024 + j2 * 512:
                                   m4 * 1024 + (j2 + 1) * 512],
                                start=(a == 0 and m4 == 0),
                                stop=(a == 3 and m4 == 3), **kw)
                    if a == 3:
                        nc.scalar.activation(
                            osb[hh // 4][grp * 32:(grp + 1) * 32,
                                         1024 + hq * HQ:
                                         1024 + (hq + 1) * HQ],
                            psO4[grp * 32:(grp + 1) * 32, :], ACTF.Copy)
                    if ck >= 37 and (ck - 37) % 4 == 0 and ck <= 61:
                        phase5_head((ck - 37) // 4)

                # ======== phase 5 tail ====================================
                phase5_head(7)
            pw.release()
    nc.compile()
    return nc


# ---------------------------------------------------------------- host side
_CACHE = {}


def _consts(W_off, b_off, W_attn, b_attn, W_val, b_val, W_out, b_out):
    M = NUM_HEADS
    # partition layout c = b*32 + h*4 + a  (old: h*16 + a*4 + b)
    woff = np.asarray(W_off, np.float32).reshape(C, M, 4, 4, 2)
    woff = np.transpose(woff, (0, 3, 1, 2, 4))          # (C, b, h, a, 2)
    wattn = np.asarray(W_attn, np.float32).reshape(C, M, 4, 4)
    # partition (b, h, a) holds attention logit (level=b, point=a) so that
    # sample (h, a, b) pairs with aw(level=b, point=a)  (reference quirk)
    wattn = np.transpose(wattn, (0, 2, 1, 3))           # (C, l, h, k)
    boff = np.asarray(b_off, np.float32).reshape(M, 4, 4, 2)
    boff = np.transpose(boff, (2, 0, 1, 3))             # (b, h, a, 2)
    battn = np.asarray(b_attn, np.float32).reshape(M, 4, 4)
    battn = np.transpose(battn, (1, 0, 2))              # (l, h, k)
    cm = {}
    cm["woffx"] = np.ascontiguousarray(woff[..., 0].reshape(C, 128))
    cm["woffy"] = np.ascontiguousarray(woff[..., 1].reshape(C, 128))
    cm["wattn"] = np.ascontiguousarray(wattn.reshape(C, 128))
    cm["boffx"] = np.ascontiguousarray(boff[..., 0].reshape(1, 128)).astype(BF)
    cm["boffy"] = np.ascontiguousarray(boff[..., 1].reshape(1, 128)).astype(BF)
    cm["battn"] = np.ascontiguousarray(battn.reshape(1, 128)).astype(BF)
    cm["wval"] = np.asarray(W_val, np.float32).astype(BF)
    cm["bval"] = np.asarray(b_val, np.float32).reshape(1, C).astype(BF)
    wof = np.asarray(W_out, np.float32).reshape(8, 32, C).transpose(1, 0, 2)
    cm["wout"] = np.ascontiguousarray(
        np.broadcast_to(wof[None], (4, 32, 8, C)).reshape(128, 8, C)
    ).astype(BF)
    sel = np.zeros((2, 128), np.float32)
    sel[0] = 1.0
    cm["selx"] = sel
    cm["sely"] = sel[::-1].copy()
    cm["onesq"] = np.ones((1, QP), np.float32).astype(BF)
    cm["onesbf"] = np.ones((1, 128), np.float32).astype(BF)
    cmk = np.zeros((1, NVPAD), np.float32)
    cmk[0, :NV] = 1.0
    cm["cmask"] = cmk.astype(BF)
    r = np.zeros((128, 32), np.float32)
    for p in range(128):
        r[p, p % 32] = 1.0
    cm["r128"] = r.astype(BF)
    s16 = np.zeros((128, 8), np.float32)
    b8 = np.zeros((8, 128), np.float32)
    for p in range(128):
        h = (p % 32) // 4
        s16[p, h] = 1.0
        b8[h, p] = 1.0
    cm["s16"] = s16.astype(BF)
    cm["b8"] = b8.astype(BF)
    lvl_of_p = np.arange(128) % 4                       # level = a = c%4
    Wl = np.array([LEVELS[l][1] for l in lvl_of_p], np.float32)
    Hl = np.array([LEVELS[l][0] for l in lvl_of_p], np.float32)
    cb = np.array([CELLSTART[l] - LEVELS[l][1] - 1 for l in lvl_of_p],
                  np.float32)
    cm["iden"] = np.eye(128, dtype=np.float32).astype(BF)
    cm["sclw"] = Wl.reshape(128, 1)
    cm["sclh"] = Hl.reshape(128, 1)
    cm["nsclw"] = (-Wl).reshape(128, 1)
    cm["nsclh"] = (-Hl).reshape(128, 1)
    cm["wlm1"] = (Wl - 1).reshape(128, 1)
    cm["hlm1"] = (Hl - 1).reshape(128, 1)
    cm["cbase"] = cb.reshape(128, 1)
    return cm


def kernel(**inputs):
    if "nc" not in _CACHE:
        _CACHE["nc"] = build_nc()
    nc = _CACHE["nc"]
    cm = _consts(inputs["W_off"], inputs["b_off"], inputs["W_attn"],
                 inputs["b_attn"], inputs["W_val"], inputs["b_val"],
                 inputs["W_out"], inputs["b_out"])
    query = np.asarray(inputs["query"], np.float32)
    refp = np.asarray(inputs["reference_points"], np.float32)
    value = np.asarray(inputs["value"], np.float32)
    vpad = np.zeros((BS, NVPAD, C), np.float32)
    vpad[:, :NV] = value
    qpad = np.zeros((BS, 4 * QP, C), np.float32)
    qpad[:, :NQ] = query
    rpad = np.zeros((BS, 4 * QP, 2), np.float32)
    rpad[:, :NQ] = refp
    vT_b = []
    for b in range(BS):
        vT = vpad[b].T.reshape(2, 128, NVPAD).transpose(1, 0, 2)
        vT_b.append(np.ascontiguousarray(vT.astype(BF)))
    in_maps = []
    for core in range(NCORES):
        b, qc = core // 4, core % 4
        nvalid = min(QP, max(0, NQ - qc * QP))
        qm = np.zeros((128, QP), np.float32)
        qm[:, :nvalid] = 1.0
        qm = qm.astype(BF)
        qs = qpad[b, qc * QP:(qc + 1) * QP]
        rs = rpad[b, qc * QP:(qc + 1) * QP]
        qT = qs.T.reshape(2, 128, QP).transpose(1, 0, 2)
        m = {"qT": np.ascontiguousarray(qT),
             "refT": np.ascontiguousarray(rs.T),
             "vT": vT_b[b],
             "qmask": qm}
        m.update({k: np.ascontiguousarray(v) for k, v in cm.items()})
        in_maps.append(m)
    res = run_bass_kernel_spmd(nc, in_maps, list(range(NCORES)),
                               **_CACHE.get("run_kw", {}))
    _CACHE["last_res"] = res
    out = np.zeros((BS, NQ + 512, C), np.float32)
    for core in range(NCORES):
        b, qc = core // 4, core % 4
        slab = res.results[core]["out"]        # [8, 3, 128, 256]
        for m in range(NUM_HEADS):
            dlt = 64 * (m % 2)
            tb = 960 * m + 256 * qc - dlt      # absolute tile base
            for j in range(3):
                if m % 2 == 0:
                    row_lo, row_hi = 0, (128, 128, 32)[j]
                else:
                    row_lo, row_hi = ((64, 0, 0)[j], (128, 128, 96)[j])
                r0 = tb + 128 * j + row_lo
                r1 = tb + 128 * j + row_hi
                r1c = min(r1, NQ + 512)
                if r0 < 0 or r1c <= r0:
                    continue
                out[b, r0:r1c] += slab[m, j, row_lo:row_lo + (r1c - r0)]
    out = out[:, :NQ] + np.asarray(inputs["b_out"], np.float32)[None, None]
    return out


# revision 24
# speedup vs baseline: 1.2428x; 1.0009x over previous
"""Multi-scale deformable attention on 8 Trainium2 NeuronCores.

Sharding: (batch x query-quarter) -> 8 cores; each core does all 8 heads for
2048 queries of one batch (value projection recomputed per core).

v3 design (cost-model-aware):
  - v-proj -> vsb_full (cell-major bf16) -> vproj_d DRAM -> 16 cheap
    DRAM->DRAM DMAs build the per-head quad maps (maps_all[8, ES, 128]).
  - phase 3 computes positions/weights per query-half; floors use trunc
    (xpp >= 0.5 always); clamp+scale on Act (Relu chain); idx transposed to
    the 16-wrap via i16 PE transposes; idxw partitions 16..127 are zeroed
    once (executor only reads [:16]).
  - weights repacked to w4all_d DRAM; per half-chunk a 0-stride-src DMA
    replicates rows to wc[128, 4096] bf16 in SBUF (sync+scalar queues).
  - 64 half-chunks (head, point, q-half): Pool gather -> DVE single
    multiply -> PE reduce into psO4 -> Act copy to osb -> per-head out-proj.
"""
import sys

sys.path.insert(0, '/opt/trn_rl_repo')

import numpy as np
import ml_dtypes

import concourse.bass as bass
import concourse.bacc as bacc
import concourse.mybir as mybir
import concourse.tile as tile
from concourse.bass_utils import run_bass_kernel_spmd

dt = mybir.dt
F32, BF16, I16, I32 = dt.float32, dt.bfloat16, dt.int16, dt.int32
F32R = dt.float32r
ALU = mybir.AluOpType
ACTF = mybir.ActivationFunctionType
BF = ml_dtypes.bfloat16

# ---------------------------------------------------------------- geometry
LEVELS = ((76, 76), (38, 38), (19, 19), (10, 10))
NUM_HEADS, NUM_LEVELS, NUM_POINTS = 8, 4, 4
C, D = 256, 32
BS, NQ = 2, 7681
QP = 2048                      # queries per core (padded)
HQ = QP // 2
NCORES = 8
SAMP = NUM_HEADS * NUM_LEVELS * NUM_POINTS * QP       # 262144
IW = SAMP // 16                                       # idxw cols = 16384
STARTS = [0]
for (_h, _w) in LEVELS:
    STARTS.append(STARTS[-1] + _h * _w)
NV = STARTS[-1]                # 7681
NVPAD = 7808                   # 61 * 128
GAP = 256
CELLSTART = []
_pos = GAP
for (_h, _w) in LEVELS:
    CELLSTART.append(_pos)
    _pos += _h * _w + GAP
ES = ((_pos - GAP) // 128 + 1) * 128      # cell-space rows (8832)
TS2 = ES // 128                           # vsb_full blocks (69)
HCH = 4096                     # half-chunk samples
NHCH = SAMP // HCH             # 64


def align_down(x, a=128):
    return (x // a) * a


def mkap(base_ap, ap_list, offset=None):
    ap = base_ap.copy()
    ap.ap = mybir.VecI64Pair([list(x) for x in ap_list])
    if offset is not None:
        ap.offset = offset
    return ap


# ---------------------------------------------------------------- program
def build_nc():
    nc = bacc.Bacc("TRN2", target_bir_lowering=False)
    qT_d = nc.dram_tensor("qT", [128, 2, QP], F32, kind="ExternalInput")
    refT_d = nc.dram_tensor("refT", [2, QP], F32, kind="ExternalInput")
    vT_d = nc.dram_tensor("vT", [128, 2, NVPAD], BF16, kind="ExternalInput")
    woffx_d = nc.dram_tensor("woffx", [C, 128], F32, kind="ExternalInput")
    woffy_d = nc.dram_tensor("woffy", [C, 128], F32, kind="ExternalInput")
    wattn_d = nc.dram_tensor("wattn", [C, 128], F32, kind="ExternalInput")
    boffx_d = nc.dram_tensor("boffx", [1, 128], BF16, kind="ExternalInput")
    boffy_d = nc.dram_tensor("boffy", [1, 128], BF16, kind="ExternalInput")
    battn_d = nc.dram_tensor("battn", [1, 128], BF16, kind="ExternalInput")
    wval_d = nc.dram_tensor("wval", [C, C], BF16, kind="ExternalInput")
    bval_d = nc.dram_tensor("bval", [1, C], BF16, kind="ExternalInput")
    cmask_d = nc.dram_tensor("cmask", [1, NVPAD], BF16, kind="ExternalInput")
    wout_d = nc.dram_tensor("wout", [128, 8, C], BF16, kind="ExternalInput")
    selx_d = nc.dram_tensor("selx", [2, 128], F32, kind="ExternalInput")
    sely_d = nc.dram_tensor("sely", [2, 128], F32, kind="ExternalInput")
    onesq_d = nc.dram_tensor("onesq", [1, QP], BF16, kind="ExternalInput")
    onesbf_d = nc.dram_tensor("onesbf", [1, 128], BF16, kind="ExternalInput")
    r128_d = nc.dram_tensor("r128", [128, 32], BF16, kind="ExternalInput")
    s16_d = nc.dram_tensor("s16", [128, 8], BF16, kind="ExternalInput")
    b8_d = nc.dram_tensor("b8", [8, 128], BF16, kind="ExternalInput")
    sclw_d = nc.dram_tensor("sclw", [128, 1], F32, kind="ExternalInput")
    sclh_d = nc.dram_tensor("sclh", [128, 1], F32, kind="ExternalInput")
    nsclw_d = nc.dram_tensor("nsclw", [128, 1], F32, kind="ExternalInput")
    nsclh_d = nc.dram_tensor("nsclh", [128, 1], F32, kind="ExternalInput")
    wlm1_d = nc.dram_tensor("wlm1", [128, 1], F32, kind="ExternalInput")
    hlm1_d = nc.dram_tensor("hlm1", [128, 1], F32, kind="ExternalInput")
    cbase_d = nc.dram_tensor("cbase", [128, 1], F32, kind="ExternalInput")
    qmask_d = nc.dram_tensor("qmask", [128, QP], BF16, kind="ExternalInput")
    iden_d = nc.dram_tensor("iden", [128, 128], BF16, kind="ExternalInput")
    out_d = nc.dram_tensor("out", [8, 3, 128, C], F32,
                           kind="ExternalOutput")
    vproj_d = nc.dram_tensor("vproj", [ES + 128, C], BF16)
    maps_d = nc.dram_tensor("mapsall", [NUM_HEADS, ES, 128], BF16)
    w4a_d = nc.dram_tensor("w4ad", [128, 2 * HCH], BF16)

    with tile.TileContext(nc) as tc:
        with tc.tile_pool(name="const", bufs=1) as cpool:
            selx = cpool.tile([2, 128], F32)
            sely = cpool.tile([2, 128], F32)
            onesq = cpool.tile([1, QP], BF16)
            onesbf = cpool.tile([1, 128], BF16)
            r128 = cpool.tile([128, 32], BF16)
            s16 = cpool.tile([128, 8], BF16)
            b8c = cpool.tile([8, 128], BF16)
            sclw = cpool.tile([128, 1], F32)
            sclh = cpool.tile([128, 1], F32)
            nsclw = cpool.tile([128, 1], F32)
            nsclh = cpool.tile([128, 1], F32)
            wlm1 = cpool.tile([128, 1], F32)
            hlm1 = cpool.tile([128, 1], F32)
            cbase = cpool.tile([128, 1], F32)
            woffx = cpool.tile([128, 2, 128], F32)
            woffy = cpool.tile([128, 2, 128], F32)
            wattn = cpool.tile([128, 2, 128], F32)
            boffx = cpool.tile([1, 128], BF16)
            boffy = cpool.tile([1, 128], BF16)
            battn = cpool.tile([1, 128], BF16)
            wvalb = cpool.tile([128, 2, C], BF16)
            bvalb = cpool.tile([1, C], BF16)
            cmask = cpool.tile([1, NVPAD], BF16)
            woutc = cpool.tile([128, 8, C], BF16)
            qmask = cpool.tile([128, QP], BF16)
            iden = cpool.tile([128, 128], BF16)
            qT = cpool.tile([128, 2, QP], F32)
            refT = cpool.tile([2, QP], F32)
            # big loads split across sync/scalar
            nc.sync.dma_start(qT[:, 0], qT_d[:, 0])
            nc.scalar.dma_start(qT[:, 1], qT_d[:, 1])
            nc.sync.dma_start(refT[:], refT_d[:])
            for t, s in [(woffx, woffx_d), (woffy, woffy_d),
                         (wattn, wattn_d), (wvalb, wval_d)]:
                nc.scalar.dma_start(
                    t[:], s[:].rearrange("(h p) x -> p h x", p=128))
            for t, s in [(onesq, onesq_d), (boffx, boffx_d),
                         (boffy, boffy_d), (selx, selx_d),
                         (sely, sely_d), (sclw, sclw_d), (sclh, sclh_d),
                         (nsclw, nsclw_d), (nsclh, nsclh_d),
                         (wlm1, wlm1_d), (hlm1, hlm1_d),
                         (cbase, cbase_d), (iden, iden_d),
                         (qmask, qmask_d)]:
                nc.sync.dma_start(t[:], s[:])
            for t, s in [(battn, battn_d), (bvalb, bval_d),
                         (cmask, cmask_d), (onesbf, onesbf_d),
                         (s16, s16_d), (b8c, b8_d),
                         (r128, r128_d), (woutc, wout_d)]:
                nc.scalar.dma_start(t[:], s[:])

            ps3 = tc.alloc_tile_pool(name="ps3", bufs=2, space="PSUM")

            def projf(hq, wof, bof, sel):
                ps = ps3.tile([128, HQ], F32, tag="pp")
                for c0 in range(0, HQ, 512):
                    cs = slice(hq * HQ + c0, hq * HQ + c0 + 512)
                    po = ps[:, c0:c0 + 512]
                    for half in range(2):
                        nc.tensor.matmul(
                            po, wof[:, half, :], qT[:, half, cs],
                            start=(half == 0), stop=False)
                    nc.tensor.matmul(po, bof[:], onesq[:, cs],
                                     start=False, stop=(sel is None))
                    if sel is not None:
                        nc.tensor.matmul(po, sel[:], refT[:, cs],
                                         start=False, stop=True)
                return ps

            # hq0 x/y projections run on PE before anything else
            psX0 = projf(0, woffx, boffx, selx)
            psY0 = projf(0, woffy, boffy, sely)

            # ======== phase 2 prologue: vT load + staging alloc ==========
            pVS = tc.alloc_tile_pool(name="pVS", bufs=1, side="right")
            zt = pVS.tile([128, C], BF16)
            nc.scalar.memzero(zt[:])
            # zero-fill vproj_d gap/pad rows straight from zt
            gaps = [(0, GAP)]
            for lvl in range(NUM_LEVELS):
                ge = CELLSTART[lvl] + LEVELS[lvl][0] * LEVELS[lvl][1]
                gn = CELLSTART[lvl + 1] if lvl < 3 else ES + 128
                gaps.append((ge, gn - ge))
            gz = 0
            for (g0, n) in gaps:
                while n > 0:
                    cnt = min(n, 128)
                    eng = nc.sync if gz % 2 == 0 else nc.scalar
                    gz += 1
                    eng.dma_start(
                        mkap(vproj_d[:], [[C, cnt], [1, C]],
                             offset=g0 * C),
                        mkap(zt[:], [[C, cnt], [1, C]]))
                    g0 += cnt
                    n -= cnt
            pVT = tc.alloc_tile_pool(name="pVT", bufs=1, side="right")
            vT = pVT.tile([128, 2, NVPAD], BF16)
            nc.sync.dma_start(vT[:, 0], vT_d[:, 0])
            nc.scalar.dma_start(vT[:, 1], vT_d[:, 1])
            ncp = [0]
            # ======== phase 3: positions / weights / indices ==========
            pw = tc.alloc_tile_pool(name="pw", bufs=1)
            idxw = pw.tile([128, IW], I16)
            # executor reads idx partitions [:16] but asserts all 128 are
            # in-range: zero the tile once (copies then fill rows 0..15)
            nc.scalar.memzero(idxw[:])
            w4p = [pw.tile([128, QP], BF16, name=f"w4p{s}")
                   for s in range(4)]
            p3 = tc.alloc_tile_pool(name="p3", bufs=1)
            ps3s = tc.alloc_tile_pool(name="ps3s", bufs=1, space="PSUM")
            psQp = tc.alloc_tile_pool(name="psQ", bufs=1, space="PSUM")
            p1t = tc.alloc_tile_pool(name="p1t", bufs=2)
            ps1 = tc.alloc_tile_pool(name="ps1", bufs=2, space="PSUM")

            def ph3(hq):
                    qs = slice(hq * HQ, (hq + 1) * HQ)

                    def corner_weights(psP, sclp, nsclp, wm1, pref):
                        rr = p3.tile([128, HQ], F32, tag="cwr")
                        xq = p3.tile([128, HQ], F32, tag=f"{pref}xq")
                        xi = p3.tile([128, HQ], I32, tag="cwxi")
                        x0p = p3.tile([128, HQ], F32, tag=f"{pref}x0p")
                        wx = p3.tile([128, HQ], F32, tag=f"{pref}wx")
                        v0 = p3.tile([128, HQ], BF16, tag="cwv0")
                        v1 = p3.tile([128, HQ], BF16, tag="cwv1")
                        wxf = p3.tile([128, HQ], BF16, tag="cwwxf")
                        a0 = p3.tile([128, HQ], BF16, tag=f"{pref}a0")
                        a1 = p3.tile([128, HQ], BF16, tag=f"{pref}a1")
                        # rr = max(1-p, 0); xq = max(scl - scl*rr, 0)
                        #   = scl*clamp01(p)
                        nc.scalar.activation(rr[:], psP[:], ACTF.Relu,
                                             bias=1.0, scale=-1.0)
                        nc.scalar.activation(xq[:], rr[:], ACTF.Relu,
                                             bias=sclp[:], scale=nsclp[:])
                        # x0p = floor(xq + 0.5) (trunc ok: arg >= 0.5)
                        nc.vector.tensor_scalar(xi[:], xq[:], 0.5, 0.0,
                                                op0=ALU.add,
                                                op1=ALU.bypass)
                        nc.vector.tensor_copy(x0p[:], xi[:])
                        # rounding-mode-agnostic floor: subtract 1 where
                        # the int conversion rounded up
                        t1 = p3.tile([128, HQ], F32, tag="cwt1")
                        nc.vector.scalar_tensor_tensor(
                            t1[:], xq[:], 0.5, x0p[:],
                            op0=ALU.add, op1=ALU.is_lt)
                        nc.vector.tensor_tensor(x0p[:], x0p[:], t1[:],
                                                op=ALU.subtract)
                        nc.vector.scalar_tensor_tensor(
                            wx[:], xq[:], 0.5, x0p[:],
                            op0=ALU.add, op1=ALU.subtract)
                        nc.vector.tensor_scalar(
                            v0[:], x0p[:], 1.0, 0.0,
                            op0=ALU.is_ge, op1=ALU.bypass)
                        nc.vector.tensor_scalar(
                            v1[:], x0p[:], wm1[:], 0.0,
                            op0=ALU.is_le, op1=ALU.bypass)
                        nc.vector.tensor_tensor(a1[:], wx[:], v1[:],
                                                op=ALU.mult)
                        nc.vector.tensor_scalar(
                            wxf[:], wx[:], -1.0, 1.0,
                            op0=ALU.mult, op1=ALU.add)
                        nc.vector.tensor_tensor(a0[:], wxf[:], v0[:],
                                                op=ALU.mult)
                        return x0p, a0, a1

                    if hq == 0:
                        psX, psY = psX0, psY0
                    else:
                        psX = projf(hq, woffx, boffx, selx)
                        psY = projf(hq, woffy, boffy, sely)
                    x0p, ax0, ax1 = corner_weights(psX, sclw, nsclw,
                                                   wlm1, "x")
                    y0p, ay0, ay1 = corner_weights(psY, sclh, nsclh,
                                                   hlm1, "y")
                    idxf = p3.tile([128, HQ], F32, tag="cwxq2")
                    idxf16 = p3.tile([128, HQ], I16, tag=f"i16{hq}")
                    nc.vector.scalar_tensor_tensor(
                        idxf[:], y0p[:], sclw[:], x0p[:],
                        op0=ALU.mult, op1=ALU.add)
                    nc.vector.tensor_scalar(
                        idxf16[:], idxf[:], cbase[:], 0.0,
                        op0=ALU.add, op1=ALU.bypass)
                    # i16 transposes -> idxw 16-wrap
                    # col = ci*512 + hq*256 + m*64 + jj  (ci=h*4+a, m=pt)
                    for jj in range(HQ // 16):
                        psQ = psQp.tile([16, 128], BF16, tag="psQ")
                        nc.tensor.transpose(
                            psQ[:],
                            idxf16[:, jj * 16:(jj + 1) * 16].bitcast(BF16),
                            iden[:])
                        dst = mkap(idxw[:], [[IW, 16], [64, 4], [256, 32]],
                                   offset=hq * 8192 + jj)
                        src = mkap(psQ[:].bitcast(I16),
                                   [[128, 16], [32, 4], [1, 32]])
                        if jj % 4 < 3:
                            nc.vector.tensor_copy(dst, src)
                        else:
                            nc.scalar.activation(dst, src, ACTF.Copy)
                    # ---- attention weights ----
                    psZ = projf(hq, wattn, battn, None)
                    esb = p3.tile([128, HQ], BF16, tag="cwesb")
                    nc.scalar.activation(esb[:], psZ[:], ACTF.Exp,
                                         bias=0.0, scale=1.0)
                    rsbq = p3.tile([8, HQ], BF16, tag="cwrsbq")
                    for c0 in range(0, HQ, 512):
                        pss = ps3s.tile([8, 512], F32, tag="pss")
                        nc.tensor.matmul(pss[:], s16[:],
                                         esb[:, c0:c0 + 512],
                                         start=True, stop=True)
                        rsb = p3.tile([8, 512], F32, tag="cwrsb")
                        nc.vector.reciprocal(rsb[:], pss[:])
                        nc.vector.tensor_tensor(
                            rsbq[:, c0:c0 + 512], rsb[:],
                            qmask[0:8, hq * HQ + c0:hq * HQ + c0 + 512],
                            op=ALU.mult)
                    psr2 = ps3.tile([128, HQ], F32, tag="pp")
                    for c0 in range(0, HQ, 512):
                        nc.tensor.matmul(psr2[:, c0:c0 + 512], b8c[:],
                                         rsbq[:, c0:c0 + 512],
                                         start=True, stop=True)
                    aw = p3.tile([128, HQ], BF16, tag="cwaw")
                    nc.vector.tensor_tensor(aw[:], esb[:], psr2[:],
                                            op=ALU.mult)
                    nc.vector.tensor_tensor(ay0[:], ay0[:], aw[:],
                                            op=ALU.mult)
                    nc.vector.tensor_tensor(ay1[:], ay1[:], aw[:],
                                            op=ALU.mult)
                    for s, (ax, ay) in enumerate(
                            [(ax0, ay0), (ax1, ay0),
                             (ax0, ay1), (ax1, ay1)]):
                        nc.vector.tensor_tensor(
                            w4p[s][:, qs], ax[:], ay[:], op=ALU.mult)
                    # repack w4p -> w4a_d DRAM: rows (s,ci), cols
                    # hq*4096 + m*1024 + q_l
                    for s in range(4):
                        for m in range(4):
                            eng = nc.sync if (s * 4 + m) % 2 == 0 \
                                else nc.scalar
                            eng.dma_start(
                                mkap(w4a_d[:],
                                     [[2 * HCH, 32], [1, HQ]],
                                     offset=s * 32 * 2 * HCH
                                     + hq * HCH + m * HQ),
                                mkap(w4p[s][:], [[QP, 32], [1, HQ]],
                                     offset=32 * m * QP + hq * HQ))
                    # replicate this hq's idx cols 16 -> 128 partitions
                    for (r0, nr) in [(16, 16), (32, 32), (64, 64)]:
                        for half in range(2):
                            eng = nc.sync if half == 0 else nc.scalar
                            cs = hq * 8192 + half * 4096
                            eng.dma_start(
                                mkap(idxw[:], [[IW, nr], [1, 4096]],
                                     offset=r0 * IW + cs),
                                mkap(idxw[:], [[IW, nr], [1, 4096]],
                                     offset=(r0 - nr) * IW + cs))

            ph3(0)
            for lvl in range(NUM_LEVELS):
                hw = LEVELS[lvl][0] * LEVELS[lvl][1]
                shift = CELLSTART[lvl] - STARTS[lvl]  # mult of 128
                c0 = STARTS[lvl]
                while c0 < STARTS[lvl] + hw:
                    tbeg = align_down(c0)
                    cend = min(tbeg + 128, STARTS[lvl] + hw)
                    lo, hi = c0 - tbeg, cend - tbeg
                    psv = ps1.tile([128, C], F32, tag="psv")
                    for half in range(2):
                        nc.tensor.matmul(
                            psv[:], vT[:, half, tbeg:tbeg + 128],
                            wvalb[:, half, :], start=(half == 0),
                            stop=False)
                    nc.tensor.matmul(psv[:], cmask[:, tbeg:tbeg + 128],
                                     bvalb[:], start=False, stop=True)
                    sp = c0 + shift
                    assert sp % 128 == lo
                    vstage = p1t.tile([128, C], BF16, tag="vstage")
                    if ncp[0] % 2 == 0:
                        nc.scalar.copy(vstage[:], psv[:])
                    else:
                        nc.vector.tensor_copy(vstage[:], psv[:])
                    eng = nc.sync if ncp[0] % 2 == 0 else nc.scalar
                    ncp[0] += 1
                    eng.dma_start(
                        mkap(vproj_d[:], [[C, hi - lo], [1, C]],
                             offset=sp * C),
                        mkap(vstage[:], [[C, hi - lo], [1, C]],
                             offset=lo * C))
                    c0 = cend
            # maps: one DRAM->DRAM DMA per (lvl, corner); ranges cover every
            # row of maps_d so the finite-checker never sees uninit DRAM
            MB = [0] + [CELLSTART[l] - LEVELS[l][1] - 1 for l in (1, 2, 3)] \
                + [ES]
            for lvl, (H, W) in enumerate(LEVELS):
                g0 = MB[lvl]
                n = MB[lvl + 1] - MB[lvl]
                for s, dl in enumerate([0, 1, W, W + 1]):
                    eng = nc.sync if (lvl * 4 + s) % 2 == 0 else nc.scalar
                    eng.dma_start(
                        mkap(maps_d[:], [[128, n], [ES * 128, 8], [1, 32]],
                             offset=g0 * 128 + s * 32),
                        mkap(vproj_d[:], [[C, n], [32, 8], [1, 32]],
                             offset=(g0 + dl) * C))
            pVT.release()
            pVS.release()

            ph3(1)
            ps1.release()
            p1t.release()
            psQp.release()
            ps3s.release()
            p3.release()
            ps3.release()
            # ======== phase 4: gather / multiply / reduce =================
            with tc.tile_pool(name="p4t", bufs=2) as p4t, \
                 tc.tile_pool(name="pwc", bufs=3) as pwc, \
                 tc.tile_pool(name="psW", bufs=2, space="PSUM") as psWp, \
                 tc.tile_pool(name="psO", bufs=1, space="PSUM") as psO, \
                 tc.tile_pool(name="pOs", bufs=1) as pOs, \
                 tc.tile_pool(name="pg", bufs=3) as pg, \
                 tc.tile_pool(name="pfo", bufs=3) as pfo:
                osb = [pOs.tile([128, QP + 2048], BF16, name=f"osb{g}",
                                tag=f"osb{g}")
                       for g in range(2)]
                for g in range(2):
                    nc.scalar.memzero(osb[g][:])

                def phase5_head(m):
                    # reference reshape quirk: out row q column c takes
                    # O[m, qq, d] with u = m*7681 + qq = 8q + (c//32 slot),
                    # W_out row 32*((m+qq)%8) + d.  7681 % 8 == 1 makes the
                    # structure identical on every core (host assembles).
                    gsb = osb[m // 4]
                    grp = m % 4
                    dlt = 64 * (m % 2)
                    for j in range(3):
                        psF4 = psWp.tile([128, 1024], F32, tag="psW")
                        psF = psF4[:, 0:C]
                        for s in range(8):
                            q0 = -m - 8 * dlt + 1024 * j + s
                            col0 = 1024 + q0
                            assert 0 <= col0 and col0 + 8 * 127 < QP + 2048
                            lhsT = mkap(
                                gsb[:], [[QP + 2048, 32], [8, 128]],
                                offset=(grp * 32) * (QP + 2048) + col0)
                            kw = {}
                            if grp == 3:
                                kw["tile_position"] = (96, 0)
                            nc.tensor.matmul(
                                psF, lhsT,
                                woutc[grp * 32:grp * 32 + 32, s, :],
                                start=(s == 0), stop=(s == 7), **kw)
                        fo = pfo.tile([128, C], F32, tag="fo")
                        nc.scalar.copy(fo[:], psF)
                        eng = nc.sync if j % 2 == 0 else nc.scalar
                        eng.dma_start(out_d[m, j], fo[:])

                psT = {}
                for ck in range(NHCH):
                    hq, hh, a = ck // 32, (ck % 32) // 4, ck % 4
                    ci = hh * 4 + a
                    grp = hh % 4
                    if grp == 0 and a == 0:
                        psT[hh // 4] = psO.tile(
                            [128, HQ], F32, name=f"psO4{hh // 4}",
                            tag=f"psO4{hh // 4}")
                    psO4 = psT[hh // 4]
                    i0 = hq * 8192 + ci * 256
                    g = pg.tile([128, 1, HCH], BF16, tag="g")
                    nc.gpsimd.dma_gather(
                        g[:], maps_d[hh],
                        idxw[:, i0:i0 + 256],
                        HCH, HCH, 128,
                        transpose=True, single_packet=False)
                    # replicate weights: wc[s*32+ch, :] = w4a_d[s*32+ci,
                    # hq*4096 + :]
                    wc = pwc.tile([128, HCH], BF16, tag="wc")
                    for half in range(2):
                        eng = nc.sync if half == 0 else nc.scalar
                        eng.dma_start(
                            mkap(wc[:], [[HCH, 128], [1, HCH // 2]],
                                 offset=half * (HCH // 2)),
                            mkap(w4a_d[:],
                                 [[32 * 2 * HCH, 4], [0, 32],
                                  [1, HCH // 2]],
                                 offset=ci * 2 * HCH + hq * HCH
                                 + half * (HCH // 2)))
                    tt = p4t.tile([128, HCH], BF16, tag="tt")
                    nc.vector.tensor_tensor(tt[:], g[:, 0, :], wc[:],
                                            op=ALU.mult)
                    for m4 in range(4):
                        for j2 in range(2):
                            kw = {}
                            if grp == 3:
                                kw["tile_position"] = (0, 96)
                            cs = slice(j2 * 512, (j2 + 1) * 512)
                            nc.tensor.matmul(
                                psO4[grp * 32:(grp + 1) * 32, cs],
                                r128[:],
                                tt[:, m4 * 1024 + j2 * 512:
                                   m4 * 1024 + (j2 + 1) * 512],
                                start=(a == 0 and m4 == 0),
                                stop=(a == 3 and m4 == 3), **kw)
                    if a == 3:
                        nc.scalar.activation(
                            osb[hh // 4][grp * 32:(grp + 1) * 32,
                                         1024 + hq * HQ:
                                         1024 + (hq + 1) * HQ],
                            psO4[grp * 32:(grp + 1) * 32, :], ACTF.Copy)
                    if ck >= 37 and (ck - 37) % 4 == 0 and ck <= 61:
                        phase5_head((ck - 37) // 4)

                # ======== phase 5 tail ====================================
                phase5_head(7)
            pw.release()
    nc.compile()
    return nc


# ---------------------------------------------------------------- host side
_CACHE = {}


def _consts(W_off, b_off, W_attn, b_attn, W_val, b_val, W_out, b_out):
    M = NUM_HEADS
    # partition layout c = b*32 + h*4 + a  (old: h*16 + a*4 + b)
    woff = np.asarray(W_off, np.float32).reshape(C, M, 4, 4, 2)
    woff = np.transpose(woff, (0, 3, 1, 2, 4))          # (C, b, h, a, 2)
    wattn = np.asarray(W_attn, np.float32).reshape(C, M, 4, 4)
    # partition (b, h, a) holds attention logit (level=b, point=a) so that
    # sample (h, a, b) pairs with aw(level=b, point=a)  (reference quirk)
    wattn = np.transpose(wattn, (0, 2, 1, 3))           # (C, l, h, k)
    boff = np.asarray(b_off, np.float32).reshape(M, 4, 4, 2)
    boff = np.transpose(boff, (2, 0, 1, 3))             # (b, h, a, 2)
    battn = np.asarray(b_attn, np.float32).reshape(M, 4, 4)
    battn = np.transpose(battn, (1, 0, 2))              # (l, h, k)
    cm = {}
    cm["woffx"] = np.ascontiguousarray(woff[..., 0].reshape(C, 128))
    cm["woffy"] = np.ascontiguousarray(woff[..., 1].reshape(C, 128))
    cm["wattn"] = np.ascontiguousarray(wattn.reshape(C, 128))
    cm["boffx"] = np.ascontiguousarray(boff[..., 0].reshape(1, 128)).astype(BF)
    cm["boffy"] = np.ascontiguousarray(boff[..., 1].reshape(1, 128)).astype(BF)
    cm["battn"] = np.ascontiguousarray(battn.reshape(1, 128)).astype(BF)
    cm["wval"] = np.asarray(W_val, np.float32).astype(BF)
    cm["bval"] = np.asarray(b_val, np.float32).reshape(1, C).astype(BF)
    wof = np.asarray(W_out, np.float32).reshape(8, 32, C).transpose(1, 0, 2)
    cm["wout"] = np.ascontiguousarray(
        np.broadcast_to(wof[None], (4, 32, 8, C)).reshape(128, 8, C)
    ).astype(BF)
    sel = np.zeros((2, 128), np.float32)
    sel[0] = 1.0
    cm["selx"] = sel
    cm["sely"] = sel[::-1].copy()
    cm["onesq"] = np.ones((1, QP), np.float32).astype(BF)
    cm["onesbf"] = np.ones((1, 128), np.float32).astype(BF)
    cmk = np.zeros((1, NVPAD), np.float32)
    cmk[0, :NV] = 1.0
    cm["cmask"] = cmk.astype(BF)
    r = np.zeros((128, 32), np.float32)
    for p in range(128):
        r[p, p % 32] = 1.0
    cm["r128"] = r.astype(BF)
    s16 = np.zeros((128, 8), np.float32)
    b8 = np.zeros((8, 128), np.float32)
    for p in range(128):
        h = (p % 32) // 4
        s16[p, h] = 1.0
        b8[h, p] = 1.0
    cm["s16"] = s16.astype(BF)
    cm["b8"] = b8.astype(BF)
    lvl_of_p = np.arange(128) % 4                       # level = a = c%4
    Wl = np.array([LEVELS[l][1] for l in lvl_of_p], np.float32)
    Hl = np.array([LEVELS[l][0] for l in lvl_of_p], np.float32)
    cb = np.array([CELLSTART[l] - LEVELS[l][1] - 1 for l in lvl_of_p],
                  np.float32)
    cm["iden"] = np.eye(128, dtype=np.float32).astype(BF)
    cm["sclw"] = Wl.reshape(128, 1)
    cm["sclh"] = Hl.reshape(128, 1)
    cm["nsclw"] = (-Wl).reshape(128, 1)
    cm["nsclh"] = (-Hl).reshape(128, 1)
    cm["wlm1"] = (Wl - 1).reshape(128, 1)
    cm["hlm1"] = (Hl - 1).reshape(128, 1)
    cm["cbase"] = cb.reshape(128, 1)
    return cm


def kernel(**inputs):
    if "nc" not in _CACHE:
        _CACHE["nc"] = build_nc()
    nc = _CACHE["nc"]
    cm = _consts(inputs["W_off"], inputs["b_off"], inputs["W_attn"],
                 inputs["b_attn"], inputs["W_val"], inputs["b_val"],
                 inputs["W_out"], inputs["b_out"])
    query = np.asarray(inputs["query"], np.float32)
    refp = np.asarray(inputs["reference_points"], np.float32)
    value = np.asarray(inputs["value"], np.float32)
    vpad = np.zeros((BS, NVPAD, C), np.float32)
    vpad[:, :NV] = value
    qpad = np.zeros((BS, 4 * QP, C), np.float32)
    qpad[:, :NQ] = query
    rpad = np.zeros((BS, 4 * QP, 2), np.float32)
    rpad[:, :NQ] = refp
    vT_b = []
    for b in range(BS):
        vT = vpad[b].T.reshape(2, 128, NVPAD).transpose(1, 0, 2)
        vT_b.append(np.ascontiguousarray(vT.astype(BF)))
    in_maps = []
    for core in range(NCORES):
        b, qc = core // 4, core % 4
        nvalid = min(QP, max(0, NQ - qc * QP))
        qm = np.zeros((128, QP), np.float32)
        qm[:, :nvalid] = 1.0
        qm = qm.astype(BF)
        qs = qpad[b, qc * QP:(qc + 1) * QP]
        rs = rpad[b, qc * QP:(qc + 1) * QP]
        qT = qs.T.reshape(2, 128, QP).transpose(1, 0, 2)
        m = {"qT": np.ascontiguousarray(qT),
             "refT": np.ascontiguousarray(rs.T),
             "vT": vT_b[b],
             "qmask": qm}
        m.update({k: np.ascontiguousarray(v) for k, v in cm.items()})
        in_maps.append(m)
    res = run_bass_kernel_spmd(nc, in_maps, list(range(NCORES)),
                               **_CACHE.get("run_kw", {}))
    _CACHE["last_res"] = res
    out = np.zeros((BS, NQ + 512, C), np.float32)
    for core in range(NCORES):
        b, qc = core // 4, core % 4
        slab = res.results[core]["out"]        # [8, 3, 128, 256]
        for m in range(NUM_HEADS):
            dlt = 64 * (m % 2)
            tb = 960 * m + 256 * qc - dlt      # absolute tile base
            for j in range(3):
                if m % 2 == 0:
                    row_lo, row_hi = 0, (128, 128, 32)[j]
                else:
                    row_lo, row_hi = ((64, 0, 0)[j], (128, 128, 96)[j])
                r0 = tb + 128 * j + row_lo
                r1 = tb + 128 * j + row_hi
                r1c = min(r1, NQ + 512)
                if r0 < 0 or r1c <= r0:
                    continue
                out[b, r0:r1c] += slab[m, j, row_lo:row_lo + (r1c - r0)]
    out = out[:, :NQ] + np.asarray(inputs["b_out"], np.float32)[None, None]
    return out


# revision 26
# speedup vs baseline: 1.2547x; 1.0096x over previous
"""Multi-scale deformable attention on 8 Trainium2 NeuronCores.

Sharding: (batch x query-quarter) -> 8 cores; each core does all 8 heads for
2048 queries of one batch (value projection recomputed per core).

v3 design (cost-model-aware):
  - v-proj -> vsb_full (cell-major bf16) -> vproj_d DRAM -> 16 cheap
    DRAM->DRAM DMAs build the per-head quad maps (maps_all[8, ES, 128]).
  - phase 3 computes positions/weights per query-half; floors use trunc
    (xpp >= 0.5 always); clamp+scale on Act (Relu chain); idx transposed to
    the 16-wrap via i16 PE transposes; idxw partitions 16..127 are zeroed
    once (executor only reads [:16]).
  - weights repacked to w4all_d DRAM; per half-chunk a 0-stride-src DMA
    replicates rows to wc[128, 4096] bf16 in SBUF (sync+scalar queues).
  - 64 half-chunks (head, point, q-half): Pool gather -> DVE single
    multiply -> PE reduce into psO4 -> Act copy to osb -> per-head out-proj.
"""
import sys

sys.path.insert(0, '/opt/trn_rl_repo')

import numpy as np
import ml_dtypes

import concourse.bass as bass
import concourse.bacc as bacc
import concourse.mybir as mybir
import concourse.tile as tile
from concourse.bass_utils import run_bass_kernel_spmd

dt = mybir.dt
F32, BF16, I16, I32 = dt.float32, dt.bfloat16, dt.int16, dt.int32
F32R = dt.float32r
ALU = mybir.AluOpType
ACTF = mybir.ActivationFunctionType
BF = ml_dtypes.bfloat16

# ---------------------------------------------------------------- geometry
LEVELS = ((76, 76), (38, 38), (19, 19), (10, 10))
NUM_HEADS, NUM_LEVELS, NUM_POINTS = 8, 4, 4
C, D = 256, 32
BS, NQ = 2, 7681
QP = 2048                      # queries per core (padded)
HQ = QP // 2
NCORES = 8
SAMP = NUM_HEADS * NUM_LEVELS * NUM_POINTS * QP       # 262144
IW = SAMP // 16                                       # idxw cols = 16384
STARTS = [0]
for (_h, _w) in LEVELS:
    STARTS.append(STARTS[-1] + _h * _w)
NV = STARTS[-1]                # 7681
NVPAD = 7808                   # 61 * 128
GAP = 256
CELLSTART = []
_pos = GAP
for (_h, _w) in LEVELS:
    CELLSTART.append(_pos)
    _pos += _h * _w + GAP
ES = ((_pos - GAP) // 128 + 1) * 128      # cell-space rows (8832)
TS2 = ES // 128                           # vsb_full blocks (69)
HCH = 4096                     # half-chunk samples
NHCH = SAMP // HCH             # 64


def align_down(x, a=128):
    return (x // a) * a


def mkap(base_ap, ap_list, offset=None):
    ap = base_ap.copy()
    ap.ap = mybir.VecI64Pair([list(x) for x in ap_list])
    if offset is not None:
        ap.offset = offset
    return ap


# ---------------------------------------------------------------- program
def build_nc():
    nc = bacc.Bacc("TRN2", target_bir_lowering=False)
    qT_d = nc.dram_tensor("qT", [128, 2, QP], F32, kind="ExternalInput")
    refT_d = nc.dram_tensor("refT", [2, QP], F32, kind="ExternalInput")
    vT_d = nc.dram_tensor("vT", [128, 2, NVPAD], BF16, kind="ExternalInput")
    woffx_d = nc.dram_tensor("woffx", [C, 128], F32, kind="ExternalInput")
    woffy_d = nc.dram_tensor("woffy", [C, 128], F32, kind="ExternalInput")
    wattn_d = nc.dram_tensor("wattn", [C, 128], F32, kind="ExternalInput")
    boffx_d = nc.dram_tensor("boffx", [1, 128], BF16, kind="ExternalInput")
    boffy_d = nc.dram_tensor("boffy", [1, 128], BF16, kind="ExternalInput")
    battn_d = nc.dram_tensor("battn", [1, 128], BF16, kind="ExternalInput")
    wval_d = nc.dram_tensor("wval", [C, C], BF16, kind="ExternalInput")
    bval_d = nc.dram_tensor("bval", [1, C], BF16, kind="ExternalInput")
    cmask_d = nc.dram_tensor("cmask", [1, NVPAD], BF16, kind="ExternalInput")
    wout_d = nc.dram_tensor("wout", [128, 8, C], BF16, kind="ExternalInput")
    selx_d = nc.dram_tensor("selx", [2, 128], F32, kind="ExternalInput")
    sely_d = nc.dram_tensor("sely", [2, 128], F32, kind="ExternalInput")
    onesq_d = nc.dram_tensor("onesq", [1, QP], BF16, kind="ExternalInput")
    onesbf_d = nc.dram_tensor("onesbf", [1, 128], BF16, kind="ExternalInput")
    r128_d = nc.dram_tensor("r128", [128, 32], BF16, kind="ExternalInput")
    s16_d = nc.dram_tensor("s16", [128, 8], BF16, kind="ExternalInput")
    b8_d = nc.dram_tensor("b8", [8, 128], BF16, kind="ExternalInput")
    sclw_d = nc.dram_tensor("sclw", [128, 1], F32, kind="ExternalInput")
    sclh_d = nc.dram_tensor("sclh", [128, 1], F32, kind="ExternalInput")
    nsclw_d = nc.dram_tensor("nsclw", [128, 1], F32, kind="ExternalInput")
    nsclh_d = nc.dram_tensor("nsclh", [128, 1], F32, kind="ExternalInput")
    wlm1_d = nc.dram_tensor("wlm1", [128, 1], F32, kind="ExternalInput")
    hlm1_d = nc.dram_tensor("hlm1", [128, 1], F32, kind="ExternalInput")
    cbase_d = nc.dram_tensor("cbase", [128, 1], F32, kind="ExternalInput")
    qmask_d = nc.dram_tensor("qmask", [128, QP], BF16, kind="ExternalInput")
    iden_d = nc.dram_tensor("iden", [128, 128], BF16, kind="ExternalInput")
    out_d = nc.dram_tensor("out", [8, 3, 128, C], F32,
                           kind="ExternalOutput")
    vproj_d = nc.dram_tensor("vproj", [ES + 128, C], BF16)
    maps_d = nc.dram_tensor("mapsall", [NUM_HEADS, ES, 128], BF16)
    w4a_d = nc.dram_tensor("w4ad", [128, 2 * HCH], BF16)

    with tile.TileContext(nc) as tc:
        with tc.tile_pool(name="const", bufs=1) as cpool:
            selx = cpool.tile([2, 128], F32)
            sely = cpool.tile([2, 128], F32)
            onesq = cpool.tile([1, QP], BF16)
            onesbf = cpool.tile([1, 128], BF16)
            r128 = cpool.tile([128, 32], BF16)
            s16 = cpool.tile([128, 8], BF16)
            b8c = cpool.tile([8, 128], BF16)
            sclw = cpool.tile([128, 1], F32)
            sclh = cpool.tile([128, 1], F32)
            nsclw = cpool.tile([128, 1], F32)
            nsclh = cpool.tile([128, 1], F32)
            wlm1 = cpool.tile([128, 1], F32)
            hlm1 = cpool.tile([128, 1], F32)
            cbase = cpool.tile([128, 1], F32)
            woffx = cpool.tile([128, 2, 128], F32)
            woffy = cpool.tile([128, 2, 128], F32)
            wattn = cpool.tile([128, 2, 128], F32)
            boffx = cpool.tile([1, 128], BF16)
            boffy = cpool.tile([1, 128], BF16)
            battn = cpool.tile([1, 128], BF16)
            wvalb = cpool.tile([128, 2, C], BF16)
            bvalb = cpool.tile([1, C], BF16)
            cmask = cpool.tile([1, NVPAD], BF16)
            woutc = cpool.tile([128, 8, C], BF16)
            qmask = cpool.tile([128, QP], BF16)
            iden = cpool.tile([128, 128], BF16)
            qT = cpool.tile([128, 2, QP], F32)
            refT = cpool.tile([2, QP], F32)
            # big loads split across sync/scalar
            nc.sync.dma_start(qT[:, 0], qT_d[:, 0])
            nc.scalar.dma_start(qT[:, 1], qT_d[:, 1])
            nc.sync.dma_start(refT[:], refT_d[:])
            for t, s in [(woffx, woffx_d), (woffy, woffy_d),
                         (wattn, wattn_d), (wvalb, wval_d)]:
                nc.scalar.dma_start(
                    t[:], s[:].rearrange("(h p) x -> p h x", p=128))
            for t, s in [(onesq, onesq_d), (boffx, boffx_d),
                         (boffy, boffy_d), (selx, selx_d),
                         (sely, sely_d), (sclw, sclw_d), (sclh, sclh_d),
                         (nsclw, nsclw_d), (nsclh, nsclh_d),
                         (wlm1, wlm1_d), (hlm1, hlm1_d),
                         (cbase, cbase_d), (iden, iden_d),
                         (qmask, qmask_d)]:
                nc.sync.dma_start(t[:], s[:])
            for t, s in [(battn, battn_d), (bvalb, bval_d),
                         (cmask, cmask_d), (onesbf, onesbf_d),
                         (s16, s16_d), (b8c, b8_d),
                         (r128, r128_d), (woutc, wout_d)]:
                nc.scalar.dma_start(t[:], s[:])

            ps3 = tc.alloc_tile_pool(name="ps3", bufs=2, space="PSUM")

            def projf(hq, wof, bof, sel):
                ps = ps3.tile([128, HQ], F32, tag="pp")
                for c0 in range(0, HQ, 512):
                    cs = slice(hq * HQ + c0, hq * HQ + c0 + 512)
                    po = ps[:, c0:c0 + 512]
                    for half in range(2):
                        nc.tensor.matmul(
                            po, wof[:, half, :], qT[:, half, cs],
                            start=(half == 0), stop=False)
                    nc.tensor.matmul(po, bof[:], onesq[:, cs],
                                     start=False, stop=(sel is None))
                    if sel is not None:
                        nc.tensor.matmul(po, sel[:], refT[:, cs],
                                         start=False, stop=True)
                return ps

            # hq0 x/y projections run on PE before anything else
            psX0 = projf(0, woffx, boffx, selx)
            psY0 = projf(0, woffy, boffy, sely)

            # ======== phase 2 prologue: vT load + staging alloc ==========
            pVS = tc.alloc_tile_pool(name="pVS", bufs=1, side="right")
            zt = pVS.tile([128, C], BF16)
            nc.scalar.memzero(zt[:])
            # zero-fill vproj_d gap/pad rows straight from zt
            gaps = [(0, GAP)]
            for lvl in range(NUM_LEVELS):
                ge = CELLSTART[lvl] + LEVELS[lvl][0] * LEVELS[lvl][1]
                gn = CELLSTART[lvl + 1] if lvl < 3 else ES + 128
                gaps.append((ge, gn - ge))
            gz = 0
            for (g0, n) in gaps:
                while n > 0:
                    cnt = min(n, 128)
                    eng = nc.sync if gz % 2 == 0 else nc.scalar
                    gz += 1
                    eng.dma_start(
                        mkap(vproj_d[:], [[C, cnt], [1, C]],
                             offset=g0 * C),
                        mkap(zt[:], [[C, cnt], [1, C]]))
                    g0 += cnt
                    n -= cnt
            pVT = tc.alloc_tile_pool(name="pVT", bufs=1, side="right")
            vT = pVT.tile([128, 2, NVPAD], BF16)
            nc.sync.dma_start(vT[:, 0], vT_d[:, 0])
            nc.scalar.dma_start(vT[:, 1], vT_d[:, 1])
            ncp = [0]
            # ======== phase 3: positions / weights / indices ==========
            pw = tc.alloc_tile_pool(name="pw", bufs=1)
            idxw = pw.tile([128, IW], I16)
            # executor reads idx partitions [:16] but asserts all 128 are
            # in-range: zero the tile once (copies then fill rows 0..15)
            nc.scalar.memzero(idxw[:])
            w4p = [pw.tile([128, QP], BF16, name=f"w4p{s}")
                   for s in range(4)]
            p3 = tc.alloc_tile_pool(name="p3", bufs=1)
            ps3s = tc.alloc_tile_pool(name="ps3s", bufs=1, space="PSUM")
            psQp = tc.alloc_tile_pool(name="psQ", bufs=1, space="PSUM")
            p1t = tc.alloc_tile_pool(name="p1t", bufs=2)
            ps1 = tc.alloc_tile_pool(name="ps1", bufs=2, space="PSUM")

            def ph3(hq):
                    qs = slice(hq * HQ, (hq + 1) * HQ)

                    def corner_weights(psP, sclp, nsclp, wm1, pref):
                        rr = p3.tile([128, HQ], F32, tag="cwr")
                        xq = p3.tile([128, HQ], F32, tag=f"{pref}xq")
                        xi = p3.tile([128, HQ], I32, tag="cwxi")
                        x0p = p3.tile([128, HQ], F32, tag=f"{pref}x0p")
                        wx = p3.tile([128, HQ], F32, tag=f"{pref}wx")
                        v0 = p3.tile([128, HQ], BF16, tag="cwv0")
                        v1 = p3.tile([128, HQ], BF16, tag="cwv1")
                        wxf = p3.tile([128, HQ], BF16, tag="cwwxf")
                        a0 = p3.tile([128, HQ], BF16, tag=f"{pref}a0")
                        a1 = p3.tile([128, HQ], BF16, tag=f"{pref}a1")
                        # rr = max(1-p, 0); xq = max(scl - scl*rr, 0)
                        #   = scl*clamp01(p)
                        nc.scalar.activation(rr[:], psP[:], ACTF.Relu,
                                             bias=1.0, scale=-1.0)
                        nc.scalar.activation(xq[:], rr[:], ACTF.Relu,
                                             bias=sclp[:], scale=nsclp[:])
                        # x0p = floor(xq + 0.5) (trunc ok: arg >= 0.5)
                        nc.vector.tensor_scalar(xi[:], xq[:], 0.5, 0.0,
                                                op0=ALU.add,
                                                op1=ALU.bypass)
                        nc.vector.tensor_copy(x0p[:], xi[:])
                        # rounding-mode-agnostic floor: subtract 1 where
                        # the int conversion rounded up
                        t1 = p3.tile([128, HQ], F32, tag="cwt1")
                        nc.vector.scalar_tensor_tensor(
                            t1[:], xq[:], 0.5, x0p[:],
                            op0=ALU.add, op1=ALU.is_lt)
                        nc.vector.tensor_tensor(x0p[:], x0p[:], t1[:],
                                                op=ALU.subtract)
                        nc.vector.scalar_tensor_tensor(
                            wx[:], xq[:], 0.5, x0p[:],
                            op0=ALU.add, op1=ALU.subtract)
                        nc.vector.tensor_scalar(
                            v0[:], x0p[:], 1.0, 0.0,
                            op0=ALU.is_ge, op1=ALU.bypass)
                        nc.vector.tensor_scalar(
                            v1[:], x0p[:], wm1[:], 0.0,
                            op0=ALU.is_le, op1=ALU.bypass)
                        nc.vector.tensor_tensor(a1[:], wx[:], v1[:],
                                                op=ALU.mult)
                        nc.vector.tensor_scalar(
                            wxf[:], wx[:], -1.0, 1.0,
                            op0=ALU.mult, op1=ALU.add)
                        nc.vector.tensor_tensor(a0[:], wxf[:], v0[:],
                                                op=ALU.mult)
                        return x0p, a0, a1

                    if hq == 0:
                        psX, psY = psX0, psY0
                    else:
                        psX = projf(hq, woffx, boffx, selx)
                        psY = projf(hq, woffy, boffy, sely)
                    x0p, ax0, ax1 = corner_weights(psX, sclw, nsclw,
                                                   wlm1, "x")
                    y0p, ay0, ay1 = corner_weights(psY, sclh, nsclh,
                                                   hlm1, "y")
                    idxf = p3.tile([128, HQ], F32, tag="cwxq2")
                    idxf16 = p3.tile([128, HQ], I16, tag=f"i16{hq}")
                    nc.vector.scalar_tensor_tensor(
                        idxf[:], y0p[:], sclw[:], x0p[:],
                        op0=ALU.mult, op1=ALU.add)
                    nc.vector.tensor_scalar(
                        idxf16[:], idxf[:], cbase[:], 0.0,
                        op0=ALU.add, op1=ALU.bypass)
                    # i16 transposes -> idxw 16-wrap
                    # col = ci*512 + hq*256 + m*64 + jj  (ci=h*4+a, m=pt)
                    for jj in range(HQ // 16):
                        psQ = psQp.tile([16, 128], BF16, tag="psQ")
                        nc.tensor.transpose(
                            psQ[:],
                            idxf16[:, jj * 16:(jj + 1) * 16].bitcast(BF16),
                            iden[:])
                        dst = mkap(idxw[:], [[IW, 16], [64, 4], [256, 32]],
                                   offset=hq * 8192 + jj)
                        src = mkap(psQ[:].bitcast(I16),
                                   [[128, 16], [32, 4], [1, 32]])
                        if jj % 4 < 3:
                            nc.vector.tensor_copy(dst, src)
                        else:
                            nc.scalar.activation(dst, src, ACTF.Copy)
                    # ---- attention weights ----
                    psZ = projf(hq, wattn, battn, None)
                    esb = p3.tile([128, HQ], BF16, tag="cwesb")
                    nc.scalar.activation(esb[:], psZ[:], ACTF.Exp,
                                         bias=0.0, scale=1.0)
                    rsbq = p3.tile([8, HQ], BF16, tag="cwrsbq")
                    for c0 in range(0, HQ, 512):
                        pss = ps3s.tile([8, 512], F32, tag="pss")
                        nc.tensor.matmul(pss[:], s16[:],
                                         esb[:, c0:c0 + 512],
                                         start=True, stop=True)
                        rsb = p3.tile([8, 512], F32, tag="cwrsb")
                        nc.vector.reciprocal(rsb[:], pss[:])
                        nc.vector.tensor_tensor(
                            rsbq[:, c0:c0 + 512], rsb[:],
                            qmask[0:8, hq * HQ + c0:hq * HQ + c0 + 512],
                            op=ALU.mult)
                    psr2 = ps3.tile([128, HQ], F32, tag="pp")
                    for c0 in range(0, HQ, 512):
                        nc.tensor.matmul(psr2[:, c0:c0 + 512], b8c[:],
                                         rsbq[:, c0:c0 + 512],
                                         start=True, stop=True)
                    aw = p3.tile([128, HQ], BF16, tag="cwaw")
                    nc.vector.tensor_tensor(aw[:], esb[:], psr2[:],
                                            op=ALU.mult)
                    nc.vector.tensor_tensor(ay0[:], ay0[:], aw[:],
                                            op=ALU.mult)
                    nc.vector.tensor_tensor(ay1[:], ay1[:], aw[:],
                                            op=ALU.mult)
                    for s, (ax, ay) in enumerate(
                            [(ax0, ay0), (ax1, ay0),
                             (ax0, ay1), (ax1, ay1)]):
                        nc.vector.tensor_tensor(
                            w4p[s][:, qs], ax[:], ay[:], op=ALU.mult)
                    # repack w4p -> w4a_d DRAM: rows (s,ci), cols
                    # hq*4096 + m*1024 + q_l
                    for s in range(4):
                        for m in range(4):
                            eng = nc.sync if (s * 4 + m) % 2 == 0 \
                                else nc.scalar
                            eng.dma_start(
                                mkap(w4a_d[:],
                                     [[2 * HCH, 32], [1, HQ]],
                                     offset=s * 32 * 2 * HCH
                                     + hq * HCH + m * HQ),
                                mkap(w4p[s][:], [[QP, 32], [1, HQ]],
                                     offset=32 * m * QP + hq * HQ))
                    # replicate this hq's idx cols 16 -> 128 partitions
                    for (r0, nr) in [(16, 16), (32, 32), (64, 64)]:
                        for half in range(2):
                            eng = nc.sync if half == 0 else nc.scalar
                            cs = hq * 8192 + half * 4096
                            eng.dma_start(
                                mkap(idxw[:], [[IW, nr], [1, 4096]],
                                     offset=r0 * IW + cs),
                                mkap(idxw[:], [[IW, nr], [1, 4096]],
                                     offset=(r0 - nr) * IW + cs))

            ph3(0)
            for lvl in range(NUM_LEVELS):
                hw = LEVELS[lvl][0] * LEVELS[lvl][1]
                shift = CELLSTART[lvl] - STARTS[lvl]  # mult of 128
                c0 = STARTS[lvl]
                while c0 < STARTS[lvl] + hw:
                    tbeg = align_down(c0)
                    cend = min(tbeg + 128, STARTS[lvl] + hw)
                    lo, hi = c0 - tbeg, cend - tbeg
                    psv = ps1.tile([128, C], F32, tag="psv")
                    for half in range(2):
                        nc.tensor.matmul(
                            psv[:], vT[:, half, tbeg:tbeg + 128],
                            wvalb[:, half, :], start=(half == 0),
                            stop=False)
                    nc.tensor.matmul(psv[:], cmask[:, tbeg:tbeg + 128],
                                     bvalb[:], start=False, stop=True)
                    sp = c0 + shift
                    assert sp % 128 == lo
                    vstage = p1t.tile([128, C], BF16, tag="vstage")
                    if ncp[0] % 2 == 0:
                        nc.scalar.copy(vstage[:], psv[:])
                    else:
                        nc.vector.tensor_copy(vstage[:], psv[:])
                    eng = nc.sync if ncp[0] % 2 == 0 else nc.scalar
                    ncp[0] += 1
                    eng.dma_start(
                        mkap(vproj_d[:], [[C, hi - lo], [1, C]],
                             offset=sp * C),
                        mkap(vstage[:], [[C, hi - lo], [1, C]],
                             offset=lo * C))
                    c0 = cend
            # maps: one DRAM->DRAM DMA per (lvl, corner); ranges cover every
            # row of maps_d so the finite-checker never sees uninit DRAM
            MB = [0] + [CELLSTART[l] - LEVELS[l][1] - 1 for l in (1, 2, 3)] \
                + [ES]
            for lvl, (H, W) in enumerate(LEVELS):
                g0 = MB[lvl]
                n = MB[lvl + 1] - MB[lvl]
                for s, dl in enumerate([0, 1, W, W + 1]):
                    eng = nc.sync if (lvl * 4 + s) % 2 == 0 else nc.scalar
                    eng.dma_start(
                        mkap(maps_d[:], [[128, n], [ES * 128, 8], [1, 32]],
                             offset=g0 * 128 + s * 32),
                        mkap(vproj_d[:], [[C, n], [32, 8], [1, 32]],
                             offset=(g0 + dl) * C))
            pVT.release()
            pVS.release()

            ph3(1)
            ps1.release()
            p1t.release()
            psQp.release()
            ps3s.release()
            p3.release()
            ps3.release()
            # ======== phase 4: gather / multiply / reduce =================
            with tc.tile_pool(name="p4t", bufs=2) as p4t, \
                 tc.tile_pool(name="pwc", bufs=2) as pwc, \
                 tc.tile_pool(name="psW", bufs=2, space="PSUM") as psWp, \
                 tc.tile_pool(name="psO", bufs=1, space="PSUM") as psO, \
                 tc.tile_pool(name="pOs", bufs=1) as pOs, \
                 tc.tile_pool(name="pg", bufs=5) as pg, \
                 tc.tile_pool(name="pfo", bufs=3) as pfo:
                osb = [pOs.tile([128, QP + 2048], BF16, name=f"osb{g}",
                                tag=f"osb{g}")
                       for g in range(2)]
                for g in range(2):
                    nc.scalar.memzero(osb[g][:])

                def phase5_head(m):
                    # reference reshape quirk: out row q column c takes
                    # O[m, qq, d] with u = m*7681 + qq = 8q + (c//32 slot),
                    # W_out row 32*((m+qq)%8) + d.  7681 % 8 == 1 makes the
                    # structure identical on every core (host assembles).
                    gsb = osb[m // 4]
                    grp = m % 4
                    dlt = 64 * (m % 2)
                    for j in range(3):
                        psF4 = psWp.tile([128, 1024], F32, tag="psW")
                        psF = psF4[:, 0:C]
                        for s in range(8):
                            q0 = -m - 8 * dlt + 1024 * j + s
                            col0 = 1024 + q0
                            assert 0 <= col0 and col0 + 8 * 127 < QP + 2048
                            lhsT = mkap(
                                gsb[:], [[QP + 2048, 32], [8, 128]],
                                offset=(grp * 32) * (QP + 2048) + col0)
                            kw = {}
                            if grp == 3:
                                kw["tile_position"] = (96, 0)
                            nc.tensor.matmul(
                                psF, lhsT,
                                woutc[grp * 32:grp * 32 + 32, s, :],
                                start=(s == 0), stop=(s == 7), **kw)
                        fo = pfo.tile([128, C], F32, tag="fo")
                        nc.scalar.copy(fo[:], psF)
                        eng = nc.sync if j % 2 == 0 else nc.scalar
                        eng.dma_start(out_d[m, j], fo[:])

                psT = {}
                for ck in range(NHCH):
                    hq, hh, a = ck // 32, (ck % 32) // 4, ck % 4
                    ci = hh * 4 + a
                    grp = hh % 4
                    if grp == 0 and a == 0:
                        psT[hh // 4] = psO.tile(
                            [128, HQ], F32, name=f"psO4{hh // 4}",
                            tag=f"psO4{hh // 4}")
                    psO4 = psT[hh // 4]
                    i0 = hq * 8192 + ci * 256
                    g = pg.tile([128, 1, HCH], BF16, tag="g")
                    nc.gpsimd.dma_gather(
                        g[:], maps_d[hh],
                        idxw[:, i0:i0 + 256],
                        HCH, HCH, 128,
                        transpose=True, single_packet=False)
                    # replicate weights: wc[s*32+ch, :] = w4a_d[s*32+ci,
                    # hq*4096 + :]
                    wc = pwc.tile([128, HCH], BF16, tag="wc")
                    for half in range(2):
                        eng = nc.sync if half == 0 else nc.scalar
                        eng.dma_start(
                            mkap(wc[:], [[HCH, 128], [1, HCH // 2]],
                                 offset=half * (HCH // 2)),
                            mkap(w4a_d[:],
                                 [[32 * 2 * HCH, 4], [0, 32],
                                  [1, HCH // 2]],
                                 offset=ci * 2 * HCH + hq * HCH
                                 + half * (HCH // 2)))
                    tt = p4t.tile([128, HCH], BF16, tag="tt")
                    nc.vector.tensor_tensor(tt[:], g[:, 0, :], wc[:],
                                            op=ALU.mult)
                    for m4 in range(4):
                        for j2 in range(2):
                            kw = {}
                            if grp == 3:
                                kw["tile_position"] = (0, 96)
                            cs = slice(j2 * 512, (j2 + 1) * 512)
                            nc.tensor.matmul(
                                psO4[grp * 32:(grp + 1) * 32, cs],
                                r128[:],
                                tt[:, m4 * 1024 + j2 * 512:
                                   m4 * 1024 + (j2 + 1) * 512],
                                start=(a == 0 and m4 == 0),
                                stop=(a == 3 and m4 == 3), **kw)
                    if a == 3:
                        nc.scalar.activation(
                            osb[hh // 4][grp * 32:(grp + 1) * 32,
                                         1024 + hq * HQ:
                                         1024 + (hq + 1) * HQ],
                            psO4[grp * 32:(grp + 1) * 32, :], ACTF.Copy)
                    if ck >= 37 and (ck - 37) % 4 == 0 and ck <= 61:
                        phase5_head((ck - 37) // 4)

                # ======== phase 5 tail ====================================
                phase5_head(7)
            pw.release()
    nc.compile()
    return nc


# ---------------------------------------------------------------- host side
_CACHE = {}


def _consts(W_off, b_off, W_attn, b_attn, W_val, b_val, W_out, b_out):
    M = NUM_HEADS
    # partition layout c = b*32 + h*4 + a  (old: h*16 + a*4 + b)
    woff = np.asarray(W_off, np.float32).reshape(C, M, 4, 4, 2)
    woff = np.transpose(woff, (0, 3, 1, 2, 4))          # (C, b, h, a, 2)
    wattn = np.asarray(W_attn, np.float32).reshape(C, M, 4, 4)
    # partition (b, h, a) holds attention logit (level=b, point=a) so that
    # sample (h, a, b) pairs with aw(level=b, point=a)  (reference quirk)
    wattn = np.transpose(wattn, (0, 2, 1, 3))           # (C, l, h, k)
    boff = np.asarray(b_off, np.float32).reshape(M, 4, 4, 2)
    boff = np.transpose(boff, (2, 0, 1, 3))             # (b, h, a, 2)
    battn = np.asarray(b_attn, np.float32).reshape(M, 4, 4)
    battn = np.transpose(battn, (1, 0, 2))              # (l, h, k)
    cm = {}
    cm["woffx"] = np.ascontiguousarray(woff[..., 0].reshape(C, 128))
    cm["woffy"] = np.ascontiguousarray(woff[..., 1].reshape(C, 128))
    cm["wattn"] = np.ascontiguousarray(wattn.reshape(C, 128))
    cm["boffx"] = np.ascontiguousarray(boff[..., 0].reshape(1, 128)).astype(BF)
    cm["boffy"] = np.ascontiguousarray(boff[..., 1].reshape(1, 128)).astype(BF)
    cm["battn"] = np.ascontiguousarray(battn.reshape(1, 128)).astype(BF)
    cm["wval"] = np.asarray(W_val, np.float32).astype(BF)
    cm["bval"] = np.asarray(b_val, np.float32).reshape(1, C).astype(BF)
    wof = np.asarray(W_out, np.float32).reshape(8, 32, C).transpose(1, 0, 2)
    cm["wout"] = np.ascontiguousarray(
        np.broadcast_to(wof[None], (4, 32, 8, C)).reshape(128, 8, C)
    ).astype(BF)
    sel = np.zeros((2, 128), np.float32)
    sel[0] = 1.0
    cm["selx"] = sel
    cm["sely"] = sel[::-1].copy()
    cm["onesq"] = np.ones((1, QP), np.float32).astype(BF)
    cm["onesbf"] = np.ones((1, 128), np.float32).astype(BF)
    cmk = np.zeros((1, NVPAD), np.float32)
    cmk[0, :NV] = 1.0
    cm["cmask"] = cmk.astype(BF)
    r = np.zeros((128, 32), np.float32)
    for p in range(128):
        r[p, p % 32] = 1.0
    cm["r128"] = r.astype(BF)
    s16 = np.zeros((128, 8), np.float32)
    b8 = np.zeros((8, 128), np.float32)
    for p in range(128):
        h = (p % 32) // 4
        s16[p, h] = 1.0
        b8[h, p] = 1.0
    cm["s16"] = s16.astype(BF)
    cm["b8"] = b8.astype(BF)
    lvl_of_p = np.arange(128) % 4                       # level = a = c%4
    Wl = np.array([LEVELS[l][1] for l in lvl_of_p], np.float32)
    Hl = np.array([LEVELS[l][0] for l in lvl_of_p], np.float32)
    cb = np.array([CELLSTART[l] - LEVELS[l][1] - 1 for l in lvl_of_p],
                  np.float32)
    cm["iden"] = np.eye(128, dtype=np.float32).astype(BF)
    cm["sclw"] = Wl.reshape(128, 1)
    cm["sclh"] = Hl.reshape(128, 1)
    cm["nsclw"] = (-Wl).reshape(128, 1)
    cm["nsclh"] = (-Hl).reshape(128, 1)
    cm["wlm1"] = (Wl - 1).reshape(128, 1)
    cm["hlm1"] = (Hl - 1).reshape(128, 1)
    cm["cbase"] = cb.reshape(128, 1)
    return cm


def kernel(**inputs):
    if "nc" not in _CACHE:
        _CACHE["nc"] = build_nc()
    nc = _CACHE["nc"]
    cm = _consts(inputs["W_off"], inputs["b_off"], inputs["W_attn"],
                 inputs["b_attn"], inputs["W_val"], inputs["b_val"],
                 inputs["W_out"], inputs["b_out"])
    query = np.asarray(inputs["query"], np.float32)
    refp = np.asarray(inputs["reference_points"], np.float32)
    value = np.asarray(inputs["value"], np.float32)
    vpad = np.zeros((BS, NVPAD, C), np.float32)
    vpad[:, :NV] = value
    qpad = np.zeros((BS, 4 * QP, C), np.float32)
    qpad[:, :NQ] = query
    rpad = np.zeros((BS, 4 * QP, 2), np.float32)
    rpad[:, :NQ] = refp
    vT_b = []
    for b in range(BS):
        vT = vpad[b].T.reshape(2, 128, NVPAD).transpose(1, 0, 2)
        vT_b.append(np.ascontiguousarray(vT.astype(BF)))
    in_maps = []
    for core in range(NCORES):
        b, qc = core // 4, core % 4
        nvalid = min(QP, max(0, NQ - qc * QP))
        qm = np.zeros((128, QP), np.float32)
        qm[:, :nvalid] = 1.0
        qm = qm.astype(BF)
        qs = qpad[b, qc * QP:(qc + 1) * QP]
        rs = rpad[b, qc * QP:(qc + 1) * QP]
        qT = qs.T.reshape(2, 128, QP).transpose(1, 0, 2)
        m = {"qT": np.ascontiguousarray(qT),
             "refT": np.ascontiguousarray(rs.T),
             "vT": vT_b[b],
             "qmask": qm}
        m.update({k: np.ascontiguousarray(v) for k, v in cm.items()})
        in_maps.append(m)
    res = run_bass_kernel_spmd(nc, in_maps, list(range(NCORES)),
                               **_CACHE.get("run_kw", {}))
    _CACHE["last_res"] = res
    out = np.zeros((BS, NQ + 512, C), np.float32)
    for core in range(NCORES):
        b, qc = core // 4, core % 4
        slab = res.results[core]["out"]        # [8, 3, 128, 256]
        for m in range(NUM_HEADS):
            dlt = 64 * (m % 2)
            tb = 960 * m + 256 * qc - dlt      # absolute tile base
            for j in range(3):
                if m % 2 == 0:
                    row_lo, row_hi = 0, (128, 128, 32)[j]
                else:
                    row_lo, row_hi = ((64, 0, 0)[j], (128, 128, 96)[j])
                r0 = tb + 128 * j + row_lo
                r1 = tb + 128 * j + row_hi
                r1c = min(r1, NQ + 512)
                if r0 < 0 or r1c <= r0:
                    continue
                out[b, r0:r1c] += slab[m, j, row_lo:row_lo + (r1c - r0)]
    out = out[:, :NQ] + np.asarray(inputs["b_out"], np.float32)[None, None]
    return out
